# revision 19
# baseline (speedup 1.0000x reference)
"""Trainium2 Bass kernel for nn_AttnMoveModel (dense_transformer).

Strategy (8 NeuronCores):
  - Only the `curr` path of the reference affects the output (hist self-attn and
    cross-attn results are dead), so only that path is computed.
  - Attention is data-parallel over batch (4 of 32 batches per core).
  - The vocab projection (gathered @ emb[2:].T) is tensor-parallel, column-split
    over the vocab (5120 padded columns per core), with an AllGather of the
    gathered activations before it and per-row-group AllGathers of exp-sums for
    the log_softmax denominator (so the subtract+writeout of row group i
    pipelines behind row group i+1's matmuls).
  - All matmul inputs are bf16 (rel err ~2e-3 vs 2e-2 budget): 1 cycle/row on
    the PE array for every shape and half the HBM traffic of fp32.
  - Attention computes S^T (keys on partitions) so the exp output IS P^T in
    SBUF: no P transposes / PSUM copies; softmax row sums come from free N=1
    matmuls against a ones vector; 1/rowsum is folded in post-AV.
  - The full candidate shard (bf16) is preloaded into SBUF during attention
    (ordered behind the gathers on the DMA engines), so the score phase runs
    back-to-back matmuls with no input DMA.
  - log(sum) is computed with a fast-log bit trick + one Newton step using Exp
    (err ~5e-4), so the kernel never touches the Ln activation table: the whole
    kernel uses one table (exp+tanh), avoiding 1.3us table swaps per use.
  - The score phase persists exp(sc-30) (the softmax numerators, bf16) instead
    of raw scores: GPSIMD cannot read PSUM, and this removes all PSUM->SBUF
    copies and subtracts. The epilogue recovers log-probs in one DVE op per
    chunk: out = bitcast_i16(p)*(ln2/128) + (K2 + 30 - lnS).

Host-side prep (inside kernel()): shard indices/batches, pre-transpose weights
and the emb vocab shard into bf16, build one-hot selection matrices from
mask_pos, positional-encoding table.
"""
import contextlib
import math
import sys

sys.path.insert(0, "/opt/trn_rl_repo")

import numpy as np
import ml_dtypes

import concourse.bass as bass
import concourse.mybir as mybir
import concourse.tile as tile
from concourse.tile import add_dep_helper
from concourse import bacc
from concourse.bass_utils import run_bass_kernel_spmd

FP32 = mybir.dt.float32
BF16 = mybir.dt.bfloat16
INT32 = mybir.dt.int32
INT16 = mybir.dt.int16
ACTF = mybir.ActivationFunctionType
ALU = mybir.AluOpType
NPBF = ml_dtypes.bfloat16

N_CORES = 8
B, S, D, H, DH = 32, 128, 512, 8, 64
B_LOC = B // N_CORES              # 4 batches per core
NM = 16                           # mask positions per batch
I_LOC = B_LOC * NM                # 64 gathered rows per core
I_TOT = B * NM                    # 512 gathered rows total
GRID = 40000
VOCAB = GRID - 2                  # 39998 candidate rows
VSH = 5120                        # padded vocab shard per core (8*5120 >= VOCAB)
VCH = 512                         # vocab chunk (matmul N)
NCH = VSH // VCH                  # 10 chunks
KD = D // 128                     # 4 contraction tiles
SH_ATT = 15.0                     # exp shift for attention softmax
SH_SC = 30.0                      # exp shift for final log_softmax
# fast-log: ln(x) ~= bitcast_i32(x)*K1 + K2, |err| <= 0.030; one Newton step
# with exp brings it to ~5e-4
FL_K1 = math.log(2.0) / (1 << 23)
FL_K2 = -(127.0 - 0.0430) * math.log(2.0)
FL_K1B = math.log(2.0) / 128          # bf16 variant (bits in the high 16)

# bf16 const blob layout (columns)
C_PE = 0                          # peT [128, KD*S]    (kd, s)
C_SEL = C_PE + KD * S             # sel [128, B_LOC*NM] (b, m); partition = s
C_ONE = C_SEL + B_LOC * NM        # ones [128, 1]
C_BVB = C_ONE + 1                 # bv broadcast [128, D]
C_ID = C_BVB + D                  # identity [128, 128] for PE transposes
C16 = C_ID + 128
# fp32 const blob layout (columns)
F_BQ = 0                          # bq [128, KD]
F_BK = F_BQ + KD
F_T2B = F_BK + KD
F_CORR = F_T2B + KD               # padding correction [128, 1]
F32 = F_CORR + 1


def _positional_embedding(d_model, max_len):
    pe = np.zeros((max_len, d_model), dtype=np.float32)
    position = np.arange(max_len, dtype=np.float32)[:, None]
    div_term = np.exp(np.arange(0, d_model, 2, dtype=np.float32) * -(math.log(10000.0) / d_model))
    pe[:, 0::2] = np.sin(position * div_term)
    pe[:, 1::2] = np.cos(position * div_term)
    return pe


def build(sim_local=False):
    nc = bacc.Bacc("TRN2", target_bir_lowering=False, debug=False, num_devices=N_CORES)

    # ---- I/O ----
    embb = nc.dram_tensor("embb", [GRID, D], BF16, kind="ExternalInput")
    candT = nc.dram_tensor("candT", [D, VSH], BF16, kind="ExternalInput")
    idx = nc.dram_tensor("idx", [B_LOC * S], INT32, kind="ExternalInput")
    w4 = nc.dram_tensor("w4", [4, D, D], BF16, kind="ExternalInput")  # wqt wkt wvt t2wt
    cst16 = nc.dram_tensor("cst16", [128, C16], BF16, kind="ExternalInput")
    cst32 = nc.dram_tensor("cst32", [128, F32], FP32, kind="ExternalInput")
    out = nc.dram_tensor("out", [I_TOT, VSH], BF16, kind="ExternalOutput")

    with tile.TileContext(nc) as tc:
        with (
            tc.tile_pool(name="const", bufs=1) as constp,
            tc.tile_pool(name="persist", bufs=1) as persp,
            tc.tile_pool(name="small", bufs=2) as smallp,
            tc.tile_pool(name="dram", bufs=1, space="DRAM") as dramp,
        ):
            # ================= constant loads (order matters on the DMA dev) ====
            idx_sb = constp.tile([S, B_LOC], INT32)
            nc.sync.dma_start(out=idx_sb[:, :],
                              in_=idx.ap().rearrange("(b s) -> s b", s=S))
            c16_sb = constp.tile([128, C16], BF16)
            nc.sync.dma_start(out=c16_sb[:, :], in_=cst16.ap())
            w4_sb = constp.tile([128, 4, KD, D], BF16)  # [d%128, which, kd, j]
            w4v = w4.ap().rearrange("w (kd p) j -> p w kd j", p=128)
            for w in range(2):  # wq, wk first (attention critical path)
                nc.sync.dma_start(out=w4_sb[:, w, :, :], in_=w4v[:, w, :, :])
            c32_sb = constp.tile([128, F32], FP32)
            nc.sync.dma_start(out=c32_sb[:, :], in_=cst32.ap())
            peT_sb = c16_sb[:, C_PE:C_SEL].rearrange("p (kd s) -> p kd s", kd=KD)
            sel_sb = c16_sb[:, C_SEL:C_ONE].rearrange("p (b m) -> p b m", b=B_LOC)
            ones_sb = c16_sb[:, C_ONE:C_ONE + 1]
            bvb_sb = c16_sb[:, C_BVB:C_BVB + D]

            shatt_sb = constp.tile([128, 1], FP32)
            nc.vector.memset(shatt_sb[:, :], -SH_ATT)
            shsc_sb = constp.tile([128, 1], FP32)
            nc.vector.memset(shsc_sb[:, :], -SH_SC)
            cm1_sb = constp.tile([128, 1], FP32)
            nc.vector.memset(cm1_sb[:, :], -1.0)

            # persistent across phases
            GT_sb = persp.tile([128, KD, I_TOT], BF16)    # [d%128, kd, i]
            candT_sb = persp.tile([128, KD, VSH], BF16)   # full candidate shard
            sums_sb = persp.tile([128, KD, NCH], FP32)    # per-chunk exp sums
            pexp_sb = persp.tile([128, KD, VSH], BF16)    # exp(sc-30) numerators
            lnS_sb = persp.tile([128, KD], FP32)

            ag_g_in = dramp.tile([D, I_LOC], BF16)
            ag_g_out = dramp.tile([N_CORES * D, I_LOC], BF16, addr_space="Shared")
            ag_s_in = [dramp.tile([128, 1], FP32, name=f"ag_s_in{m}")
                       for m in range(KD + 1)]
            ag_s_out = [dramp.tile([N_CORES * 128, 1], FP32, addr_space="Shared",
                                   name=f"ag_s_out{m}")
                        for m in range(KD + 1)]

            # ================= Phase A: gather + self-attention =================
            with (
                tc.tile_pool(name="acts", bufs=1) as actsp,
                tc.tile_pool(name="gath", bufs=1) as gathp,
                tc.tile_pool(name="ph", bufs=8) as php,
                tc.tile_pool(name="ps_proj", bufs=2, space="PSUM") as ps_proj,
                tc.tile_pool(name="ps_st", bufs=3, space="PSUM") as ps_st,
                tc.tile_pool(name="ps_rs", bufs=1, space="PSUM") as ps_rs,
                tc.tile_pool(name="ps_av", bufs=2, space="PSUM") as ps_av,
            ):
                # per-batch indirect gathers (multi-column offset APs gather
                # with a different layout than assumed — verified broken on HW)
                with tc.high_priority():
                    g_all = gathp.tile([S, B_LOC, D], BF16, tag="gather")
                    for b in range(B_LOC):
                        gi = nc.gpsimd.indirect_dma_start(
                            out=g_all[:, b, :], out_offset=None,
                            in_=embb.ap(),
                            in_offset=bass.IndirectOffsetOnAxis(ap=idx_sb[:, b:b + 1], axis=0),
                        )

                # wv/t2w and the candidate shard stream behind the gather on
                # the serialized DMA device (they are needed later)
                for w in range(2, 4):
                    wd = nc.sync.dma_start(out=w4_sb[:, w, :, :], in_=w4v[:, w, :, :])
                    add_dep_helper(wd.ins, gi.ins,
                                   reason="wv/t2w stream behind the emb gather")
                cv = candT.ap().rearrange("(kd p) n -> p kd n", p=128)
                HV = VSH // 2
                for hh in range(2):
                    cd = nc.sync.dma_start(
                        out=candT_sb[:, :, hh * HV:(hh + 1) * HV],
                        in_=cv[:, :, hh * HV:(hh + 1) * HV])
                    add_dep_helper(cd.ins, gi.ins,
                                   reason="candT streams behind the emb gather")

                # currT[d%128, kd, (b s)] = transpose(gather) + peT, in bf16
                currT_sb = actsp.tile([128, KD, B_LOC * S], BF16)
                for b in range(B_LOC):
                    tp_ps = ps_st.tile([128, KD, 128], BF16, tag="st")
                    for kd in range(KD):
                        nc.tensor.transpose(tp_ps[:, kd, :],
                                            g_all[:, b, kd * 128:(kd + 1) * 128],
                                            c16_sb[:, C_ID:C_ID + 128])
                    nc.vector.tensor_add(
                        out=currT_sb[:, :, b * S:(b + 1) * S],
                        in0=tp_ps[:, :, :],
                        in1=peT_sb[:, :, :],
                    )

                # projections: QT/KT [j%128, kj, (b,s)] bf16 with bias
                QT_sb = actsp.tile([128, KD, B_LOC * S], BF16)
                KT_sb = actsp.tile([128, KD, B_LOC * S], BF16)
                for kj in range(KD):
                    q_ps = ps_proj.tile([128, B_LOC * S], FP32, tag="big")
                    for kd in range(KD):
                        nc.tensor.matmul(q_ps[:, :],
                                         w4_sb[:, 0, kd, kj * 128:(kj + 1) * 128],
                                         currT_sb[:, kd, :],
                                         start=(kd == 0), stop=(kd == KD - 1))
                    nc.vector.tensor_scalar_add(QT_sb[:, kj, :], q_ps[:, :],
                                                c32_sb[:, F_BQ + kj:F_BQ + kj + 1])
                    k_ps = ps_proj.tile([128, B_LOC * S], FP32, tag="big")
                    for kd in range(KD):
                        nc.tensor.matmul(k_ps[:, :],
                                         w4_sb[:, 1, kd, kj * 128:(kj + 1) * 128],
                                         currT_sb[:, kd, :],
                                         start=(kd == 0), stop=(kd == KD - 1))
                    nc.vector.tensor_scalar_add(KT_sb[:, kj, :], k_ps[:, :],
                                                c32_sb[:, F_BK + kj:F_BK + kj + 1])
                # attention: S^T = K_h-stationary x Q_h -> [k, q] per head;
                # exp -> P^T bf16 in SBUF (all 8 exps issue back-to-back on Act)
                th_sb = actsp.tile([128, B_LOC, D], BF16)  # tanh(attn) [s, b, j]
                V_sb = actsp.tile([128, B_LOC, D], BF16)
                p_tiles = {}
                for b in range(B_LOC):
                    for half in range(2):
                        st_ps = ps_st.tile([128, 4 * S], FP32, tag="st")
                        for hh in range(4):  # head = hh*2 + half
                            qs = QT_sb[half * 64:(half + 1) * 64, hh, b * S:(b + 1) * S]
                            ks = KT_sb[half * 64:(half + 1) * 64, hh, b * S:(b + 1) * S]
                            nc.tensor.matmul(st_ps[:, hh * S:(hh + 1) * S], ks, qs,
                                             start=True, stop=True)
                        p_sb = php.tile([128, 4 * S], BF16, tag="p")
                        nc.scalar.activation(p_sb[:, :], st_ps[:, :], ACTF.Exp,
                                             bias=shatt_sb[:, :1])
                        p_tiles[(b, half)] = p_sb
                # per batch: V projection (overlaps the exps on Act), rowsums via
                # N=1 matmuls, AV, per-head 1/rowsum rescale, tanh
                for b in range(B_LOC):
                    v_ps = ps_proj.tile([128, D], FP32, tag="big")
                    for kd in range(KD):
                        nc.tensor.matmul(v_ps[:, :],
                                         currT_sb[:, kd, b * S:(b + 1) * S],
                                         w4_sb[:, 2, kd, :],
                                         start=(kd == 0), stop=(kd == KD - 1))
                    nc.vector.tensor_add(out=V_sb[:, b, :], in0=v_ps[:, :], in1=bvb_sb[:, :])
                    rs_ps = ps_rs.tile([128, H], FP32, tag="rs")
                    av_ps = ps_av.tile([128, D], FP32, tag="av")
                    last_av = None
                    for half in range(2):
                        for hh in range(4):
                            h = hh * 2 + half
                            nc.tensor.matmul(rs_ps[:, h:h + 1],
                                             p_tiles[(b, half)][:, hh * S:(hh + 1) * S],
                                             ones_sb[:, :],
                                             start=True, stop=True)
                            last_av = nc.tensor.matmul(
                                av_ps[:, h * DH:(h + 1) * DH],
                                p_tiles[(b, half)][:, hh * S:(hh + 1) * S],
                                V_sb[:, b, h * DH:(h + 1) * DH],
                                start=True, stop=True)
                    rec_sb = smallp.tile([128, H], FP32, tag="rec")
                    nc.vector.reciprocal(rec_sb[:, :], rs_ps[:, :])
                    # 1/rowsum rescale as one broadcast mult (rec stride-0 over
                    # dh); the bank has 8 matmul writers and this is a full-bank
                    # read, so the dep helper pins the final drain
                    att_sb = php.tile([128, D], BF16, tag="att")
                    op = nc.vector.tensor_mul(
                        out=att_sb[:, :].rearrange("p (h x) -> p h x", h=H),
                        in0=av_ps[:, :].rearrange("p (h x) -> p h x", h=H),
                        in1=rec_sb[:, :].rearrange("p (h one) -> p h one", one=1)
                            .to_broadcast([128, H, DH]))
                    add_dep_helper(op.ins, last_av.ins,
                                   reason="att bank read after all AV writes")
                    nc.scalar.activation(th_sb[:, b, :], att_sb[:, :], ACTF.Tanh)

                # select mask positions (transposed): thselT [d%128, kd, i_loc] bf16
                thsel_sb = actsp.tile([128, KD, I_LOC], BF16)
                for kd in range(KD):
                    ts_ps = ps_st.tile([128, I_LOC], FP32, tag="st")
                    last_ts = None
                    for b in range(B_LOC):
                        last_ts = nc.tensor.matmul(ts_ps[:, b * NM:(b + 1) * NM],
                                                   th_sb[:, b, kd * 128:(kd + 1) * 128],
                                                   sel_sb[:, b, :],
                                                   start=True, stop=True)
                    op = nc.vector.tensor_copy(out=thsel_sb[:, kd, :], in_=ts_ps[:, :])
                    add_dep_helper(op.ins, last_ts.ins,
                                   reason="ts bank read after all sel writes")
                # t2 projection -> G_localT [d, i_loc] bf16 -> DRAM for AllGather
                gt_sb = actsp.tile([128, KD, I_LOC], BF16)
                for mj in range(KD):
                    g_ps = ps_proj.tile([128, I_LOC], FP32, tag="big")
                    for kd in range(KD):
                        nc.tensor.matmul(g_ps[:, :],
                                         w4_sb[:, 3, kd, mj * 128:(mj + 1) * 128],
                                         thsel_sb[:, kd, :],
                                         start=(kd == 0), stop=(kd == KD - 1))
                    nc.vector.tensor_scalar_add(gt_sb[:, mj, :], g_ps[:, :],
                                                c32_sb[:, F_T2B + mj:F_T2B + mj + 1])
                nc.sync.dma_start(out=ag_g_in[:, :].rearrange("(mj p) i -> p mj i", p=128),
                                  in_=gt_sb[:, :, :])

                # ---- AllGather G ----
                if sim_local:
                    nc.sync.dma_start(
                        out=ag_g_out[:, :].rearrange("(c d) i -> c d i", c=N_CORES),
                        in_=ag_g_in[:, :].rearrange("(one d) i -> one d i", one=1)
                            .to_broadcast([N_CORES, D, I_LOC]))
                else:
                    nc.gpsimd.collective_compute(
                        "AllGather", mybir.AluOpType.bypass,
                        replica_groups=[list(range(N_CORES))],
                        ins=[ag_g_in[:, :].opt()], outs=[ag_g_out[:, :].opt()],
                    )
                ag_g_view = ag_g_out[:, :].rearrange("(c kd p) i -> p kd c i", p=128, kd=KD)
                for kd in range(KD):
                    nc.sync.dma_start(
                        out=GT_sb[:, kd, :].rearrange("p (c i) -> p c i", c=N_CORES),
                        in_=ag_g_view[:, kd, :, :],
                    )

            # ================= Phase B: scores, exp, sums, sub, writeout =========
            # row-group-major: group mi's AllGather + subtract + output DMA overlap
            # groups mi+1..3's matmuls
            with (
                tc.tile_pool(name="ps_sc", bufs=8, space="PSUM") as ps_sc,
            ):
                def epilogue(mi):
                    # stot readback -> lnS (fast-log + 1 Newton step via Exp)
                    # -> subtract -> quarter writeout DMAs.
                    # Emitted AFTER group mi+1's exps/copies so the AllGather
                    # wait never head-of-line-blocks the in-order engine queues.
                    last = mi == KD - 1
                    nread = 2 * N_CORES if last else N_CORES
                    stot_sb = smallp.tile([128, 2 * N_CORES], FP32, tag="stot")
                    nc.sync.dma_start(
                        out=stot_sb[:, 0:N_CORES],
                        in_=ag_s_out[mi][:, 0].rearrange("(c p) -> p c", p=128))
                    if last:
                        nc.sync.dma_start(
                            out=stot_sb[:, N_CORES:],
                            in_=ag_s_out[KD][:, 0].rearrange("(c p) -> p c", p=128))
                    stl_sb = smallp.tile([128, 3], FP32, tag="stl")
                    nc.vector.reduce_sum(stl_sb[:, 0:1], stot_sb[:, 0:nread],
                                         axis=mybir.AxisListType.X)
                    # y0 = fast-log(S); lnS30 = y0 + S*exp(-y0) - 1 + SH_SC
                    nc.vector.tensor_scalar(
                        out=stl_sb[:, 1:2], in0=stl_sb[:, 0:1].bitcast(INT32),
                        scalar1=FL_K1, scalar2=FL_K2, op0=ALU.mult, op1=ALU.add)
                    ey_sb = smallp.tile([128, 1], FP32, tag="ey")
                    nc.scalar.activation(ey_sb[:, :], stl_sb[:, 1:2], ACTF.Exp,
                                         scale=cm1_sb[:, :1])
                    nc.vector.tensor_mul(out=stl_sb[:, 2:3], in0=stl_sb[:, 0:1],
                                          in1=ey_sb[:, :])
                    nc.vector.tensor_add(out=stl_sb[:, 2:3], in0=stl_sb[:, 2:3],
                                         in1=stl_sb[:, 1:2])
                    # cc = FL_K2 + SH_SC - lnS30  (lnS30 = y1 - 1 + SH_SC)
                    cc_sb = smallp.tile([128, 1], FP32, tag="cc")
                    nc.vector.tensor_scalar(
                        out=cc_sb[:, :], in0=stl_sb[:, 2:3],
                        scalar1=-1.0, scalar2=FL_K2 + 1.0, op0=ALU.mult, op1=ALU.add)
                    QV = VSH // 4
                    for v in range(NCH):
                        sl = pexp_sb[:, mi, v * VCH:(v + 1) * VCH]
                        nc.vector.tensor_scalar(
                            out=sl, in0=sl.bitcast(INT16),
                            scalar1=FL_K1B, scalar2=cc_sb[:, :1],
                            op0=ALU.mult, op1=ALU.add)
                    for qq in range(4):
                        nc.sync.dma_start(
                            out=out.ap()[mi * 128:(mi + 1) * 128,
                                         qq * QV:(qq + 1) * QV],
                            in_=pexp_sb[:, mi, qq * QV:(qq + 1) * QV],
                        )

                def launch_ag(slot, src_ap):
                    nc.sync.dma_start(out=ag_s_in[slot][:, :], in_=src_ap)
                    if sim_local:
                        nc.sync.dma_start(
                            out=ag_s_out[slot][:, :].rearrange("(c i) one -> c i one", c=N_CORES),
                            in_=ag_s_in[slot][:, :].rearrange("(one i) x -> one i x", one=1)
                                .to_broadcast([N_CORES, 128, 1]))
                    else:
                        nc.gpsimd.collective_compute(
                            "AllGather", mybir.AluOpType.bypass,
                            replica_groups=[list(range(N_CORES))],
                            ins=[ag_s_in[slot][:, :].opt()], outs=[ag_s_out[slot][:, :].opt()],
                        )

                for mi in range(KD):
                    last = mi == KD - 1
                    for v in range(NCH):
                        sc_ps = ps_sc.tile([128, VCH], FP32, tag="sc")
                        for kd in range(KD):
                            nc.tensor.matmul(sc_ps[:, :],
                                             GT_sb[:, kd, mi * 128:(mi + 1) * 128],
                                             candT_sb[:, kd, v * VCH:(v + 1) * VCH],
                                             start=(kd == 0), stop=(kd == KD - 1))
                        nc.scalar.activation(pexp_sb[:, mi, v * VCH:(v + 1) * VCH],
                                             sc_ps[:, :],
                                             ACTF.Exp, bias=shsc_sb[:, :1],
                                             accum_out=sums_sb[:, mi, v:v + 1])
                        if last and v == NCH - 2:
                            # last group: AllGather chunks 0..8 early (hides
                            # under chunk 9); chunk 9's sum goes in a second,
                            # concurrent AllGather right after its accum lands
                            sl_sb = smallp.tile([128, 1], FP32, tag="sl")
                            nc.vector.reduce_sum(sl_sb[:, :], sums_sb[:, mi, 0:NCH - 1],
                                                 axis=mybir.AxisListType.X)
                            nc.vector.tensor_sub(out=sl_sb[:, :], in0=sl_sb[:, :],
                                                 in1=c32_sb[:, F_CORR:F_CORR + 1])
                            launch_ag(mi, sl_sb[:, :])
                    if not last:
                        sl_sb = smallp.tile([128, 1], FP32, tag="sl")
                        nc.vector.reduce_sum(sl_sb[:, :], sums_sb[:, mi, :],
                                             axis=mybir.AxisListType.X)
                        nc.vector.tensor_sub(out=sl_sb[:, :], in0=sl_sb[:, :],
                                             in1=c32_sb[:, F_CORR:F_CORR + 1])
                        launch_ag(mi, sl_sb[:, :])
                        if mi >= 1:
                            epilogue(mi - 1)
                    else:
                        launch_ag(KD, sums_sb[:, mi, NCH - 1:NCH])
                        epilogue(mi - 1)
                epilogue(KD - 1)
    nc.compile()
    return nc


_NC_CACHE = None


def _get_nc():
    global _NC_CACHE
    if _NC_CACHE is None:
        _NC_CACHE = build()
    return _NC_CACHE


def prepare_in_maps(inputs):
    emb = np.asarray(inputs["emb"], dtype=np.float32)
    embb = np.ascontiguousarray(emb.astype(NPBF))
    mask_curr = np.asarray(inputs["mask_curr_traj_grid"]).astype(np.int32)
    mask_pos = np.asarray(inputs["mask_pos"]).astype(np.int32)
    w4 = np.stack([
        np.asarray(inputs["c_wq"], dtype=np.float32).T,
        np.asarray(inputs["c_wk"], dtype=np.float32).T,
        np.asarray(inputs["c_wv"], dtype=np.float32).T,
        np.asarray(inputs["t2_w"], dtype=np.float32).T,
    ]).astype(NPBF)
    bq = np.asarray(inputs["c_bq"], dtype=np.float32)
    bk = np.asarray(inputs["c_bk"], dtype=np.float32)
    bv = np.asarray(inputs["c_bv"], dtype=np.float32)
    t2b = np.asarray(inputs["t2_b"], dtype=np.float32)
    peT = _positional_embedding(D, S).T  # [D, S]

    candTb = np.ascontiguousarray(emb[2:].T.astype(NPBF))  # [D, VOCAB]

    # bf16 const blob
    c16 = np.zeros((128, C16), dtype=NPBF)
    c16[:, C_PE:C_SEL] = peT.reshape(KD, 128, S).transpose(1, 0, 2).reshape(128, KD * S)
    c16[:, C_ONE] = 1.0
    c16[:, C_BVB:C_BVB + D] = np.broadcast_to(bv, (128, D))
    c16[:, C_ID:C_ID + 128] = np.eye(128, dtype=NPBF)
    # fp32 const blob (core-independent part)
    c32 = np.zeros((128, F32), dtype=np.float32)
    c32[:, F_BQ:F_BQ + KD] = bq.reshape(KD, 128).T
    c32[:, F_BK:F_BK + KD] = bk.reshape(KD, 128).T
    c32[:, F_T2B:F_T2B + KD] = t2b.reshape(KD, 128).T

    in_maps = []
    for c in range(N_CORES):
        lo = c * VSH
        hi = min((c + 1) * VSH, VOCAB)
        shard = np.zeros((D, VSH), dtype=NPBF)
        shard[:, : hi - lo] = candTb[:, lo:hi]
        n_inv = VSH - (hi - lo)
        c32_c = c32.copy()
        c32_c[:, F_CORR] = n_inv * math.exp(-SH_SC)
        mp = mask_pos[c * B_LOC:(c + 1) * B_LOC]  # [B_LOC, NM]
        c16_c = c16.copy()
        sel_c = np.zeros((S, B_LOC, NM), dtype=NPBF)
        for b in range(B_LOC):
            sel_c[mp[b], b, np.arange(NM)] = 1.0
        c16_c[:, C_SEL:C_ONE] = sel_c.reshape(S, B_LOC * NM)
        in_maps.append(dict(
            embb=embb,
            candT=np.ascontiguousarray(shard),
            idx=np.ascontiguousarray(mask_curr[c * B_LOC:(c + 1) * B_LOC].reshape(-1)),
            w4=w4, cst16=c16_c, cst32=c32_c,
        ))
    return in_maps


def assemble_output(results):
    parts = []
    for c in range(N_CORES):
        lo = c * VSH
        hi = min((c + 1) * VSH, VOCAB)
        parts.append(results[c]["out"][:, : hi - lo].astype(np.float32))
    return np.ascontiguousarray(np.concatenate(parts, axis=1))


def kernel(**inputs):
    nc = _get_nc()
    in_maps = prepare_in_maps(inputs)
    res = run_bass_kernel_spmd(nc, in_maps, core_ids=list(range(N_CORES)))
    return assemble_output(res.results)


# revision 24
# speedup vs baseline: 1.0109x; 1.0109x over previous
"""Trainium2 Bass kernel for nn_AttnMoveModel (dense_transformer).

Strategy (8 NeuronCores):
  - Only the `curr` path of the reference affects the output (hist self-attn and
    cross-attn results are dead), so only that path is computed.
  - Attention is data-parallel over batch (4 of 32 batches per core).
  - The vocab projection (gathered @ emb[2:].T) is tensor-parallel, column-split
    over the vocab (5120 padded columns per core), with an AllGather of the
    gathered activations before it and per-row-group AllGathers of exp-sums for
    the log_softmax denominator (so the subtract+writeout of row group i
    pipelines behind row group i+1's matmuls).
  - All matmul inputs are bf16 (rel err ~2e-3 vs 2e-2 budget): 1 cycle/row on
    the PE array for every shape and half the HBM traffic of fp32.
  - Attention computes S^T (keys on partitions) so the exp output IS P^T in
    SBUF: no P transposes / PSUM copies; softmax row sums come from free N=1
    matmuls against a ones vector; 1/rowsum is folded in post-AV.
  - The full candidate shard (bf16) is preloaded into SBUF during attention
    (ordered behind the gathers on the DMA engines), so the score phase runs
    back-to-back matmuls with no input DMA.
  - log(sum) is computed with a fast-log bit trick + one Newton step using Exp
    (err ~5e-4), so the kernel never touches the Ln activation table: the whole
    kernel uses one table (exp+tanh), avoiding 1.3us table swaps per use.
  - The score phase persists exp(sc-30) (the softmax numerators, bf16) instead
    of raw scores: GPSIMD cannot read PSUM, and this removes all PSUM->SBUF
    copies and subtracts. The epilogue recovers log-probs in one DVE op per
    chunk: out = bitcast_i16(p)*(ln2/128) + (K2 + 30 - lnS).

Host-side prep (inside kernel()): shard indices/batches, pre-transpose weights
and the emb vocab shard into bf16, build one-hot selection matrices from
mask_pos, positional-encoding table.
"""
import contextlib
import math
import sys

sys.path.insert(0, "/opt/trn_rl_repo")

import numpy as np
import ml_dtypes

import concourse.bass as bass
import concourse.mybir as mybir
import concourse.tile as tile
from concourse.tile import add_dep_helper
from concourse import bacc
from concourse.bass_utils import run_bass_kernel_spmd

FP32 = mybir.dt.float32
BF16 = mybir.dt.bfloat16
INT32 = mybir.dt.int32
INT16 = mybir.dt.int16
ACTF = mybir.ActivationFunctionType
ALU = mybir.AluOpType
NPBF = ml_dtypes.bfloat16

N_CORES = 8
B, S, D, H, DH = 32, 128, 512, 8, 64
B_LOC = B // N_CORES              # 4 batches per core
NM = 16                           # mask positions per batch
I_LOC = B_LOC * NM                # 64 gathered rows per core
I_TOT = B * NM                    # 512 gathered rows total
GRID = 40000
VOCAB = GRID - 2                  # 39998 candidate rows
VSH = 5120                        # padded vocab shard per core (8*5120 >= VOCAB)
VCH = 512                         # vocab chunk (matmul N)
NCH = VSH // VCH                  # 10 chunks
KD = D // 128                     # 4 contraction tiles
SH_ATT = 15.0                     # exp shift for attention softmax
SH_SC = 30.0                      # exp shift for final log_softmax
# fast-log: ln(x) ~= bitcast_i32(x)*K1 + K2, |err| <= 0.030; one Newton step
# with exp brings it to ~5e-4
FL_K1 = math.log(2.0) / (1 << 23)
FL_K2 = -(127.0 - 0.0430) * math.log(2.0)
FL_K1B = math.log(2.0) / 128          # bf16 variant (bits in the high 16)

# bf16 const blob layout (columns)
C_PE = 0                          # peT [128, KD*S]    (kd, s)
C_SEL = C_PE + KD * S             # sel [128, B_LOC*NM] (b, m); partition = s
C_ONE = C_SEL + B_LOC * NM        # ones [128, 1]
C_BVB = C_ONE + 1                 # bv broadcast [128, D]
C_ID = C_BVB + D                  # identity [128, 128] for PE transposes
C16 = C_ID + 128
# fp32 const blob layout (columns)
F_BQ = 0                          # bq [128, KD]
F_BK = F_BQ + KD
F_T2B = F_BK + KD
F_CORR = F_T2B + KD               # padding correction [128, 1]
F32 = F_CORR + 1


def _positional_embedding(d_model, max_len):
    pe = np.zeros((max_len, d_model), dtype=np.float32)
    position = np.arange(max_len, dtype=np.float32)[:, None]
    div_term = np.exp(np.arange(0, d_model, 2, dtype=np.float32) * -(math.log(10000.0) / d_model))
    pe[:, 0::2] = np.sin(position * div_term)
    pe[:, 1::2] = np.cos(position * div_term)
    return pe


def build(sim_local=False):
    nc = bacc.Bacc("TRN2", target_bir_lowering=False, debug=False, num_devices=N_CORES)

    # ---- I/O ----
    embb = nc.dram_tensor("embb", [GRID, D], BF16, kind="ExternalInput")
    candT = nc.dram_tensor("candT", [D, VSH], BF16, kind="ExternalInput")
    idx = nc.dram_tensor("idx", [B_LOC * S], INT32, kind="ExternalInput")
    w4 = nc.dram_tensor("w4", [4, D, D], BF16, kind="ExternalInput")  # wqt wkt wvt t2wt
    cst16 = nc.dram_tensor("cst16", [128, C16], BF16, kind="ExternalInput")
    cst32 = nc.dram_tensor("cst32", [128, F32], FP32, kind="ExternalInput")
    out = nc.dram_tensor("out", [I_TOT, VSH], BF16, kind="ExternalOutput")

    with tile.TileContext(nc) as tc:
        with (
            tc.tile_pool(name="const", bufs=1) as constp,
            tc.tile_pool(name="persist", bufs=1) as persp,
            tc.tile_pool(name="small", bufs=2) as smallp,
            tc.tile_pool(name="dram", bufs=1, space="DRAM") as dramp,
        ):
            # ================= constant loads (order matters on the DMA dev) ====
            idx_sb = constp.tile([S, B_LOC], INT32)
            nc.sync.dma_start(out=idx_sb[:, :],
                              in_=idx.ap().rearrange("(b s) -> s b", s=S))
            c16_sb = constp.tile([128, C16], BF16)
            nc.sync.dma_start(out=c16_sb[:, :], in_=cst16.ap())
            w4_sb = constp.tile([128, 4, KD, D], BF16)  # [d%128, which, kd, j]
            w4v = w4.ap().rearrange("w (kd p) j -> p w kd j", p=128)
            for w in range(2):  # wq, wk first (attention critical path)
                nc.sync.dma_start(out=w4_sb[:, w, :, :], in_=w4v[:, w, :, :])
            c32_sb = constp.tile([128, F32], FP32)
            nc.sync.dma_start(out=c32_sb[:, :], in_=cst32.ap())
            peT_sb = c16_sb[:, C_PE:C_SEL].rearrange("p (kd s) -> p kd s", kd=KD)
            sel_sb = c16_sb[:, C_SEL:C_ONE].rearrange("p (b m) -> p b m", b=B_LOC)
            ones_sb = c16_sb[:, C_ONE:C_ONE + 1]
            bvb_sb = c16_sb[:, C_BVB:C_BVB + D]

            shatt_sb = constp.tile([128, 1], FP32)
            nc.vector.memset(shatt_sb[:, :], -SH_ATT)
            shsc_sb = constp.tile([128, 1], FP32)
            nc.vector.memset(shsc_sb[:, :], -SH_SC)
            cm1_sb = constp.tile([128, 1], FP32)
            nc.vector.memset(cm1_sb[:, :], -1.0)

            # persistent across phases
            GT_sb = persp.tile([128, KD, I_TOT], BF16)    # [d%128, kd, i]
            candT_sb = persp.tile([128, KD, VSH], BF16)   # full candidate shard
            sums_sb = persp.tile([128, KD, NCH], FP32)    # per-chunk exp sums
            pexp_sb = persp.tile([128, KD, VSH], BF16)    # exp(sc-30) numerators
            lnS_sb = persp.tile([128, KD], FP32)

            ag_g_in = dramp.tile([D, I_LOC], BF16)
            ag_g_out = dramp.tile([N_CORES * D, I_LOC], BF16, addr_space="Shared")
            ag_s_in = [dramp.tile([128, 1], FP32, name=f"ag_s_in{m}")
                       for m in range(KD + 1)]
            ag_s_out = [dramp.tile([N_CORES * 128, 1], FP32, addr_space="Shared",
                                   name=f"ag_s_out{m}")
                        for m in range(KD + 1)]

            # ================= Phase A: gather + self-attention =================
            with (
                tc.tile_pool(name="acts", bufs=1) as actsp,
                tc.tile_pool(name="gath", bufs=1) as gathp,
                tc.tile_pool(name="ph", bufs=8) as php,
                tc.tile_pool(name="ps_proj", bufs=2, space="PSUM") as ps_proj,
                tc.tile_pool(name="ps_st", bufs=3, space="PSUM") as ps_st,
                tc.tile_pool(name="ps_rs", bufs=1, space="PSUM") as ps_rs,
                tc.tile_pool(name="ps_av", bufs=2, space="PSUM") as ps_av,
            ):
                # per-batch indirect gathers (multi-column offset APs gather
                # with a different layout than assumed — verified broken on HW)
                with tc.high_priority():
                    g_all = gathp.tile([S, B_LOC, D], BF16, tag="gather")
                    for b in range(B_LOC):
                        gi = nc.gpsimd.indirect_dma_start(
                            out=g_all[:, b, :], out_offset=None,
                            in_=embb.ap(),
                            in_offset=bass.IndirectOffsetOnAxis(ap=idx_sb[:, b:b + 1], axis=0),
                        )

                # wv/t2w and the candidate shard stream behind the gather on
                # the serialized DMA device (they are needed later)
                for w in range(2, 4):
                    wd = nc.sync.dma_start(out=w4_sb[:, w, :, :], in_=w4v[:, w, :, :])
                    add_dep_helper(wd.ins, gi.ins,
                                   reason="wv/t2w stream behind the emb gather")
                cv = candT.ap().rearrange("(kd p) n -> p kd n", p=128)
                HV = VSH // 2
                for hh in range(2):
                    cd = nc.sync.dma_start(
                        out=candT_sb[:, :, hh * HV:(hh + 1) * HV],
                        in_=cv[:, :, hh * HV:(hh + 1) * HV])
                    add_dep_helper(cd.ins, gi.ins,
                                   reason="candT streams behind the emb gather")

                # currT[d%128, kd, (b s)] = transpose(gather) + peT, in bf16
                currT_sb = actsp.tile([128, KD, B_LOC * S], BF16)
                for b in range(B_LOC):
                    tp_ps = ps_st.tile([128, KD, 128], BF16, tag="st")
                    for kd in range(KD):
                        nc.tensor.transpose(tp_ps[:, kd, :],
                                            g_all[:, b, kd * 128:(kd + 1) * 128],
                                            c16_sb[:, C_ID:C_ID + 128])
                    nc.vector.tensor_add(
                        out=currT_sb[:, :, b * S:(b + 1) * S],
                        in0=tp_ps[:, :, :],
                        in1=peT_sb[:, :, :],
                    )

                # projections: QT/KT [j%128, kj, (b,s)] bf16 with bias, streamed
                # per batch-pair so the first pair starts before gathers b2/b3
                QT_sb = actsp.tile([128, KD, B_LOC * S], BF16)
                KT_sb = actsp.tile([128, KD, B_LOC * S], BF16)
                th_sb = actsp.tile([128, B_LOC, D], BF16)  # tanh(attn) [s, b, j]
                V_sb = actsp.tile([128, B_LOC, D], BF16)
                p_tiles = {}
                HBS = 2 * S
                for bh in range(2):
                    bsl = slice(bh * HBS, (bh + 1) * HBS)
                    for kj in range(KD):
                        q_ps = ps_proj.tile([128, HBS], FP32, tag="big")
                        for kd in range(KD):
                            nc.tensor.matmul(q_ps[:, :],
                                             w4_sb[:, 0, kd, kj * 128:(kj + 1) * 128],
                                             currT_sb[:, kd, bsl],
                                             start=(kd == 0), stop=(kd == KD - 1))
                        nc.vector.tensor_scalar_add(QT_sb[:, kj, bsl], q_ps[:, :],
                                                    c32_sb[:, F_BQ + kj:F_BQ + kj + 1])
                        k_ps = ps_proj.tile([128, HBS], FP32, tag="big")
                        for kd in range(KD):
                            nc.tensor.matmul(k_ps[:, :],
                                             w4_sb[:, 1, kd, kj * 128:(kj + 1) * 128],
                                             currT_sb[:, kd, bsl],
                                             start=(kd == 0), stop=(kd == KD - 1))
                        nc.vector.tensor_scalar_add(KT_sb[:, kj, bsl], k_ps[:, :],
                                                    c32_sb[:, F_BK + kj:F_BK + kj + 1])
                    # S^T + exp for this batch pair immediately: these 8 exps on
                    # Act overlap the next pair's QK matmuls on PE
                    for b in (2 * bh, 2 * bh + 1):
                        for half in range(2):
                            st_ps = ps_st.tile([128, 4 * S], FP32, tag="st")
                            for hh in range(4):  # head = hh*2 + half
                                qs = QT_sb[half * 64:(half + 1) * 64, hh, b * S:(b + 1) * S]
                                ks = KT_sb[half * 64:(half + 1) * 64, hh, b * S:(b + 1) * S]
                                nc.tensor.matmul(st_ps[:, hh * S:(hh + 1) * S], ks, qs,
                                                 start=True, stop=True)
                            p_sb = php.tile([128, 4 * S], BF16, tag="p")
                            nc.scalar.activation(p_sb[:, :], st_ps[:, :], ACTF.Exp,
                                                 bias=shatt_sb[:, :1])
                            p_tiles[(b, half)] = p_sb
                # per batch: V projection (overlaps the exps on Act), rowsums via
                # N=1 matmuls, AV, per-head 1/rowsum rescale, tanh
                for b in range(B_LOC):
                    v_ps = ps_proj.tile([128, D], FP32, tag="big")
                    for kd in range(KD):
                        nc.tensor.matmul(v_ps[:, :],
                                         currT_sb[:, kd, b * S:(b + 1) * S],
                                         w4_sb[:, 2, kd, :],
                                         start=(kd == 0), stop=(kd == KD - 1))
                    nc.vector.tensor_add(out=V_sb[:, b, :], in0=v_ps[:, :], in1=bvb_sb[:, :])
                    rs_ps = ps_rs.tile([128, H], FP32, tag="rs")
                    av_ps = ps_av.tile([128, D], FP32, tag="av")
                    last_av = None
                    for half in range(2):
                        for hh in range(4):
                            h = hh * 2 + half
                            nc.tensor.matmul(rs_ps[:, h:h + 1],
                                             p_tiles[(b, half)][:, hh * S:(hh + 1) * S],
                                             ones_sb[:, :],
                                             start=True, stop=True)
                            last_av = nc.tensor.matmul(
                                av_ps[:, h * DH:(h + 1) * DH],
                                p_tiles[(b, half)][:, hh * S:(hh + 1) * S],
                                V_sb[:, b, h * DH:(h + 1) * DH],
                                start=True, stop=True)
                    rec_sb = smallp.tile([128, H], FP32, tag="rec")
                    nc.vector.reciprocal(rec_sb[:, :], rs_ps[:, :])
                    # 1/rowsum rescale as one broadcast mult (rec stride-0 over
                    # dh); the bank has 8 matmul writers and this is a full-bank
                    # read, so the dep helper pins the final drain
                    att_sb = php.tile([128, D], BF16, tag="att")
                    op = nc.vector.tensor_mul(
                        out=att_sb[:, :].rearrange("p (h x) -> p h x", h=H),
                        in0=av_ps[:, :].rearrange("p (h x) -> p h x", h=H),
                        in1=rec_sb[:, :].rearrange("p (h one) -> p h one", one=1)
                            .to_broadcast([128, H, DH]))
                    add_dep_helper(op.ins, last_av.ins,
                                   reason="att bank read after all AV writes")
                    nc.scalar.activation(th_sb[:, b, :], att_sb[:, :], ACTF.Tanh)

                # select mask positions (transposed): thselT [d%128, kd, i_loc] bf16
                thsel_sb = actsp.tile([128, KD, I_LOC], BF16)
                for kd in range(KD):
                    ts_ps = ps_st.tile([128, I_LOC], FP32, tag="st")
                    last_ts = None
                    for b in range(B_LOC):
                        last_ts = nc.tensor.matmul(ts_ps[:, b * NM:(b + 1) * NM],
                                                   th_sb[:, b, kd * 128:(kd + 1) * 128],
                                                   sel_sb[:, b, :],
                                                   start=True, stop=True)
                    op = nc.vector.tensor_copy(out=thsel_sb[:, kd, :], in_=ts_ps[:, :])
                    add_dep_helper(op.ins, last_ts.ins,
                                   reason="ts bank read after all sel writes")
                # t2 projection -> G_localT [d, i_loc] bf16 -> DRAM for AllGather
                gt_sb = actsp.tile([128, KD, I_LOC], BF16)
                for mj in range(KD):
                    g_ps = ps_proj.tile([128, I_LOC], FP32, tag="big")
                    for kd in range(KD):
                        nc.tensor.matmul(g_ps[:, :],
                                         w4_sb[:, 3, kd, mj * 128:(mj + 1) * 128],
                                         thsel_sb[:, kd, :],
                                         start=(kd == 0), stop=(kd == KD - 1))
                    nc.vector.tensor_scalar_add(gt_sb[:, mj, :], g_ps[:, :],
                                                c32_sb[:, F_T2B + mj:F_T2B + mj + 1])
                nc.sync.dma_start(out=ag_g_in[:, :].rearrange("(mj p) i -> p mj i", p=128),
                                  in_=gt_sb[:, :, :])

                # ---- AllGather G ----
                if sim_local:
                    nc.sync.dma_start(
                        out=ag_g_out[:, :].rearrange("(c d) i -> c d i", c=N_CORES),
                        in_=ag_g_in[:, :].rearrange("(one d) i -> one d i", one=1)
                            .to_broadcast([N_CORES, D, I_LOC]))
                else:
                    nc.gpsimd.collective_compute(
                        "AllGather", mybir.AluOpType.bypass,
                        replica_groups=[list(range(N_CORES))],
                        ins=[ag_g_in[:, :].opt()], outs=[ag_g_out[:, :].opt()],
                    )
                ag_g_view = ag_g_out[:, :].rearrange("(c kd p) i -> p kd c i", p=128, kd=KD)
                for kd in range(KD):
                    nc.sync.dma_start(
                        out=GT_sb[:, kd, :].rearrange("p (c i) -> p c i", c=N_CORES),
                        in_=ag_g_view[:, kd, :, :],
                    )

            # ================= Phase B: scores, exp, sums, sub, writeout =========
            # row-group-major: group mi's AllGather + subtract + output DMA overlap
            # groups mi+1..3's matmuls
            with (
                tc.tile_pool(name="ps_sc", bufs=8, space="PSUM") as ps_sc,
            ):
                def epilogue(mi):
                    # stot readback -> lnS (fast-log + 1 Newton step via Exp)
                    # -> subtract -> quarter writeout DMAs.
                    # Emitted AFTER group mi+1's exps/copies so the AllGather
                    # wait never head-of-line-blocks the in-order engine queues.
                    last = mi == KD - 1
                    nread = 2 * N_CORES if last else N_CORES
                    stot_sb = smallp.tile([128, 2 * N_CORES], FP32, tag="stot")
                    nc.sync.dma_start(
                        out=stot_sb[:, 0:N_CORES],
                        in_=ag_s_out[mi][:, 0].rearrange("(c p) -> p c", p=128))
                    if last:
                        nc.sync.dma_start(
                            out=stot_sb[:, N_CORES:],
                            in_=ag_s_out[KD][:, 0].rearrange("(c p) -> p c", p=128))
                    stl_sb = smallp.tile([128, 3], FP32, tag="stl")
                    nc.vector.reduce_sum(stl_sb[:, 0:1], stot_sb[:, 0:nread],
                                         axis=mybir.AxisListType.X)
                    # y0 = fast-log(S); lnS30 = y0 + S*exp(-y0) - 1 + SH_SC
                    nc.vector.tensor_scalar(
                        out=stl_sb[:, 1:2], in0=stl_sb[:, 0:1].bitcast(INT32),
                        scalar1=FL_K1, scalar2=FL_K2, op0=ALU.mult, op1=ALU.add)
                    ey_sb = smallp.tile([128, 1], FP32, tag="ey")
                    nc.scalar.activation(ey_sb[:, :], stl_sb[:, 1:2], ACTF.Exp,
                                         scale=cm1_sb[:, :1])
                    nc.vector.tensor_mul(out=stl_sb[:, 2:3], in0=stl_sb[:, 0:1],
                                          in1=ey_sb[:, :])
                    nc.vector.tensor_add(out=stl_sb[:, 2:3], in0=stl_sb[:, 2:3],
                                         in1=stl_sb[:, 1:2])
                    # cc = FL_K2 + SH_SC - lnS30  (lnS30 = y1 - 1 + SH_SC)
                    cc_sb = smallp.tile([128, 1], FP32, tag="cc")
                    nc.vector.tensor_scalar(
                        out=cc_sb[:, :], in0=stl_sb[:, 2:3],
                        scalar1=-1.0, scalar2=FL_K2 + 1.0, op0=ALU.mult, op1=ALU.add)
                    QV = VSH // 4
                    for v in range(NCH):
                        sl = pexp_sb[:, mi, v * VCH:(v + 1) * VCH]
                        nc.vector.tensor_scalar(
                            out=sl, in0=sl.bitcast(INT16),
                            scalar1=FL_K1B, scalar2=cc_sb[:, :1],
                            op0=ALU.mult, op1=ALU.add)
                    for qq in range(4):
                        nc.sync.dma_start(
                            out=out.ap()[mi * 128:(mi + 1) * 128,
                                         qq * QV:(qq + 1) * QV],
                            in_=pexp_sb[:, mi, qq * QV:(qq + 1) * QV],
                        )

                def launch_ag(slot, src_ap):
                    nc.sync.dma_start(out=ag_s_in[slot][:, :], in_=src_ap)
                    if sim_local:
                        nc.sync.dma_start(
                            out=ag_s_out[slot][:, :].rearrange("(c i) one -> c i one", c=N_CORES),
                            in_=ag_s_in[slot][:, :].rearrange("(one i) x -> one i x", one=1)
                                .to_broadcast([N_CORES, 128, 1]))
                    else:
                        nc.gpsimd.collective_compute(
                            "AllGather", mybir.AluOpType.bypass,
                            replica_groups=[list(range(N_CORES))],
                            ins=[ag_s_in[slot][:, :].opt()], outs=[ag_s_out[slot][:, :].opt()],
                        )

                for mi in range(KD):
                    last = mi == KD - 1
                    for v in range(NCH):
                        sc_ps = ps_sc.tile([128, VCH], FP32, tag="sc")
                        for kd in range(KD):
                            nc.tensor.matmul(sc_ps[:, :],
                                             GT_sb[:, kd, mi * 128:(mi + 1) * 128],
                                             candT_sb[:, kd, v * VCH:(v + 1) * VCH],
                                             start=(kd == 0), stop=(kd == KD - 1))
                        if v % 2 == 0:
                            nc.scalar.activation(pexp_sb[:, mi, v * VCH:(v + 1) * VCH],
                                                 sc_ps[:, :],
                                                 ACTF.Exp, bias=shsc_sb[:, :1],
                                                 accum_out=sums_sb[:, mi, v:v + 1])
                        else:
                            nc.scalar.activation(pexp_sb[:, mi, v * VCH:(v + 1) * VCH],
                                                 sc_ps[:, :],
                                                 ACTF.Exp, bias=shsc_sb[:, :1])
                            nc.vector.reduce_sum(sums_sb[:, mi, v:v + 1],
                                                 pexp_sb[:, mi, v * VCH:(v + 1) * VCH],
                                                 axis=mybir.AxisListType.X)
                        if last and v == NCH - 2:
                            # last group: AllGather chunks 0..8 early (hides
                            # under chunk 9); chunk 9's sum goes in a second,
                            # concurrent AllGather right after its accum lands
                            sl_sb = smallp.tile([128, 1], FP32, tag="sl")
                            nc.vector.reduce_sum(sl_sb[:, :], sums_sb[:, mi, 0:NCH - 1],
                                                 axis=mybir.AxisListType.X)
                            nc.vector.tensor_sub(out=sl_sb[:, :], in0=sl_sb[:, :],
                                                 in1=c32_sb[:, F_CORR:F_CORR + 1])
                            launch_ag(mi, sl_sb[:, :])
                    if not last:
                        sl_sb = smallp.tile([128, 1], FP32, tag="sl")
                        nc.vector.reduce_sum(sl_sb[:, :], sums_sb[:, mi, :],
                                             axis=mybir.AxisListType.X)
                        nc.vector.tensor_sub(out=sl_sb[:, :], in0=sl_sb[:, :],
                                             in1=c32_sb[:, F_CORR:F_CORR + 1])
                        launch_ag(mi, sl_sb[:, :])
                        if mi >= 1:
                            epilogue(mi - 1)
                    else:
                        launch_ag(KD, sums_sb[:, mi, NCH - 1:NCH])
                        epilogue(mi - 1)
                epilogue(KD - 1)
    nc.compile()
    return nc


_NC_CACHE = None


def _get_nc():
    global _NC_CACHE
    if _NC_CACHE is None:
        _NC_CACHE = build()
    return _NC_CACHE


def prepare_in_maps(inputs):
    emb = np.asarray(inputs["emb"], dtype=np.float32)
    embb = np.ascontiguousarray(emb.astype(NPBF))
    mask_curr = np.asarray(inputs["mask_curr_traj_grid"]).astype(np.int32)
    mask_pos = np.asarray(inputs["mask_pos"]).astype(np.int32)
    w4 = np.stack([
        np.asarray(inputs["c_wq"], dtype=np.float32).T,
        np.asarray(inputs["c_wk"], dtype=np.float32).T,
        np.asarray(inputs["c_wv"], dtype=np.float32).T,
        np.asarray(inputs["t2_w"], dtype=np.float32).T,
    ]).astype(NPBF)
    bq = np.asarray(inputs["c_bq"], dtype=np.float32)
    bk = np.asarray(inputs["c_bk"], dtype=np.float32)
    bv = np.asarray(inputs["c_bv"], dtype=np.float32)
    t2b = np.asarray(inputs["t2_b"], dtype=np.float32)
    peT = _positional_embedding(D, S).T  # [D, S]

    candTb = np.ascontiguousarray(emb[2:].T.astype(NPBF))  # [D, VOCAB]

    # bf16 const blob
    c16 = np.zeros((128, C16), dtype=NPBF)
    c16[:, C_PE:C_SEL] = peT.reshape(KD, 128, S).transpose(1, 0, 2).reshape(128, KD * S)
    c16[:, C_ONE] = 1.0
    c16[:, C_BVB:C_BVB + D] = np.broadcast_to(bv, (128, D))
    c16[:, C_ID:C_ID + 128] = np.eye(128, dtype=NPBF)
    # fp32 const blob (core-independent part)
    c32 = np.zeros((128, F32), dtype=np.float32)
    c32[:, F_BQ:F_BQ + KD] = bq.reshape(KD, 128).T
    c32[:, F_BK:F_BK + KD] = bk.reshape(KD, 128).T
    c32[:, F_T2B:F_T2B + KD] = t2b.reshape(KD, 128).T

    in_maps = []
    for c in range(N_CORES):
        lo = c * VSH
        hi = min((c + 1) * VSH, VOCAB)
        shard = np.zeros((D, VSH), dtype=NPBF)
        shard[:, : hi - lo] = candTb[:, lo:hi]
        n_inv = VSH - (hi - lo)
        c32_c = c32.copy()
        c32_c[:, F_CORR] = n_inv * math.exp(-SH_SC)
        mp = mask_pos[c * B_LOC:(c + 1) * B_LOC]  # [B_LOC, NM]
        c16_c = c16.copy()
        sel_c = np.zeros((S, B_LOC, NM), dtype=NPBF)
        for b in range(B_LOC):
            sel_c[mp[b], b, np.arange(NM)] = 1.0
        c16_c[:, C_SEL:C_ONE] = sel_c.reshape(S, B_LOC * NM)
        in_maps.append(dict(
            embb=embb,
            candT=np.ascontiguousarray(shard),
            idx=np.ascontiguousarray(mask_curr[c * B_LOC:(c + 1) * B_LOC].reshape(-1)),
            w4=w4, cst16=c16_c, cst32=c32_c,
        ))
    return in_maps


def assemble_output(results):
    parts = []
    for c in range(N_CORES):
        lo = c * VSH
        hi = min((c + 1) * VSH, VOCAB)
        parts.append(results[c]["out"][:, : hi - lo].astype(np.float32))
    return np.ascontiguousarray(np.concatenate(parts, axis=1))


def kernel(**inputs):
    nc = _get_nc()
    in_maps = prepare_in_maps(inputs)
    res = run_bass_kernel_spmd(nc, in_maps, core_ids=list(range(N_CORES)))
    return assemble_output(res.results)


# revision 34
# speedup vs baseline: 1.0517x; 1.0404x over previous
"""Trainium2 Bass kernel for nn_AttnMoveModel (dense_transformer).

Strategy (8 NeuronCores):
  - Only the `curr` path of the reference affects the output (hist self-attn and
    cross-attn results are dead), so only that path is computed.
  - Attention is data-parallel over batch (4 of 32 batches per core).
  - The vocab projection (gathered @ emb[2:].T) is tensor-parallel, column-split
    over the vocab (5120 padded columns per core), with an AllGather of the
    gathered activations before it and per-row-group AllGathers of exp-sums for
    the log_softmax denominator (so the subtract+writeout of row group i
    pipelines behind row group i+1's matmuls).
  - All matmul inputs are bf16 (rel err ~2e-3 vs 2e-2 budget): 1 cycle/row on
    the PE array for every shape and half the HBM traffic of fp32.
  - Attention computes S^T (keys on partitions) so the exp output IS P^T in
    SBUF: no P transposes / PSUM copies; softmax row sums come from free N=1
    matmuls against a ones vector; 1/rowsum is folded in post-AV.
  - The full candidate shard (bf16) is preloaded into SBUF during attention
    (ordered behind the gathers on the DMA engines), so the score phase runs
    back-to-back matmuls with no input DMA.
  - log(sum) is computed with a fast-log bit trick + one Newton step using Exp
    (err ~5e-4), so the kernel never touches the Ln activation table: the whole
    kernel uses one table (exp+tanh), avoiding 1.3us table swaps per use.
  - The score phase persists exp(sc-30) (the softmax numerators, bf16) instead
    of raw scores: GPSIMD cannot read PSUM, and this removes all PSUM->SBUF
    copies and subtracts. The epilogue recovers log-probs in one DVE op per
    chunk: out = bitcast_i16(p)*(ln2/128) + (K2 + 30 - lnS).

Host-side prep (inside kernel()): shard indices/batches, pre-transpose weights
and the emb vocab shard into bf16, build one-hot selection matrices from
mask_pos, positional-encoding table.
"""
import contextlib
import math
import sys

sys.path.insert(0, "/opt/trn_rl_repo")

import numpy as np
import ml_dtypes

import concourse.bass as bass
import concourse.mybir as mybir
import concourse.tile as tile
from concourse.tile import add_dep_helper
from concourse import bacc
from concourse.bass_utils import run_bass_kernel_spmd

FP32 = mybir.dt.float32
BF16 = mybir.dt.bfloat16
INT32 = mybir.dt.int32
INT16 = mybir.dt.int16
ACTF = mybir.ActivationFunctionType
ALU = mybir.AluOpType
NPBF = ml_dtypes.bfloat16

N_CORES = 8
B, S, D, H, DH = 32, 128, 512, 8, 64
B_LOC = B // N_CORES              # 4 batches per core
NM = 16                           # mask positions per batch
I_LOC = B_LOC * NM                # 64 gathered rows per core
I_TOT = B * NM                    # 512 gathered rows total
GRID = 40000
VOCAB = GRID - 2                  # 39998 candidate rows
VSH = 5120                        # padded vocab shard per core (8*5120 >= VOCAB)
VCH = 512                         # vocab chunk (matmul N)
NCH = VSH // VCH                  # 10 chunks
KD = D // 128                     # 4 contraction tiles
SH_ATT = 15.0                     # exp shift for attention softmax
SH_SC = 30.0                      # exp shift for final log_softmax
# fast-log: ln(x) ~= bitcast_i32(x)*K1 + K2, |err| <= 0.030; one Newton step
# with exp brings it to ~5e-4
FL_K1 = math.log(2.0) / (1 << 23)
FL_K2 = -(127.0 - 0.0430) * math.log(2.0)
FL_K1B = math.log(2.0) / 128          # bf16 variant (bits in the high 16)

# bf16 const blob layout (columns)
C_PE = 0                          # peT [128, KD*S]    (kd, s)
C_SEL = C_PE + KD * S             # sel [128, B_LOC*NM] (b, m); partition = s
C_ONE = C_SEL + B_LOC * NM        # ones [128, 1]
C_BVB = C_ONE + 1                 # bv broadcast [128, D]
C_ID = C_BVB + D                  # identity [128, 128] for PE transposes
C16 = C_ID + 128
# fp32 const blob layout (columns)
F_BQ = 0                          # bq [128, KD]
F_BK = F_BQ + KD
F_T2B = F_BK + KD
F_CORR = F_T2B + KD               # padding correction [128, 1]
F32 = F_CORR + 1


def _positional_embedding(d_model, max_len):
    pe = np.zeros((max_len, d_model), dtype=np.float32)
    position = np.arange(max_len, dtype=np.float32)[:, None]
    div_term = np.exp(np.arange(0, d_model, 2, dtype=np.float32) * -(math.log(10000.0) / d_model))
    pe[:, 0::2] = np.sin(position * div_term)
    pe[:, 1::2] = np.cos(position * div_term)
    return pe


def build(sim_local=False):
    nc = bacc.Bacc("TRN2", target_bir_lowering=False, debug=False, num_devices=N_CORES)

    # ---- I/O ----
    embb = nc.dram_tensor("embb", [GRID, D], BF16, kind="ExternalInput")
    candT = nc.dram_tensor("candT", [D, VSH], BF16, kind="ExternalInput")
    idx = nc.dram_tensor("idx", [B_LOC * S], INT32, kind="ExternalInput")
    w4 = nc.dram_tensor("w4", [4, D, D], BF16, kind="ExternalInput")  # wqt wkt wvt t2wt
    cst16 = nc.dram_tensor("cst16", [128, C16], BF16, kind="ExternalInput")
    cst32 = nc.dram_tensor("cst32", [128, F32], FP32, kind="ExternalInput")
    out = nc.dram_tensor("out", [I_TOT, VSH], BF16, kind="ExternalOutput")

    with tile.TileContext(nc) as tc:
        with (
            tc.tile_pool(name="const", bufs=1) as constp,
            tc.tile_pool(name="persist", bufs=1) as persp,
            tc.tile_pool(name="small", bufs=2) as smallp,
            tc.tile_pool(name="dram", bufs=1, space="DRAM") as dramp,
        ):
            # ================= constant loads (order matters on the DMA dev) ====
            idx_sb = constp.tile([S, B_LOC], INT32)
            nc.sync.dma_start(out=idx_sb[:, :],
                              in_=idx.ap().rearrange("(b s) -> s b", s=S))
            c16_sb = constp.tile([128, C16], BF16)
            nc.sync.dma_start(out=c16_sb[:, :], in_=cst16.ap())
            w4_sb = constp.tile([128, 4, KD, D], BF16)  # [d%128, which, kd, j]
            w4v = w4.ap().rearrange("w (kd p) j -> p w kd j", p=128)
            for w in range(2):  # wq, wk first (attention critical path)
                nc.sync.dma_start(out=w4_sb[:, w, :, :], in_=w4v[:, w, :, :])
            c32_sb = constp.tile([128, F32], FP32)
            nc.sync.dma_start(out=c32_sb[:, :], in_=cst32.ap())
            peT_sb = c16_sb[:, C_PE:C_SEL].rearrange("p (kd s) -> p kd s", kd=KD)
            sel_sb = c16_sb[:, C_SEL:C_ONE].rearrange("p (b m) -> p b m", b=B_LOC)
            ones_sb = c16_sb[:, C_ONE:C_ONE + 1]
            bvb_sb = c16_sb[:, C_BVB:C_BVB + D]

            shatt_sb = constp.tile([128, 1], FP32)
            nc.vector.memset(shatt_sb[:, :], -SH_ATT)
            shsc_sb = constp.tile([128, 1], FP32)
            nc.vector.memset(shsc_sb[:, :], -SH_SC)
            cm1_sb = constp.tile([128, 1], FP32)
            nc.vector.memset(cm1_sb[:, :], -1.0)

            # persistent across phases
            GT_sb = persp.tile([128, KD, I_TOT], BF16)    # [d%128, kd, i]
            candT_sb = persp.tile([128, KD, VSH], BF16)   # full candidate shard
            sums_sb = persp.tile([128, KD, NCH], FP32)    # per-chunk exp sums
            pexp_sb = persp.tile([128, KD, VSH], BF16)    # exp(sc-30) numerators
            lnS_sb = persp.tile([128, KD], FP32)

            ag_g_in = dramp.tile([D, I_LOC], BF16)
            ag_g_out = dramp.tile([N_CORES * D, I_LOC], BF16, addr_space="Shared")
            ag_s_in = [dramp.tile([128, 1], FP32, name=f"ag_s_in{m}")
                       for m in range(KD + 1)]
            ag_s_out = [dramp.tile([N_CORES * 128, 1], FP32, addr_space="Shared",
                                   name=f"ag_s_out{m}")
                        for m in range(KD + 1)]

            # ================= Phase A: gather + self-attention =================
            with (
                tc.tile_pool(name="acts", bufs=1) as actsp,
                tc.tile_pool(name="gath", bufs=1) as gathp,
                tc.tile_pool(name="ph", bufs=8) as php,
                tc.tile_pool(name="ps_proj", bufs=2, space="PSUM") as ps_proj,
                tc.tile_pool(name="ps_st", bufs=3, space="PSUM") as ps_st,
                tc.tile_pool(name="ps_rs", bufs=1, space="PSUM") as ps_rs,
                tc.tile_pool(name="ps_av", bufs=2, space="PSUM") as ps_av,
            ):
                # per-batch indirect gathers (multi-column offset APs gather
                # with a different layout than assumed — verified broken on HW)
                with tc.high_priority():
                    g_all = gathp.tile([S, B_LOC, D], BF16, tag="gather")
                    for b in range(B_LOC):
                        gi = nc.gpsimd.indirect_dma_start(
                            out=g_all[:, b, :], out_offset=None,
                            in_=embb.ap(),
                            in_offset=bass.IndirectOffsetOnAxis(ap=idx_sb[:, b:b + 1], axis=0),
                        )

                # wv/t2w and the candidate shard stream behind the gather on
                # the serialized DMA device (they are needed later)
                for w in range(2, 4):
                    wd = nc.sync.dma_start(out=w4_sb[:, w, :, :], in_=w4v[:, w, :, :])
                    add_dep_helper(wd.ins, gi.ins,
                                   reason="wv/t2w stream behind the emb gather")
                cv = candT.ap().rearrange("(kd p) n -> p kd n", p=128)
                HV = VSH // 2
                for hh in range(2):
                    cd = nc.sync.dma_start(
                        out=candT_sb[:, :, hh * HV:(hh + 1) * HV],
                        in_=cv[:, :, hh * HV:(hh + 1) * HV])
                    add_dep_helper(cd.ins, gi.ins,
                                   reason="candT streams behind the emb gather")

                # currT[d%128, kd, (b s)] = transpose(gather) + peT, in bf16
                currT_sb = actsp.tile([128, KD, B_LOC * S], BF16)
                for b in range(B_LOC):
                    tp_ps = ps_st.tile([128, KD, 128], BF16, tag="st")
                    for kd in range(KD):
                        nc.tensor.transpose(tp_ps[:, kd, :],
                                            g_all[:, b, kd * 128:(kd + 1) * 128],
                                            c16_sb[:, C_ID:C_ID + 128])
                    nc.vector.tensor_add(
                        out=currT_sb[:, :, b * S:(b + 1) * S],
                        in0=tp_ps[:, :, :],
                        in1=peT_sb[:, :, :],
                    )

                # projections: QT/KT [j%128, kj, (b,s)] bf16 with bias, streamed
                # per batch-pair so the first pair starts before gathers b2/b3
                QT_sb = actsp.tile([128, KD, B_LOC * S], BF16)
                KT_sb = actsp.tile([128, KD, B_LOC * S], BF16)
                th_sb = actsp.tile([128, B_LOC, D], BF16)  # tanh(attn) [s, b, j]
                V_sb = actsp.tile([128, B_LOC, D], BF16)
                p_tiles = {}
                HBS = 2 * S
                for bh in range(2):
                    bsl = slice(bh * HBS, (bh + 1) * HBS)
                    for kj in range(KD):
                        q_ps = ps_proj.tile([128, HBS], FP32, tag="big")
                        for kd in range(KD):
                            nc.tensor.matmul(q_ps[:, :],
                                             w4_sb[:, 0, kd, kj * 128:(kj + 1) * 128],
                                             currT_sb[:, kd, bsl],
                                             start=(kd == 0), stop=(kd == KD - 1))
                        nc.vector.tensor_scalar_add(QT_sb[:, kj, bsl], q_ps[:, :],
                                                    c32_sb[:, F_BQ + kj:F_BQ + kj + 1])
                        k_ps = ps_proj.tile([128, HBS], FP32, tag="big")
                        for kd in range(KD):
                            nc.tensor.matmul(k_ps[:, :],
                                             w4_sb[:, 1, kd, kj * 128:(kj + 1) * 128],
                                             currT_sb[:, kd, bsl],
                                             start=(kd == 0), stop=(kd == KD - 1))
                        nc.vector.tensor_scalar_add(KT_sb[:, kj, bsl], k_ps[:, :],
                                                    c32_sb[:, F_BK + kj:F_BK + kj + 1])
                    # S^T + exp for this batch pair immediately: these 8 exps on
                    # Act overlap the next pair's QK matmuls on PE
                    for b in (2 * bh, 2 * bh + 1):
                        for half in range(2):
                            st_ps = ps_st.tile([128, 4 * S], FP32, tag="st")
                            for hh in range(4):  # head = hh*2 + half
                                qs = QT_sb[half * 64:(half + 1) * 64, hh, b * S:(b + 1) * S]
                                ks = KT_sb[half * 64:(half + 1) * 64, hh, b * S:(b + 1) * S]
                                nc.tensor.matmul(st_ps[:, hh * S:(hh + 1) * S], ks, qs,
                                                 start=True, stop=True)
                            p_sb = php.tile([128, 4 * S], BF16, tag="p")
                            nc.scalar.activation(p_sb[:, :], st_ps[:, :], ACTF.Exp,
                                                 bias=shatt_sb[:, :1])
                            p_tiles[(b, half)] = p_sb
                # per batch: V projection (overlaps the exps on Act), rowsums via
                # N=1 matmuls, AV, per-head 1/rowsum rescale, tanh
                for b in range(B_LOC):
                    v_ps = ps_proj.tile([128, D], FP32, tag="big")
                    for kd in range(KD):
                        nc.tensor.matmul(v_ps[:, :],
                                         currT_sb[:, kd, b * S:(b + 1) * S],
                                         w4_sb[:, 2, kd, :],
                                         start=(kd == 0), stop=(kd == KD - 1))
                    nc.vector.tensor_add(out=V_sb[:, b, :], in0=v_ps[:, :], in1=bvb_sb[:, :])
                    rs_ps = ps_rs.tile([128, H], FP32, tag="rs")
                    av_ps = ps_av.tile([128, D], FP32, tag="av")
                    last_av = None
                    for half in range(2):
                        for hh in range(4):
                            h = hh * 2 + half
                            nc.tensor.matmul(rs_ps[:, h:h + 1],
                                             p_tiles[(b, half)][:, hh * S:(hh + 1) * S],
                                             ones_sb[:, :],
                                             start=True, stop=True)
                            last_av = nc.tensor.matmul(
                                av_ps[:, h * DH:(h + 1) * DH],
                                p_tiles[(b, half)][:, hh * S:(hh + 1) * S],
                                V_sb[:, b, h * DH:(h + 1) * DH],
                                start=True, stop=True)
                    rec_sb = smallp.tile([128, H], FP32, tag="rec")
                    nc.vector.reciprocal(rec_sb[:, :], rs_ps[:, :])
                    # 1/rowsum rescale as one broadcast mult (rec stride-0 over
                    # dh); the bank has 8 matmul writers and this is a full-bank
                    # read, so the dep helper pins the final drain
                    att_sb = php.tile([128, D], BF16, tag="att")
                    op = nc.vector.tensor_mul(
                        out=att_sb[:, :].rearrange("p (h x) -> p h x", h=H),
                        in0=av_ps[:, :].rearrange("p (h x) -> p h x", h=H),
                        in1=rec_sb[:, :].rearrange("p (h one) -> p h one", one=1)
                            .to_broadcast([128, H, DH]))
                    add_dep_helper(op.ins, last_av.ins,
                                   reason="att bank read after all AV writes")
                    nc.scalar.activation(th_sb[:, b, :], att_sb[:, :], ACTF.Tanh)

                # select mask positions (transposed): thselT [d%128, kd, i_loc] bf16
                thsel_sb = actsp.tile([128, KD, I_LOC], BF16)
                for kd in range(KD):
                    ts_ps = ps_st.tile([128, I_LOC], FP32, tag="st")
                    last_ts = None
                    for b in range(B_LOC):
                        last_ts = nc.tensor.matmul(ts_ps[:, b * NM:(b + 1) * NM],
                                                   th_sb[:, b, kd * 128:(kd + 1) * 128],
                                                   sel_sb[:, b, :],
                                                   start=True, stop=True)
                    op = nc.vector.tensor_copy(out=thsel_sb[:, kd, :], in_=ts_ps[:, :])
                    add_dep_helper(op.ins, last_ts.ins,
                                   reason="ts bank read after all sel writes")
                # t2 projection -> G_localT [d, i_loc] bf16 -> DRAM for AllGather
                gt_sb = actsp.tile([128, KD, I_LOC], BF16)
                for mj in range(KD):
                    g_ps = ps_proj.tile([128, I_LOC], FP32, tag="big")
                    for kd in range(KD):
                        nc.tensor.matmul(g_ps[:, :],
                                         w4_sb[:, 3, kd, mj * 128:(mj + 1) * 128],
                                         thsel_sb[:, kd, :],
                                         start=(kd == 0), stop=(kd == KD - 1))
                    nc.vector.tensor_scalar_add(gt_sb[:, mj, :], g_ps[:, :],
                                                c32_sb[:, F_T2B + mj:F_T2B + mj + 1])
                nc.sync.dma_start(out=ag_g_in[:, :].rearrange("(mj p) i -> p mj i", p=128),
                                  in_=gt_sb[:, :, :])

                # ---- AllGather G ----
                if sim_local:
                    agg_i = nc.sync.dma_start(
                        out=ag_g_out[:, :].rearrange("(c d) i -> c d i", c=N_CORES),
                        in_=ag_g_in[:, :].rearrange("(one d) i -> one d i", one=1)
                            .to_broadcast([N_CORES, D, I_LOC]))
                else:
                    agg_i = nc.gpsimd.collective_compute(
                        "AllGather", mybir.AluOpType.bypass,
                        replica_groups=[list(range(N_CORES))],
                        ins=[ag_g_in[:, :].opt()], outs=[ag_g_out[:, :].opt()],
                    )
                ag_g_view = ag_g_out[:, :].rearrange("(c kd p) i -> p kd c i", p=128, kd=KD)
                rb_is = []
                for kd in range(KD):
                    rb = nc.sync.dma_start(
                        out=GT_sb[:, kd, :].rearrange("p (c i) -> p c i", c=N_CORES),
                        in_=ag_g_view[:, kd, :, :],
                    )
                    rb_is.append(rb)

            # ================= Phase B: scores, exp, sums, sub, writeout =========
            # row-group-major: group mi's AllGather + subtract + output DMA overlap
            # groups mi+1..3's matmuls
            with (
                tc.tile_pool(name="ps_sc", bufs=8, space="PSUM") as ps_sc,
            ):
                warm_ps = ps_sc.tile([128, VCH], FP32, tag="sc")
                for wi in range(10):
                    wm = nc.tensor.matmul(warm_ps[:, :],
                                          c16_sb[:, C_ID:C_ID + 128],
                                          candT_sb[:, 0, 0:VCH],
                                          start=True, stop=True)
                    if wi >= 4:
                        add_dep_helper(wm.ins, agg_i.ins,
                                       reason="pe ramp warm-up spans AllGather")

                def epilogue(mi):
                    # stot readback -> lnS (fast-log + 1 Newton step via Exp)
                    # -> subtract -> quarter writeout DMAs.
                    # Emitted AFTER group mi+1's exps/copies so the AllGather
                    # wait never head-of-line-blocks the in-order engine queues.
                    last = mi == KD - 1
                    nread = 2 * N_CORES if last else N_CORES
                    stot_sb = smallp.tile([128, 2 * N_CORES], FP32, tag="stot")
                    nc.sync.dma_start(
                        out=stot_sb[:, 0:N_CORES],
                        in_=ag_s_out[mi][:, 0].rearrange("(c p) -> p c", p=128))
                    if last:
                        nc.sync.dma_start(
                            out=stot_sb[:, N_CORES:],
                            in_=ag_s_out[KD][:, 0].rearrange("(c p) -> p c", p=128))
                    stl_sb = smallp.tile([128, 3], FP32, tag="stl")
                    nc.vector.reduce_sum(stl_sb[:, 0:1], stot_sb[:, 0:nread],
                                         axis=mybir.AxisListType.X)
                    # y0 = fast-log(S); lnS30 = y0 + S*exp(-y0) - 1 + SH_SC
                    nc.vector.tensor_scalar(
                        out=stl_sb[:, 1:2], in0=stl_sb[:, 0:1].bitcast(INT32),
                        scalar1=FL_K1, scalar2=FL_K2, op0=ALU.mult, op1=ALU.add)
                    ey_sb = smallp.tile([128, 1], FP32, tag="ey")
                    nc.scalar.activation(ey_sb[:, :], stl_sb[:, 1:2], ACTF.Exp,
                                         scale=cm1_sb[:, :1])
                    nc.vector.tensor_mul(out=stl_sb[:, 2:3], in0=stl_sb[:, 0:1],
                                          in1=ey_sb[:, :])
                    nc.vector.tensor_add(out=stl_sb[:, 2:3], in0=stl_sb[:, 2:3],
                                         in1=stl_sb[:, 1:2])
                    # cc = FL_K2 + SH_SC - lnS30  (lnS30 = y1 - 1 + SH_SC)
                    cc_sb = smallp.tile([128, 1], FP32, tag="cc")
                    nc.vector.tensor_scalar(
                        out=cc_sb[:, :], in0=stl_sb[:, 2:3],
                        scalar1=-1.0, scalar2=FL_K2 + 1.0, op0=ALU.mult, op1=ALU.add)
                    QV = VSH // 4
                    for v in range(NCH):
                        sl = pexp_sb[:, mi, v * VCH:(v + 1) * VCH]
                        nc.vector.tensor_scalar(
                            out=sl, in0=sl.bitcast(INT16),
                            scalar1=FL_K1B, scalar2=cc_sb[:, :1],
                            op0=ALU.mult, op1=ALU.add)
                    for qq in range(4):
                        nc.sync.dma_start(
                            out=out.ap()[mi * 128:(mi + 1) * 128,
                                         qq * QV:(qq + 1) * QV],
                            in_=pexp_sb[:, mi, qq * QV:(qq + 1) * QV],
                        )

                def launch_ag(slot, src_ap):
                    nc.sync.dma_start(out=ag_s_in[slot][:, :], in_=src_ap)
                    if sim_local:
                        nc.sync.dma_start(
                            out=ag_s_out[slot][:, :].rearrange("(c i) one -> c i one", c=N_CORES),
                            in_=ag_s_in[slot][:, :].rearrange("(one i) x -> one i x", one=1)
                                .to_broadcast([N_CORES, 128, 1]))
                    else:
                        nc.gpsimd.collective_compute(
                            "AllGather", mybir.AluOpType.bypass,
                            replica_groups=[list(range(N_CORES))],
                            ins=[ag_s_in[slot][:, :].opt()], outs=[ag_s_out[slot][:, :].opt()],
                        )

                for mi in range(KD):
                    last = mi == KD - 1
                    for v in range(NCH):
                        sc_ps = ps_sc.tile([128, VCH], FP32, tag="sc")
                        for kd in range(KD):
                            nc.tensor.matmul(sc_ps[:, :],
                                             GT_sb[:, kd, mi * 128:(mi + 1) * 128],
                                             candT_sb[:, kd, v * VCH:(v + 1) * VCH],
                                             start=(kd == 0), stop=(kd == KD - 1))
                        if v % 2 == 0:
                            nc.scalar.activation(pexp_sb[:, mi, v * VCH:(v + 1) * VCH],
                                                 sc_ps[:, :],
                                                 ACTF.Exp, bias=shsc_sb[:, :1],
                                                 accum_out=sums_sb[:, mi, v:v + 1])
                        else:
                            nc.scalar.activation(pexp_sb[:, mi, v * VCH:(v + 1) * VCH],
                                                 sc_ps[:, :],
                                                 ACTF.Exp, bias=shsc_sb[:, :1])
                            nc.vector.reduce_sum(sums_sb[:, mi, v:v + 1],
                                                 pexp_sb[:, mi, v * VCH:(v + 1) * VCH],
                                                 axis=mybir.AxisListType.X)
                        if last and v == NCH - 2:
                            # last group: AllGather chunks 0..8 early (hides
                            # under chunk 9); chunk 9's sum goes in a second,
                            # concurrent AllGather right after its accum lands
                            sl_sb = smallp.tile([128, 1], FP32, tag="sl")
                            nc.vector.reduce_sum(sl_sb[:, :], sums_sb[:, mi, 0:NCH - 1],
                                                 axis=mybir.AxisListType.X)
                            nc.vector.tensor_sub(out=sl_sb[:, :], in0=sl_sb[:, :],
                                                 in1=c32_sb[:, F_CORR:F_CORR + 1])
                            launch_ag(mi, sl_sb[:, :])
                    if not last:
                        sl_sb = smallp.tile([128, 1], FP32, tag="sl")
                        nc.vector.reduce_sum(sl_sb[:, :], sums_sb[:, mi, :],
                                             axis=mybir.AxisListType.X)
                        nc.vector.tensor_sub(out=sl_sb[:, :], in0=sl_sb[:, :],
                                             in1=c32_sb[:, F_CORR:F_CORR + 1])
                        launch_ag(mi, sl_sb[:, :])
                        if mi >= 1:
                            epilogue(mi - 1)
                    else:
                        launch_ag(KD, sums_sb[:, mi, NCH - 1:NCH])
                        epilogue(mi - 1)
                epilogue(KD - 1)
    nc.compile()
    return nc


_NC_CACHE = None


def _get_nc():
    global _NC_CACHE
    if _NC_CACHE is None:
        _NC_CACHE = build()
    return _NC_CACHE


def prepare_in_maps(inputs):
    emb = np.asarray(inputs["emb"], dtype=np.float32)
    embb = np.ascontiguousarray(emb.astype(NPBF))
    mask_curr = np.asarray(inputs["mask_curr_traj_grid"]).astype(np.int32)
    mask_pos = np.asarray(inputs["mask_pos"]).astype(np.int32)
    w4 = np.stack([
        np.asarray(inputs["c_wq"], dtype=np.float32).T,
        np.asarray(inputs["c_wk"], dtype=np.float32).T,
        np.asarray(inputs["c_wv"], dtype=np.float32).T,
        np.asarray(inputs["t2_w"], dtype=np.float32).T,
    ]).astype(NPBF)
    bq = np.asarray(inputs["c_bq"], dtype=np.float32)
    bk = np.asarray(inputs["c_bk"], dtype=np.float32)
    bv = np.asarray(inputs["c_bv"], dtype=np.float32)
    t2b = np.asarray(inputs["t2_b"], dtype=np.float32)
    peT = _positional_embedding(D, S).T  # [D, S]

    candTb = np.ascontiguousarray(emb[2:].T.astype(NPBF))  # [D, VOCAB]

    # bf16 const blob
    c16 = np.zeros((128, C16), dtype=NPBF)
    c16[:, C_PE:C_SEL] = peT.reshape(KD, 128, S).transpose(1, 0, 2).reshape(128, KD * S)
    c16[:, C_ONE] = 1.0
    c16[:, C_BVB:C_BVB + D] = np.broadcast_to(bv, (128, D))
    c16[:, C_ID:C_ID + 128] = np.eye(128, dtype=NPBF)
    # fp32 const blob (core-independent part)
    c32 = np.zeros((128, F32), dtype=np.float32)
    c32[:, F_BQ:F_BQ + KD] = bq.reshape(KD, 128).T
    c32[:, F_BK:F_BK + KD] = bk.reshape(KD, 128).T
    c32[:, F_T2B:F_T2B + KD] = t2b.reshape(KD, 128).T

    in_maps = []
    for c in range(N_CORES):
        lo = c * VSH
        hi = min((c + 1) * VSH, VOCAB)
        shard = np.zeros((D, VSH), dtype=NPBF)
        shard[:, : hi - lo] = candTb[:, lo:hi]
        n_inv = VSH - (hi - lo)
        c32_c = c32.copy()
        c32_c[:, F_CORR] = n_inv * math.exp(-SH_SC)
        mp = mask_pos[c * B_LOC:(c + 1) * B_LOC]  # [B_LOC, NM]
        c16_c = c16.copy()
        sel_c = np.zeros((S, B_LOC, NM), dtype=NPBF)
        for b in range(B_LOC):
            sel_c[mp[b], b, np.arange(NM)] = 1.0
        c16_c[:, C_SEL:C_ONE] = sel_c.reshape(S, B_LOC * NM)
        in_maps.append(dict(
            embb=embb,
            candT=np.ascontiguousarray(shard),
            idx=np.ascontiguousarray(mask_curr[c * B_LOC:(c + 1) * B_LOC].reshape(-1)),
            w4=w4, cst16=c16_c, cst32=c32_c,
        ))
    return in_maps


def assemble_output(results):
    parts = []
    for c in range(N_CORES):
        lo = c * VSH
        hi = min((c + 1) * VSH, VOCAB)
        parts.append(results[c]["out"][:, : hi - lo].astype(np.float32))
    return np.ascontiguousarray(np.concatenate(parts, axis=1))


def kernel(**inputs):
    nc = _get_nc()
    in_maps = prepare_in_maps(inputs)
    res = run_bass_kernel_spmd(nc, in_maps, core_ids=list(range(N_CORES)))
    return assemble_output(res.results)


# revision 40
# speedup vs baseline: 1.0620x; 1.0098x over previous
"""Trainium2 Bass kernel for nn_AttnMoveModel (dense_transformer).

Strategy (8 NeuronCores):
  - Only the `curr` path of the reference affects the output (hist self-attn and
    cross-attn results are dead), so only that path is computed.
  - Attention is data-parallel over batch (4 of 32 batches per core).
  - The vocab projection (gathered @ emb[2:].T) is tensor-parallel, column-split
    over the vocab (5120 padded columns per core), with an AllGather of the
    gathered activations before it and per-row-group AllGathers of exp-sums for
    the log_softmax denominator (so the subtract+writeout of row group i
    pipelines behind row group i+1's matmuls).
  - All matmul inputs are bf16 (rel err ~2e-3 vs 2e-2 budget): 1 cycle/row on
    the PE array for every shape and half the HBM traffic of fp32.
  - Attention computes S^T (keys on partitions) so the exp output IS P^T in
    SBUF: no P transposes / PSUM copies; softmax row sums come from free N=1
    matmuls against a ones vector; 1/rowsum is folded in post-AV.
  - The full candidate shard (bf16) is preloaded into SBUF during attention
    (ordered behind the gathers on the DMA engines), so the score phase runs
    back-to-back matmuls with no input DMA.
  - log(sum) is computed with a fast-log bit trick + one Newton step using Exp
    (err ~5e-4), so the kernel never touches the Ln activation table: the whole
    kernel uses one table (exp+tanh), avoiding 1.3us table swaps per use.
  - The score phase persists exp(sc-30) (the softmax numerators, bf16) instead
    of raw scores: GPSIMD cannot read PSUM, and this removes all PSUM->SBUF
    copies and subtracts. The epilogue recovers log-probs in one DVE op per
    chunk: out = bitcast_i16(p)*(ln2/128) + (K2 + 30 - lnS).

Host-side prep (inside kernel()): shard indices/batches, pre-transpose weights
and the emb vocab shard into bf16, build one-hot selection matrices from
mask_pos, positional-encoding table.
"""
import contextlib
import math
import sys

sys.path.insert(0, "/opt/trn_rl_repo")

import numpy as np
import ml_dtypes

import concourse.bass as bass
import concourse.mybir as mybir
import concourse.tile as tile
from concourse.tile import add_dep_helper
from concourse import bacc
from concourse.bass_utils import run_bass_kernel_spmd

FP32 = mybir.dt.float32
BF16 = mybir.dt.bfloat16
INT32 = mybir.dt.int32
INT16 = mybir.dt.int16
ACTF = mybir.ActivationFunctionType
ALU = mybir.AluOpType
NPBF = ml_dtypes.bfloat16

N_CORES = 8
B, S, D, H, DH = 32, 128, 512, 8, 64
B_LOC = B // N_CORES              # 4 batches per core
NM = 16                           # mask positions per batch
I_LOC = B_LOC * NM                # 64 gathered rows per core
I_TOT = B * NM                    # 512 gathered rows total
GRID = 40000
VOCAB = GRID - 2                  # 39998 candidate rows
VSH = 5120                        # padded vocab shard per core (8*5120 >= VOCAB)
VCH = 512                         # vocab chunk (matmul N)
NCH = VSH // VCH                  # 10 chunks
KD = D // 128                     # 4 contraction tiles
SH_ATT = 15.0                     # exp shift for attention softmax
SH_SC = 30.0                      # exp shift for final log_softmax
# fast-log: ln(x) ~= bitcast_i32(x)*K1 + K2, |err| <= 0.030; one Newton step
# with exp brings it to ~5e-4
FL_K1 = math.log(2.0) / (1 << 23)
FL_K2 = -(127.0 - 0.0430) * math.log(2.0)
FL_K1B = math.log(2.0) / 128          # bf16 variant (bits in the high 16)

# bf16 const blob layout (columns)
C_PE = 0                          # peT [128, KD*S]    (kd, s)
C_SEL = C_PE + KD * S             # sel [128, B_LOC*NM] (b, m); partition = s
C_ONE = C_SEL + B_LOC * NM        # ones [128, 1]
C_BVB = C_ONE + 1                 # bv broadcast [128, D]
C_ID = C_BVB + D                  # identity [128, 128] for PE transposes
C16 = C_ID + 128
# fp32 const blob layout (columns)
F_BQ = 0                          # bq [128, KD]
F_BK = F_BQ + KD
F_T2B = F_BK + KD
F_CORR = F_T2B + KD               # padding correction [128, 1]
F32 = F_CORR + 1


def _positional_embedding(d_model, max_len):
    pe = np.zeros((max_len, d_model), dtype=np.float32)
    position = np.arange(max_len, dtype=np.float32)[:, None]
    div_term = np.exp(np.arange(0, d_model, 2, dtype=np.float32) * -(math.log(10000.0) / d_model))
    pe[:, 0::2] = np.sin(position * div_term)
    pe[:, 1::2] = np.cos(position * div_term)
    return pe


def build(sim_local=False):
    nc = bacc.Bacc("TRN2", target_bir_lowering=False, debug=False, num_devices=N_CORES)

    # ---- I/O ----
    embb = nc.dram_tensor("embb", [GRID, D], BF16, kind="ExternalInput")
    candT = nc.dram_tensor("candT", [D, VSH], BF16, kind="ExternalInput")
    idx = nc.dram_tensor("idx", [B_LOC * S], INT32, kind="ExternalInput")
    w4 = nc.dram_tensor("w4", [4, D, D], BF16, kind="ExternalInput")  # wqt wkt wvt t2wt
    cst16 = nc.dram_tensor("cst16", [128, C16], BF16, kind="ExternalInput")
    cst32 = nc.dram_tensor("cst32", [128, F32], FP32, kind="ExternalInput")
    out = nc.dram_tensor("out", [I_TOT, VSH], BF16, kind="ExternalOutput")

    with tile.TileContext(nc) as tc:
        with (
            tc.tile_pool(name="const", bufs=1) as constp,
            tc.tile_pool(name="persist", bufs=1) as persp,
            tc.tile_pool(name="small", bufs=2) as smallp,
            tc.tile_pool(name="dram", bufs=1, space="DRAM") as dramp,
        ):
            # ================= constant loads (order matters on the DMA dev) ====
            idx_sb = constp.tile([S, B_LOC], INT32)
            nc.sync.dma_start(out=idx_sb[:, :],
                              in_=idx.ap().rearrange("(b s) -> s b", s=S))
            c16_sb = constp.tile([128, C16], BF16)
            nc.sync.dma_start(out=c16_sb[:, :], in_=cst16.ap())
            w4_sb = constp.tile([128, 4, KD, D], BF16)  # [d%128, which, kd, j]
            w4v = w4.ap().rearrange("w (kd p) j -> p w kd j", p=128)
            for w in range(2):  # wq, wk first (attention critical path)
                nc.sync.dma_start(out=w4_sb[:, w, :, :], in_=w4v[:, w, :, :])
            c32_sb = constp.tile([128, F32], FP32)
            nc.sync.dma_start(out=c32_sb[:, :], in_=cst32.ap())
            peT_sb = c16_sb[:, C_PE:C_SEL].rearrange("p (kd s) -> p kd s", kd=KD)
            sel_sb = c16_sb[:, C_SEL:C_ONE].rearrange("p (b m) -> p b m", b=B_LOC)
            ones_sb = c16_sb[:, C_ONE:C_ONE + 1]
            bvb_sb = c16_sb[:, C_BVB:C_BVB + D]

            shatt_sb = constp.tile([128, 1], FP32)
            nc.vector.memset(shatt_sb[:, :], -SH_ATT)
            shsc_sb = constp.tile([128, 1], FP32)
            nc.vector.memset(shsc_sb[:, :], -SH_SC)
            cm1_sb = constp.tile([128, 1], FP32)
            nc.vector.memset(cm1_sb[:, :], -1.0)

            # persistent across phases
            GT_sb = persp.tile([128, KD, I_TOT], BF16)    # [d%128, kd, i]
            candT_sb = persp.tile([128, KD, VSH], BF16)   # full candidate shard
            sums_sb = persp.tile([128, KD, NCH], FP32)    # per-chunk exp sums
            pexp_sb = persp.tile([128, KD, VSH], BF16)    # exp(sc-30) numerators
            lnS_sb = persp.tile([128, KD], FP32)

            ag_g_in = dramp.tile([D, I_LOC], BF16)
            ag_g_out = dramp.tile([N_CORES * D, I_LOC], BF16, addr_space="Shared")
            ag_s_in = [dramp.tile([128, 1], FP32, name=f"ag_s_in{m}")
                       for m in range(KD + 1)]
            ag_s_out = [dramp.tile([N_CORES * 128, 1], FP32, addr_space="Shared",
                                   name=f"ag_s_out{m}")
                        for m in range(KD + 1)]

            # ================= Phase A: gather + self-attention =================
            with (
                tc.tile_pool(name="acts", bufs=1) as actsp,
                tc.tile_pool(name="gath", bufs=1) as gathp,
                tc.tile_pool(name="ph", bufs=8) as php,
                tc.tile_pool(name="ps_proj", bufs=2, space="PSUM") as ps_proj,
                tc.tile_pool(name="ps_st", bufs=3, space="PSUM") as ps_st,
                tc.tile_pool(name="ps_rs", bufs=1, space="PSUM") as ps_rs,
                tc.tile_pool(name="ps_av", bufs=2, space="PSUM") as ps_av,
            ):
                # per-batch indirect gathers (multi-column offset APs gather
                # with a different layout than assumed — verified broken on HW)
                with tc.high_priority():
                    g_all = gathp.tile([S, B_LOC, D], BF16, tag="gather")
                    for b in range(B_LOC):
                        gi = nc.gpsimd.indirect_dma_start(
                            out=g_all[:, b, :], out_offset=None,
                            in_=embb.ap(),
                            in_offset=bass.IndirectOffsetOnAxis(ap=idx_sb[:, b:b + 1], axis=0),
                        )

                # wv/t2w and the candidate shard stream behind the gather on
                # the serialized DMA device (they are needed later)
                for w in range(2, 4):
                    wd = nc.sync.dma_start(out=w4_sb[:, w, :, :], in_=w4v[:, w, :, :])
                    add_dep_helper(wd.ins, gi.ins,
                                   reason="wv/t2w stream behind the emb gather")
                cv = candT.ap().rearrange("(kd p) n -> p kd n", p=128)
                HV = VSH // 2
                for hh in range(2):
                    cd = nc.sync.dma_start(
                        out=candT_sb[:, :, hh * HV:(hh + 1) * HV],
                        in_=cv[:, :, hh * HV:(hh + 1) * HV])
                    add_dep_helper(cd.ins, gi.ins,
                                   reason="candT streams behind the emb gather")

                # PE pstate warm-up during the gather wait: the transposes and
                # projections then start at speed (scratch bank, values unused)
                warmA_ps = ps_proj.tile([128, 4 * S], FP32, tag="big")
                for wi in range(3):
                    nc.tensor.matmul(warmA_ps[:, :],
                                     c16_sb[:, C_ID:C_ID + 128],
                                     c16_sb[:, C_PE:C_PE + 4 * S],
                                     start=True, stop=True)

                # currT[d%128, kd, (b s)] = transpose(gather) + peT, in bf16
                currT_sb = actsp.tile([128, KD, B_LOC * S], BF16)
                for b in range(B_LOC):
                    tp_ps = ps_st.tile([128, KD, 128], BF16, tag="st")
                    for kd in range(KD):
                        nc.tensor.transpose(tp_ps[:, kd, :],
                                            g_all[:, b, kd * 128:(kd + 1) * 128],
                                            c16_sb[:, C_ID:C_ID + 128])
                    nc.vector.tensor_add(
                        out=currT_sb[:, :, b * S:(b + 1) * S],
                        in0=tp_ps[:, :, :],
                        in1=peT_sb[:, :, :],
                    )

                # projections: QT/KT [j%128, kj, (b,s)] bf16 with bias, streamed
                # per batch-pair so the first pair starts before gathers b2/b3
                QT_sb = actsp.tile([128, KD, B_LOC * S], BF16)
                KT_sb = actsp.tile([128, KD, B_LOC * S], BF16)
                th_sb = actsp.tile([128, B_LOC, D], BF16)  # tanh(attn) [s, b, j]
                V_sb = actsp.tile([128, B_LOC, D], BF16)
                p_tiles = {}
                HBS = 2 * S
                for bh in range(2):
                    bsl = slice(bh * HBS, (bh + 1) * HBS)
                    for kj in range(KD):
                        q_ps = ps_proj.tile([128, HBS], FP32, tag="big")
                        for kd in range(KD):
                            nc.tensor.matmul(q_ps[:, :],
                                             w4_sb[:, 0, kd, kj * 128:(kj + 1) * 128],
                                             currT_sb[:, kd, bsl],
                                             start=(kd == 0), stop=(kd == KD - 1))
                        nc.vector.tensor_scalar_add(QT_sb[:, kj, bsl], q_ps[:, :],
                                                    c32_sb[:, F_BQ + kj:F_BQ + kj + 1])
                        k_ps = ps_proj.tile([128, HBS], FP32, tag="big")
                        for kd in range(KD):
                            nc.tensor.matmul(k_ps[:, :],
                                             w4_sb[:, 1, kd, kj * 128:(kj + 1) * 128],
                                             currT_sb[:, kd, bsl],
                                             start=(kd == 0), stop=(kd == KD - 1))
                        nc.vector.tensor_scalar_add(KT_sb[:, kj, bsl], k_ps[:, :],
                                                    c32_sb[:, F_BK + kj:F_BK + kj + 1])
                    # S^T + exp for this batch pair immediately: these 8 exps on
                    # Act overlap the next pair's QK matmuls on PE
                    for b in (2 * bh, 2 * bh + 1):
                        for half in range(2):
                            st_ps = ps_st.tile([128, 4 * S], FP32, tag="st")
                            for hh in range(4):  # head = hh*2 + half
                                qs = QT_sb[half * 64:(half + 1) * 64, hh, b * S:(b + 1) * S]
                                ks = KT_sb[half * 64:(half + 1) * 64, hh, b * S:(b + 1) * S]
                                nc.tensor.matmul(st_ps[:, hh * S:(hh + 1) * S], ks, qs,
                                                 start=True, stop=True)
                            p_sb = php.tile([128, 4 * S], BF16, tag="p")
                            nc.scalar.activation(p_sb[:, :], st_ps[:, :], ACTF.Exp,
                                                 bias=shatt_sb[:, :1])
                            p_tiles[(b, half)] = p_sb
                # per batch: V projection (overlaps the exps on Act), rowsums via
                # N=1 matmuls, AV, per-head 1/rowsum rescale, tanh
                for b in range(B_LOC):
                    v_ps = ps_proj.tile([128, D], FP32, tag="big")
                    for kd in range(KD):
                        nc.tensor.matmul(v_ps[:, :],
                                         currT_sb[:, kd, b * S:(b + 1) * S],
                                         w4_sb[:, 2, kd, :],
                                         start=(kd == 0), stop=(kd == KD - 1))
                    nc.vector.tensor_add(out=V_sb[:, b, :], in0=v_ps[:, :], in1=bvb_sb[:, :])
                    rs_ps = ps_rs.tile([128, H], FP32, tag="rs")
                    av_ps = ps_av.tile([128, D], FP32, tag="av")
                    last_av = None
                    for half in range(2):
                        for hh in range(4):
                            h = hh * 2 + half
                            nc.tensor.matmul(rs_ps[:, h:h + 1],
                                             p_tiles[(b, half)][:, hh * S:(hh + 1) * S],
                                             ones_sb[:, :],
                                             start=True, stop=True)
                            last_av = nc.tensor.matmul(
                                av_ps[:, h * DH:(h + 1) * DH],
                                p_tiles[(b, half)][:, hh * S:(hh + 1) * S],
                                V_sb[:, b, h * DH:(h + 1) * DH],
                                start=True, stop=True)
                    rec_sb = smallp.tile([128, H], FP32, tag="rec")
                    nc.vector.reciprocal(rec_sb[:, :], rs_ps[:, :])
                    # 1/rowsum rescale as one broadcast mult (rec stride-0 over
                    # dh); the bank has 8 matmul writers and this is a full-bank
                    # read, so the dep helper pins the final drain
                    att_sb = php.tile([128, D], BF16, tag="att")
                    op = nc.vector.tensor_mul(
                        out=att_sb[:, :].rearrange("p (h x) -> p h x", h=H),
                        in0=av_ps[:, :].rearrange("p (h x) -> p h x", h=H),
                        in1=rec_sb[:, :].rearrange("p (h one) -> p h one", one=1)
                            .to_broadcast([128, H, DH]))
                    add_dep_helper(op.ins, last_av.ins,
                                   reason="att bank read after all AV writes")
                    nc.scalar.activation(th_sb[:, b, :], att_sb[:, :], ACTF.Tanh)

                # select mask positions (transposed): thselT [d%128, kd, i_loc] bf16
                thsel_sb = actsp.tile([128, KD, I_LOC], BF16)
                for kd in range(KD):
                    ts_ps = ps_st.tile([128, I_LOC], FP32, tag="st")
                    last_ts = None
                    for b in range(B_LOC):
                        last_ts = nc.tensor.matmul(ts_ps[:, b * NM:(b + 1) * NM],
                                                   th_sb[:, b, kd * 128:(kd + 1) * 128],
                                                   sel_sb[:, b, :],
                                                   start=True, stop=True)
                    op = nc.vector.tensor_copy(out=thsel_sb[:, kd, :], in_=ts_ps[:, :])
                    add_dep_helper(op.ins, last_ts.ins,
                                   reason="ts bank read after all sel writes")
                # t2 projection -> G_localT [d, i_loc] bf16 -> DRAM for AllGather
                gt_sb = actsp.tile([128, KD, I_LOC], BF16)
                for mj in range(KD):
                    g_ps = ps_proj.tile([128, I_LOC], FP32, tag="big")
                    for kd in range(KD):
                        nc.tensor.matmul(g_ps[:, :],
                                         w4_sb[:, 3, kd, mj * 128:(mj + 1) * 128],
                                         thsel_sb[:, kd, :],
                                         start=(kd == 0), stop=(kd == KD - 1))
                    nc.vector.tensor_scalar_add(gt_sb[:, mj, :], g_ps[:, :],
                                                c32_sb[:, F_T2B + mj:F_T2B + mj + 1])
                nc.sync.dma_start(out=ag_g_in[:, :].rearrange("(mj p) i -> p mj i", p=128),
                                  in_=gt_sb[:, :, :])

                # ---- AllGather G ----
                if sim_local:
                    agg_i = nc.sync.dma_start(
                        out=ag_g_out[:, :].rearrange("(c d) i -> c d i", c=N_CORES),
                        in_=ag_g_in[:, :].rearrange("(one d) i -> one d i", one=1)
                            .to_broadcast([N_CORES, D, I_LOC]))
                else:
                    agg_i = nc.gpsimd.collective_compute(
                        "AllGather", mybir.AluOpType.bypass,
                        replica_groups=[list(range(N_CORES))],
                        ins=[ag_g_in[:, :].opt()], outs=[ag_g_out[:, :].opt()],
                    )
                ag_g_view = ag_g_out[:, :].rearrange("(c kd p) i -> p kd c i", p=128, kd=KD)
                rb_is = []
                for kd in range(KD):
                    rb = nc.sync.dma_start(
                        out=GT_sb[:, kd, :].rearrange("p (c i) -> p c i", c=N_CORES),
                        in_=ag_g_view[:, kd, :, :],
                    )
                    rb_is.append(rb)

            # ================= Phase B: scores, exp, sums, sub, writeout =========
            # row-group-major: group mi's AllGather + subtract + output DMA overlap
            # groups mi+1..3's matmuls
            with (
                tc.tile_pool(name="ps_sc", bufs=8, space="PSUM") as ps_sc,
            ):
                warm_ps = ps_sc.tile([128, VCH], FP32, tag="sc")
                for wi in range(10):
                    wm = nc.tensor.matmul(warm_ps[:, :],
                                          c16_sb[:, C_ID:C_ID + 128],
                                          candT_sb[:, 0, 0:VCH],
                                          start=True, stop=True)
                    if wi >= 4:
                        add_dep_helper(wm.ins, agg_i.ins,
                                       reason="pe ramp warm-up spans AllGather")

                def epilogue(mi):
                    # stot readback -> lnS (fast-log + 1 Newton step via Exp)
                    # -> subtract -> quarter writeout DMAs.
                    # Emitted AFTER group mi+1's exps/copies so the AllGather
                    # wait never head-of-line-blocks the in-order engine queues.
                    last = mi == KD - 1
                    nread = 2 * N_CORES if last else N_CORES
                    stot_sb = smallp.tile([128, 2 * N_CORES], FP32, tag="stot")
                    nc.sync.dma_start(
                        out=stot_sb[:, 0:N_CORES],
                        in_=ag_s_out[mi][:, 0].rearrange("(c p) -> p c", p=128))
                    if last:
                        nc.sync.dma_start(
                            out=stot_sb[:, N_CORES:],
                            in_=ag_s_out[KD][:, 0].rearrange("(c p) -> p c", p=128))
                    stl_sb = smallp.tile([128, 3], FP32, tag="stl")
                    nc.vector.reduce_sum(stl_sb[:, 0:1], stot_sb[:, 0:nread],
                                         axis=mybir.AxisListType.X)
                    # y0 = fast-log(S); lnS30 = y0 + S*exp(-y0) - 1 + SH_SC
                    nc.vector.tensor_scalar(
                        out=stl_sb[:, 1:2], in0=stl_sb[:, 0:1].bitcast(INT32),
                        scalar1=FL_K1, scalar2=FL_K2, op0=ALU.mult, op1=ALU.add)
                    ey_sb = smallp.tile([128, 1], FP32, tag="ey")
                    nc.scalar.activation(ey_sb[:, :], stl_sb[:, 1:2], ACTF.Exp,
                                         scale=cm1_sb[:, :1])
                    nc.vector.tensor_mul(out=stl_sb[:, 2:3], in0=stl_sb[:, 0:1],
                                          in1=ey_sb[:, :])
                    nc.vector.tensor_add(out=stl_sb[:, 2:3], in0=stl_sb[:, 2:3],
                                         in1=stl_sb[:, 1:2])
                    # cc = FL_K2 + SH_SC - lnS30  (lnS30 = y1 - 1 + SH_SC)
                    cc_sb = smallp.tile([128, 1], FP32, tag="cc")
                    nc.vector.tensor_scalar(
                        out=cc_sb[:, :], in0=stl_sb[:, 2:3],
                        scalar1=-1.0, scalar2=FL_K2 + 1.0, op0=ALU.mult, op1=ALU.add)
                    QV = VSH // 4
                    for v in range(NCH):
                        sl = pexp_sb[:, mi, v * VCH:(v + 1) * VCH]
                        nc.vector.tensor_scalar(
                            out=sl, in0=sl.bitcast(INT16),
                            scalar1=FL_K1B, scalar2=cc_sb[:, :1],
                            op0=ALU.mult, op1=ALU.add)
                    for qq in range(4):
                        nc.sync.dma_start(
                            out=out.ap()[mi * 128:(mi + 1) * 128,
                                         qq * QV:(qq + 1) * QV],
                            in_=pexp_sb[:, mi, qq * QV:(qq + 1) * QV],
                        )

                def launch_ag(slot, src_ap):
                    nc.sync.dma_start(out=ag_s_in[slot][:, :], in_=src_ap)
                    if sim_local:
                        nc.sync.dma_start(
                            out=ag_s_out[slot][:, :].rearrange("(c i) one -> c i one", c=N_CORES),
                            in_=ag_s_in[slot][:, :].rearrange("(one i) x -> one i x", one=1)
                                .to_broadcast([N_CORES, 128, 1]))
                    else:
                        nc.gpsimd.collective_compute(
                            "AllGather", mybir.AluOpType.bypass,
                            replica_groups=[list(range(N_CORES))],
                            ins=[ag_s_in[slot][:, :].opt()], outs=[ag_s_out[slot][:, :].opt()],
                        )

                for mi in range(KD):
                    last = mi == KD - 1
                    for v in range(NCH):
                        sc_ps = ps_sc.tile([128, VCH], FP32, tag="sc")
                        for kd in range(KD):
                            nc.tensor.matmul(sc_ps[:, :],
                                             GT_sb[:, kd, mi * 128:(mi + 1) * 128],
                                             candT_sb[:, kd, v * VCH:(v + 1) * VCH],
                                             start=(kd == 0), stop=(kd == KD - 1))
                        if v % 2 == 0:
                            nc.scalar.activation(pexp_sb[:, mi, v * VCH:(v + 1) * VCH],
                                                 sc_ps[:, :],
                                                 ACTF.Exp, bias=shsc_sb[:, :1],
                                                 accum_out=sums_sb[:, mi, v:v + 1])
                        else:
                            nc.scalar.activation(pexp_sb[:, mi, v * VCH:(v + 1) * VCH],
                                                 sc_ps[:, :],
                                                 ACTF.Exp, bias=shsc_sb[:, :1])
                            nc.vector.reduce_sum(sums_sb[:, mi, v:v + 1],
                                                 pexp_sb[:, mi, v * VCH:(v + 1) * VCH],
                                                 axis=mybir.AxisListType.X)
                        if last and v == NCH - 2:
                            # last group: AllGather chunks 0..8 early (hides
                            # under chunk 9); chunk 9's sum goes in a second,
                            # concurrent AllGather right after its accum lands
                            sl_sb = smallp.tile([128, 1], FP32, tag="sl")
                            nc.vector.reduce_sum(sl_sb[:, :], sums_sb[:, mi, 0:NCH - 1],
                                                 axis=mybir.AxisListType.X)
                            nc.vector.tensor_sub(out=sl_sb[:, :], in0=sl_sb[:, :],
                                                 in1=c32_sb[:, F_CORR:F_CORR + 1])
                            launch_ag(mi, sl_sb[:, :])
                    if not last:
                        sl_sb = smallp.tile([128, 1], FP32, tag="sl")
                        nc.vector.reduce_sum(sl_sb[:, :], sums_sb[:, mi, :],
                                             axis=mybir.AxisListType.X)
                        nc.vector.tensor_sub(out=sl_sb[:, :], in0=sl_sb[:, :],
                                             in1=c32_sb[:, F_CORR:F_CORR + 1])
                        launch_ag(mi, sl_sb[:, :])
                        if mi >= 1:
                            epilogue(mi - 1)
                    else:
                        launch_ag(KD, sums_sb[:, mi, NCH - 1:NCH])
                        epilogue(mi - 1)
                epilogue(KD - 1)
    nc.compile()
    return nc


_NC_CACHE = None


def _get_nc():
    global _NC_CACHE
    if _NC_CACHE is None:
        _NC_CACHE = build()
    return _NC_CACHE


def prepare_in_maps(inputs):
    emb = np.asarray(inputs["emb"], dtype=np.float32)
    embb = np.ascontiguousarray(emb.astype(NPBF))
    mask_curr = np.asarray(inputs["mask_curr_traj_grid"]).astype(np.int32)
    mask_pos = np.asarray(inputs["mask_pos"]).astype(np.int32)
    w4 = np.stack([
        np.asarray(inputs["c_wq"], dtype=np.float32).T,
        np.asarray(inputs["c_wk"], dtype=np.float32).T,
        np.asarray(inputs["c_wv"], dtype=np.float32).T,
        np.asarray(inputs["t2_w"], dtype=np.float32).T,
    ]).astype(NPBF)
    bq = np.asarray(inputs["c_bq"], dtype=np.float32)
    bk = np.asarray(inputs["c_bk"], dtype=np.float32)
    bv = np.asarray(inputs["c_bv"], dtype=np.float32)
    t2b = np.asarray(inputs["t2_b"], dtype=np.float32)
    peT = _positional_embedding(D, S).T  # [D, S]

    candTb = np.ascontiguousarray(emb[2:].T.astype(NPBF))  # [D, VOCAB]

    # bf16 const blob
    c16 = np.zeros((128, C16), dtype=NPBF)
    c16[:, C_PE:C_SEL] = peT.reshape(KD, 128, S).transpose(1, 0, 2).reshape(128, KD * S)
    c16[:, C_ONE] = 1.0
    c16[:, C_BVB:C_BVB + D] = np.broadcast_to(bv, (128, D))
    c16[:, C_ID:C_ID + 128] = np.eye(128, dtype=NPBF)
    # fp32 const blob (core-independent part)
    c32 = np.zeros((128, F32), dtype=np.float32)
    c32[:, F_BQ:F_BQ + KD] = bq.reshape(KD, 128).T
    c32[:, F_BK:F_BK + KD] = bk.reshape(KD, 128).T
    c32[:, F_T2B:F_T2B + KD] = t2b.reshape(KD, 128).T

    in_maps = []
    for c in range(N_CORES):
        lo = c * VSH
        hi = min((c + 1) * VSH, VOCAB)
        shard = np.zeros((D, VSH), dtype=NPBF)
        shard[:, : hi - lo] = candTb[:, lo:hi]
        n_inv = VSH - (hi - lo)
        c32_c = c32.copy()
        c32_c[:, F_CORR] = n_inv * math.exp(-SH_SC)
        mp = mask_pos[c * B_LOC:(c + 1) * B_LOC]  # [B_LOC, NM]
        c16_c = c16.copy()
        sel_c = np.zeros((S, B_LOC, NM), dtype=NPBF)
        for b in range(B_LOC):
            sel_c[mp[b], b, np.arange(NM)] = 1.0
        c16_c[:, C_SEL:C_ONE] = sel_c.reshape(S, B_LOC * NM)
        in_maps.append(dict(
            embb=embb,
            candT=np.ascontiguousarray(shard),
            idx=np.ascontiguousarray(mask_curr[c * B_LOC:(c + 1) * B_LOC].reshape(-1)),
            w4=w4, cst16=c16_c, cst32=c32_c,
        ))
    return in_maps


def assemble_output(results):
    parts = []
    for c in range(N_CORES):
        lo = c * VSH
        hi = min((c + 1) * VSH, VOCAB)
        parts.append(results[c]["out"][:, : hi - lo].astype(np.float32))
    return np.ascontiguousarray(np.concatenate(parts, axis=1))


def kernel(**inputs):
    nc = _get_nc()
    in_maps = prepare_in_maps(inputs)
    res = run_bass_kernel_spmd(nc, in_maps, core_ids=list(range(N_CORES)))
    return assemble_output(res.results)


# revision 43
# speedup vs baseline: 1.0666x; 1.0042x over previous
"""Trainium2 Bass kernel for nn_AttnMoveModel (dense_transformer).

Strategy (8 NeuronCores):
  - Only the `curr` path of the reference affects the output (hist self-attn and
    cross-attn results are dead), so only that path is computed.
  - Attention is data-parallel over batch (4 of 32 batches per core).
  - The vocab projection (gathered @ emb[2:].T) is tensor-parallel, column-split
    over the vocab (5120 padded columns per core), with an AllGather of the
    gathered activations before it and per-row-group AllGathers of exp-sums for
    the log_softmax denominator (so the subtract+writeout of row group i
    pipelines behind row group i+1's matmuls).
  - All matmul inputs are bf16 (rel err ~2e-3 vs 2e-2 budget): 1 cycle/row on
    the PE array for every shape and half the HBM traffic of fp32.
  - Attention computes S^T (keys on partitions) so the exp output IS P^T in
    SBUF: no P transposes / PSUM copies; softmax row sums come from free N=1
    matmuls against a ones vector; 1/rowsum is folded in post-AV.
  - The full candidate shard (bf16) is preloaded into SBUF during attention
    (ordered behind the gathers on the DMA engines), so the score phase runs
    back-to-back matmuls with no input DMA.
  - log(sum) is computed with a fast-log bit trick + one Newton step using Exp
    (err ~5e-4), so the kernel never touches the Ln activation table: the whole
    kernel uses one table (exp+tanh), avoiding 1.3us table swaps per use.
  - The score phase persists exp(sc-30) (the softmax numerators, bf16) instead
    of raw scores: GPSIMD cannot read PSUM, and this removes all PSUM->SBUF
    copies and subtracts. The epilogue recovers log-probs in one DVE op per
    chunk: out = bitcast_i16(p)*(ln2/128) + (K2 + 30 - lnS).

Host-side prep (inside kernel()): shard indices/batches, pre-transpose weights
and the emb vocab shard into bf16, build one-hot selection matrices from
mask_pos, positional-encoding table.
"""
import contextlib
import math
import sys

sys.path.insert(0, "/opt/trn_rl_repo")

import numpy as np
import ml_dtypes

import concourse.bass as bass
import concourse.mybir as mybir
import concourse.tile as tile
from concourse.tile import add_dep_helper
from concourse import bacc
from concourse.bass_utils import run_bass_kernel_spmd

FP32 = mybir.dt.float32
BF16 = mybir.dt.bfloat16
INT32 = mybir.dt.int32
INT16 = mybir.dt.int16
ACTF = mybir.ActivationFunctionType
ALU = mybir.AluOpType
NPBF = ml_dtypes.bfloat16

N_CORES = 8
B, S, D, H, DH = 32, 128, 512, 8, 64
B_LOC = B // N_CORES              # 4 batches per core
NM = 16                           # mask positions per batch
I_LOC = B_LOC * NM                # 64 gathered rows per core
I_TOT = B * NM                    # 512 gathered rows total
GRID = 40000
VOCAB = GRID - 2                  # 39998 candidate rows
VSH = 5120                        # padded vocab shard per core (8*5120 >= VOCAB)
VCH = 512                         # vocab chunk (matmul N)
NCH = VSH // VCH                  # 10 chunks
KD = D // 128                     # 4 contraction tiles
SH_ATT = 15.0                     # exp shift for attention softmax
SH_SC = 30.0                      # exp shift for final log_softmax
# fast-log: ln(x) ~= bitcast_i32(x)*K1 + K2, |err| <= 0.030; one Newton step
# with exp brings it to ~5e-4
FL_K1 = math.log(2.0) / (1 << 23)
FL_K2 = -(127.0 - 0.0430) * math.log(2.0)
FL_K1B = math.log(2.0) / 128          # bf16 variant (bits in the high 16)

# bf16 const blob layout (columns)
C_PE = 0                          # peT [128, KD*S]    (kd, s)
C_SEL = C_PE + KD * S             # sel [128, B_LOC*NM] (b, m); partition = s
C_ONE = C_SEL + B_LOC * NM        # ones [128, 1]
C_BVB = C_ONE + 1                 # bv broadcast [128, D]
C_ID = C_BVB + D                  # identity [128, 128] for PE transposes
C16 = C_ID + 128
# fp32 const blob layout (columns)
F_BQ = 0                          # bq [128, KD]
F_BK = F_BQ + KD
F_T2B = F_BK + KD
F_CORR = F_T2B + KD               # padding correction [128, 1]
F32 = F_CORR + 1


def _positional_embedding(d_model, max_len):
    pe = np.zeros((max_len, d_model), dtype=np.float32)
    position = np.arange(max_len, dtype=np.float32)[:, None]
    div_term = np.exp(np.arange(0, d_model, 2, dtype=np.float32) * -(math.log(10000.0) / d_model))
    pe[:, 0::2] = np.sin(position * div_term)
    pe[:, 1::2] = np.cos(position * div_term)
    return pe


def build(sim_local=False):
    nc = bacc.Bacc("TRN2", target_bir_lowering=False, debug=False, num_devices=N_CORES)

    # ---- I/O ----
    embb = nc.dram_tensor("embb", [GRID, D], BF16, kind="ExternalInput")
    candT = nc.dram_tensor("candT", [D, VSH], BF16, kind="ExternalInput")
    idx = nc.dram_tensor("idx", [B_LOC * S], INT32, kind="ExternalInput")
    w4 = nc.dram_tensor("w4", [4, D, D], BF16, kind="ExternalInput")  # wqt wkt wvt t2wt
    cst16 = nc.dram_tensor("cst16", [128, C16], BF16, kind="ExternalInput")
    cst32 = nc.dram_tensor("cst32", [128, F32], FP32, kind="ExternalInput")
    out = nc.dram_tensor("out", [I_TOT, VSH], BF16, kind="ExternalOutput")

    with tile.TileContext(nc) as tc:
        with (
            tc.tile_pool(name="const", bufs=1) as constp,
            tc.tile_pool(name="persist", bufs=1) as persp,
            tc.tile_pool(name="small", bufs=2) as smallp,
            tc.tile_pool(name="dram", bufs=1, space="DRAM") as dramp,
        ):
            # ================= constant loads (order matters on the DMA dev) ====
            idx_sb = constp.tile([S, B_LOC], INT32)
            nc.sync.dma_start(out=idx_sb[:, :],
                              in_=idx.ap().rearrange("(b s) -> s b", s=S))
            c16_sb = constp.tile([128, C16], BF16)
            nc.sync.dma_start(out=c16_sb[:, :], in_=cst16.ap())
            w4_sb = constp.tile([128, 4, KD, D], BF16)  # [d%128, which, kd, j]
            w4v = w4.ap().rearrange("w (kd p) j -> p w kd j", p=128)
            for w in range(2):  # wq, wk first (attention critical path)
                nc.sync.dma_start(out=w4_sb[:, w, :, :], in_=w4v[:, w, :, :])
            c32_sb = constp.tile([128, F32], FP32)
            nc.sync.dma_start(out=c32_sb[:, :], in_=cst32.ap())
            peT_sb = c16_sb[:, C_PE:C_SEL].rearrange("p (kd s) -> p kd s", kd=KD)
            sel_sb = c16_sb[:, C_SEL:C_ONE].rearrange("p (b m) -> p b m", b=B_LOC)
            ones_sb = c16_sb[:, C_ONE:C_ONE + 1]
            bvb_sb = c16_sb[:, C_BVB:C_BVB + D]

            shatt_sb = constp.tile([128, 1], FP32)
            nc.vector.memset(shatt_sb[:, :], -SH_ATT)
            shsc_sb = constp.tile([128, 1], FP32)
            nc.vector.memset(shsc_sb[:, :], -SH_SC)
            cm1_sb = constp.tile([128, 1], FP32)
            nc.vector.memset(cm1_sb[:, :], -1.0)

            # persistent across phases
            GT_sb = persp.tile([128, KD, I_TOT], BF16)    # [d%128, kd, i]
            candT_sb = persp.tile([128, KD, VSH], BF16)   # full candidate shard
            sums_sb = persp.tile([128, KD, NCH], FP32)    # per-chunk exp sums
            pexp_sb = persp.tile([128, KD, VSH], BF16)    # exp(sc-30) numerators
            lnS_sb = persp.tile([128, KD], FP32)

            ag_g_in = dramp.tile([D, I_LOC], BF16)
            ag_g_out = dramp.tile([N_CORES * D, I_LOC], BF16, addr_space="Shared")
            ag_s_in = [dramp.tile([128, 1], FP32, name=f"ag_s_in{m}")
                       for m in range(KD + 1)]
            ag_s_out = [dramp.tile([N_CORES * 128, 1], FP32, addr_space="Shared",
                                   name=f"ag_s_out{m}")
                        for m in range(KD + 1)]

            # ================= Phase A: gather + self-attention =================
            with (
                tc.tile_pool(name="acts", bufs=1) as actsp,
                tc.tile_pool(name="gath", bufs=1) as gathp,
                tc.tile_pool(name="ph", bufs=8) as php,
                tc.tile_pool(name="ps_proj", bufs=2, space="PSUM") as ps_proj,
                tc.tile_pool(name="ps_st", bufs=3, space="PSUM") as ps_st,
                tc.tile_pool(name="ps_rs", bufs=1, space="PSUM") as ps_rs,
                tc.tile_pool(name="ps_av", bufs=2, space="PSUM") as ps_av,
            ):
                # per-batch indirect gathers (multi-column offset APs gather
                # with a different layout than assumed — verified broken on HW)
                with tc.high_priority():
                    g_all = gathp.tile([S, B_LOC, D], BF16, tag="gather")
                    for b in range(B_LOC):
                        gi = nc.gpsimd.indirect_dma_start(
                            out=g_all[:, b, :], out_offset=None,
                            in_=embb.ap(),
                            in_offset=bass.IndirectOffsetOnAxis(ap=idx_sb[:, b:b + 1], axis=0),
                        )

                # wv/t2w and the candidate shard stream behind the gather on
                # the serialized DMA device (they are needed later)
                for w in range(2, 4):
                    wd = nc.sync.dma_start(out=w4_sb[:, w, :, :], in_=w4v[:, w, :, :])
                    add_dep_helper(wd.ins, gi.ins,
                                   reason="wv/t2w stream behind the emb gather")
                cv = candT.ap().rearrange("(kd p) n -> p kd n", p=128)
                HV = VSH // 2
                for hh in range(2):
                    cd = nc.sync.dma_start(
                        out=candT_sb[:, :, hh * HV:(hh + 1) * HV],
                        in_=cv[:, :, hh * HV:(hh + 1) * HV])
                    add_dep_helper(cd.ins, gi.ins,
                                   reason="candT streams behind the emb gather")

                # PE pstate warm-up during the gather wait: the transposes and
                # projections then start at speed (scratch bank, values unused)
                warmA_ps = ps_proj.tile([128, 4 * S], FP32, tag="big")
                for wi in range(3):
                    nc.tensor.matmul(warmA_ps[:, :],
                                     c16_sb[:, C_ID:C_ID + 128],
                                     c16_sb[:, C_PE:C_PE + 4 * S],
                                     start=True, stop=True)

                # currT[d%128, kd, (b s)] = transpose(gather) + peT, in bf16
                currT_sb = actsp.tile([128, KD, B_LOC * S], BF16)
                for b in range(B_LOC):
                    tp_ps = ps_st.tile([128, KD, 128], BF16, tag="st")
                    for kd in range(KD):
                        nc.tensor.transpose(tp_ps[:, kd, :],
                                            g_all[:, b, kd * 128:(kd + 1) * 128],
                                            c16_sb[:, C_ID:C_ID + 128])
                    nc.vector.tensor_add(
                        out=currT_sb[:, :, b * S:(b + 1) * S],
                        in0=tp_ps[:, :, :],
                        in1=peT_sb[:, :, :],
                    )

                # projections: QT/KT [j%128, kj, (b,s)] bf16 with bias, streamed
                # per batch-pair so the first pair starts before gathers b2/b3
                QT_sb = actsp.tile([128, KD, B_LOC * S], BF16)
                KT_sb = actsp.tile([128, KD, B_LOC * S], BF16)
                th_sb = actsp.tile([128, B_LOC, D], BF16)  # tanh(attn) [s, b, j]
                V_sb = actsp.tile([128, B_LOC, D], BF16)
                p_tiles = {}
                HBS = 2 * S
                for bh in range(2):
                    bsl = slice(bh * HBS, (bh + 1) * HBS)
                    for kj in range(KD):
                        q_ps = ps_proj.tile([128, HBS], FP32, tag="big")
                        for kd in range(KD):
                            nc.tensor.matmul(q_ps[:, :],
                                             w4_sb[:, 0, kd, kj * 128:(kj + 1) * 128],
                                             currT_sb[:, kd, bsl],
                                             start=(kd == 0), stop=(kd == KD - 1))
                        nc.vector.tensor_scalar_add(QT_sb[:, kj, bsl], q_ps[:, :],
                                                    c32_sb[:, F_BQ + kj:F_BQ + kj + 1])
                        k_ps = ps_proj.tile([128, HBS], FP32, tag="big")
                        for kd in range(KD):
                            nc.tensor.matmul(k_ps[:, :],
                                             w4_sb[:, 1, kd, kj * 128:(kj + 1) * 128],
                                             currT_sb[:, kd, bsl],
                                             start=(kd == 0), stop=(kd == KD - 1))
                        nc.vector.tensor_scalar_add(KT_sb[:, kj, bsl], k_ps[:, :],
                                                    c32_sb[:, F_BK + kj:F_BK + kj + 1])
                    # S^T + exp for this batch pair immediately: these 8 exps on
                    # Act overlap the next pair's QK matmuls on PE
                    for b in (2 * bh, 2 * bh + 1):
                        for half in range(2):
                            st_ps = ps_st.tile([128, 4 * S], FP32, tag="st")
                            for hh in range(4):  # head = hh*2 + half
                                qs = QT_sb[half * 64:(half + 1) * 64, hh, b * S:(b + 1) * S]
                                ks = KT_sb[half * 64:(half + 1) * 64, hh, b * S:(b + 1) * S]
                                nc.tensor.matmul(st_ps[:, hh * S:(hh + 1) * S], ks, qs,
                                                 start=True, stop=True)
                            p_sb = php.tile([128, 4 * S], BF16, tag="p")
                            nc.scalar.activation(p_sb[:, :], st_ps[:, :], ACTF.Exp,
                                                 bias=shatt_sb[:, :1])
                            p_tiles[(b, half)] = p_sb
                # per batch: V projection (overlaps the exps on Act), rowsums via
                # N=1 matmuls, AV, per-head 1/rowsum rescale, tanh
                for b in range(B_LOC):
                    v_ps = ps_proj.tile([128, D], FP32, tag="big")
                    for kd in range(KD):
                        nc.tensor.matmul(v_ps[:, :],
                                         currT_sb[:, kd, b * S:(b + 1) * S],
                                         w4_sb[:, 2, kd, :],
                                         start=(kd == 0), stop=(kd == KD - 1))
                    nc.vector.tensor_add(out=V_sb[:, b, :], in0=v_ps[:, :], in1=bvb_sb[:, :])
                    rs_ps = ps_rs.tile([128, H], FP32, tag="rs")
                    av_ps = ps_av.tile([128, D], FP32, tag="av")
                    last_av = None
                    for half in range(2):
                        for hh in range(4):
                            h = hh * 2 + half
                            nc.tensor.matmul(rs_ps[:, h:h + 1],
                                             p_tiles[(b, half)][:, hh * S:(hh + 1) * S],
                                             ones_sb[:, :],
                                             start=True, stop=True)
                            last_av = nc.tensor.matmul(
                                av_ps[:, h * DH:(h + 1) * DH],
                                p_tiles[(b, half)][:, hh * S:(hh + 1) * S],
                                V_sb[:, b, h * DH:(h + 1) * DH],
                                start=True, stop=True)
                    rec_sb = smallp.tile([128, H], FP32, tag="rec")
                    nc.vector.reciprocal(rec_sb[:, :], rs_ps[:, :])
                    # 1/rowsum rescale as one broadcast mult (rec stride-0 over
                    # dh); the bank has 8 matmul writers and this is a full-bank
                    # read, so the dep helper pins the final drain
                    att_sb = php.tile([128, D], BF16, tag="att")
                    op = nc.vector.tensor_mul(
                        out=att_sb[:, :].rearrange("p (h x) -> p h x", h=H),
                        in0=av_ps[:, :].rearrange("p (h x) -> p h x", h=H),
                        in1=rec_sb[:, :].rearrange("p (h one) -> p h one", one=1)
                            .to_broadcast([128, H, DH]))
                    add_dep_helper(op.ins, last_av.ins,
                                   reason="att bank read after all AV writes")
                    nc.scalar.activation(th_sb[:, b, :], att_sb[:, :], ACTF.Tanh)

                # select mask positions (transposed): thselT [d%128, kd, i_loc] bf16
                thsel_sb = actsp.tile([128, KD, I_LOC], BF16)
                for kd in range(KD):
                    ts_ps = ps_st.tile([128, I_LOC], FP32, tag="st")
                    last_ts = None
                    for b in range(B_LOC):
                        last_ts = nc.tensor.matmul(ts_ps[:, b * NM:(b + 1) * NM],
                                                   th_sb[:, b, kd * 128:(kd + 1) * 128],
                                                   sel_sb[:, b, :],
                                                   start=True, stop=True)
                    op = nc.vector.tensor_copy(out=thsel_sb[:, kd, :], in_=ts_ps[:, :])
                    add_dep_helper(op.ins, last_ts.ins,
                                   reason="ts bank read after all sel writes")
                # t2 projection -> G_localT [d, i_loc] bf16 -> DRAM for AllGather
                gt_sb = actsp.tile([128, KD, I_LOC], BF16)
                for mj in range(KD):
                    g_ps = ps_proj.tile([128, I_LOC], FP32, tag="big")
                    for kd in range(KD):
                        nc.tensor.matmul(g_ps[:, :],
                                         w4_sb[:, 3, kd, mj * 128:(mj + 1) * 128],
                                         thsel_sb[:, kd, :],
                                         start=(kd == 0), stop=(kd == KD - 1))
                    nc.vector.tensor_scalar_add(gt_sb[:, mj, :], g_ps[:, :],
                                                c32_sb[:, F_T2B + mj:F_T2B + mj + 1])
                nc.sync.dma_start(out=ag_g_in[:, :].rearrange("(mj p) i -> p mj i", p=128),
                                  in_=gt_sb[:, :, :])

                # ---- AllGather G ----
                if sim_local:
                    agg_i = nc.sync.dma_start(
                        out=ag_g_out[:, :].rearrange("(c d) i -> c d i", c=N_CORES),
                        in_=ag_g_in[:, :].rearrange("(one d) i -> one d i", one=1)
                            .to_broadcast([N_CORES, D, I_LOC]))
                else:
                    agg_i = nc.gpsimd.collective_compute(
                        "AllGather", mybir.AluOpType.bypass,
                        replica_groups=[list(range(N_CORES))],
                        ins=[ag_g_in[:, :].opt()], outs=[ag_g_out[:, :].opt()],
                    )
                ag_g_view = ag_g_out[:, :].rearrange("(c kd p) i -> p kd c i", p=128, kd=KD)
                rb_is = []
                for kd in range(KD):
                    rb = nc.sync.dma_start(
                        out=GT_sb[:, kd, :].rearrange("p (c i) -> p c i", c=N_CORES),
                        in_=ag_g_view[:, kd, :, :],
                    )
                    rb_is.append(rb)

            # ================= Phase B: scores, exp, sums, sub, writeout =========
            # row-group-major: group mi's AllGather + subtract + output DMA overlap
            # groups mi+1..3's matmuls
            with (
                tc.tile_pool(name="ps_sc", bufs=8, space="PSUM") as ps_sc,
            ):
                warm_ps = ps_sc.tile([128, VCH], FP32, tag="sc")
                for wi in range(10):
                    wm = nc.tensor.matmul(warm_ps[:, :],
                                          c16_sb[:, C_ID:C_ID + 128],
                                          candT_sb[:, 0, 0:VCH],
                                          start=True, stop=True)
                    if wi >= 4:
                        add_dep_helper(wm.ins, agg_i.ins,
                                       reason="pe ramp warm-up spans AllGather")

                def epilogue(mi):
                    # stot readback -> lnS (fast-log + 1 Newton step via Exp)
                    # -> subtract -> quarter writeout DMAs.
                    # Emitted AFTER group mi+1's exps/copies so the AllGather
                    # wait never head-of-line-blocks the in-order engine queues.
                    last = mi == KD - 1
                    nread = 2 * N_CORES if last else N_CORES
                    stot_sb = smallp.tile([128, 2 * N_CORES], FP32, tag="stot")
                    nc.sync.dma_start(
                        out=stot_sb[:, 0:N_CORES],
                        in_=ag_s_out[mi][:, 0].rearrange("(c p) -> p c", p=128))
                    if last:
                        nc.sync.dma_start(
                            out=stot_sb[:, N_CORES:],
                            in_=ag_s_out[KD][:, 0].rearrange("(c p) -> p c", p=128))
                    stl_sb = smallp.tile([128, 3], FP32, tag="stl")
                    nc.vector.reduce_sum(stl_sb[:, 0:1], stot_sb[:, 0:nread],
                                         axis=mybir.AxisListType.X)
                    # y0 = fast-log(S); lnS30 = y0 + S*exp(-y0) - 1 + SH_SC
                    nc.vector.tensor_scalar(
                        out=stl_sb[:, 1:2], in0=stl_sb[:, 0:1].bitcast(INT32),
                        scalar1=FL_K1, scalar2=FL_K2, op0=ALU.mult, op1=ALU.add)
                    ey_sb = smallp.tile([128, 1], FP32, tag="ey")
                    nc.scalar.activation(ey_sb[:, :], stl_sb[:, 1:2], ACTF.Exp,
                                         scale=cm1_sb[:, :1])
                    nc.vector.tensor_mul(out=stl_sb[:, 2:3], in0=stl_sb[:, 0:1],
                                          in1=ey_sb[:, :])
                    nc.vector.tensor_add(out=stl_sb[:, 2:3], in0=stl_sb[:, 2:3],
                                         in1=stl_sb[:, 1:2])
                    # cc = FL_K2 + SH_SC - lnS30  (lnS30 = y1 - 1 + SH_SC)
                    cc_sb = smallp.tile([128, 1], FP32, tag="cc")
                    nc.vector.tensor_scalar(
                        out=cc_sb[:, :], in0=stl_sb[:, 2:3],
                        scalar1=-1.0, scalar2=FL_K2 + 1.0, op0=ALU.mult, op1=ALU.add)
                    QV = VSH // 4
                    for v in range(NCH):
                        sl = pexp_sb[:, mi, v * VCH:(v + 1) * VCH]
                        nc.vector.tensor_scalar(
                            out=sl, in0=sl.bitcast(INT16),
                            scalar1=FL_K1B, scalar2=cc_sb[:, :1],
                            op0=ALU.mult, op1=ALU.add)
                    for qq in range(4):
                        nc.sync.dma_start(
                            out=out.ap()[mi * 128:(mi + 1) * 128,
                                         qq * QV:(qq + 1) * QV],
                            in_=pexp_sb[:, mi, qq * QV:(qq + 1) * QV],
                        )

                def launch_ag(slot, src_ap):
                    nc.sync.dma_start(out=ag_s_in[slot][:, :], in_=src_ap)
                    if sim_local:
                        nc.sync.dma_start(
                            out=ag_s_out[slot][:, :].rearrange("(c i) one -> c i one", c=N_CORES),
                            in_=ag_s_in[slot][:, :].rearrange("(one i) x -> one i x", one=1)
                                .to_broadcast([N_CORES, 128, 1]))
                    else:
                        nc.gpsimd.collective_compute(
                            "AllGather", mybir.AluOpType.bypass,
                            replica_groups=[list(range(N_CORES))],
                            ins=[ag_s_in[slot][:, :].opt()], outs=[ag_s_out[slot][:, :].opt()],
                        )

                for mi in range(KD):
                    last = mi == KD - 1
                    for v in range(NCH):
                        sc_ps = ps_sc.tile([128, VCH], FP32, tag="sc")
                        for kd in range(KD):
                            nc.tensor.matmul(sc_ps[:, :],
                                             GT_sb[:, kd, mi * 128:(mi + 1) * 128],
                                             candT_sb[:, kd, v * VCH:(v + 1) * VCH],
                                             start=(kd == 0), stop=(kd == KD - 1))
                        if v % 2 == 0 or (last and v == NCH - 1):
                            nc.scalar.activation(pexp_sb[:, mi, v * VCH:(v + 1) * VCH],
                                                 sc_ps[:, :],
                                                 ACTF.Exp, bias=shsc_sb[:, :1],
                                                 accum_out=sums_sb[:, mi, v:v + 1])
                        else:
                            nc.scalar.activation(pexp_sb[:, mi, v * VCH:(v + 1) * VCH],
                                                 sc_ps[:, :],
                                                 ACTF.Exp, bias=shsc_sb[:, :1])
                            nc.vector.reduce_sum(sums_sb[:, mi, v:v + 1],
                                                 pexp_sb[:, mi, v * VCH:(v + 1) * VCH],
                                                 axis=mybir.AxisListType.X)
                        if last and v == NCH - 2:
                            # last group: AllGather chunks 0..8 early (hides
                            # under chunk 9); chunk 9's sum goes in a second,
                            # concurrent AllGather right after its accum lands
                            sl_sb = smallp.tile([128, 1], FP32, tag="sl")
                            nc.vector.reduce_sum(sl_sb[:, :], sums_sb[:, mi, 0:NCH - 1],
                                                 axis=mybir.AxisListType.X)
                            nc.vector.tensor_sub(out=sl_sb[:, :], in0=sl_sb[:, :],
                                                 in1=c32_sb[:, F_CORR:F_CORR + 1])
                            launch_ag(mi, sl_sb[:, :])
                    if not last:
                        sl_sb = smallp.tile([128, 1], FP32, tag="sl")
                        nc.vector.reduce_sum(sl_sb[:, :], sums_sb[:, mi, :],
                                             axis=mybir.AxisListType.X)
                        nc.vector.tensor_sub(out=sl_sb[:, :], in0=sl_sb[:, :],
                                             in1=c32_sb[:, F_CORR:F_CORR + 1])
                        launch_ag(mi, sl_sb[:, :])
                        if mi >= 1:
                            epilogue(mi - 1)
                    else:
                        launch_ag(KD, sums_sb[:, mi, NCH - 1:NCH])
                        epilogue(mi - 1)
                epilogue(KD - 1)
    nc.compile()
    return nc


_NC_CACHE = None


def _get_nc():
    global _NC_CACHE
    if _NC_CACHE is None:
        _NC_CACHE = build()
    return _NC_CACHE


def prepare_in_maps(inputs):
    emb = np.asarray(inputs["emb"], dtype=np.float32)
    embb = np.ascontiguousarray(emb.astype(NPBF))
    mask_curr = np.asarray(inputs["mask_curr_traj_grid"]).astype(np.int32)
    mask_pos = np.asarray(inputs["mask_pos"]).astype(np.int32)
    w4 = np.stack([
        np.asarray(inputs["c_wq"], dtype=np.float32).T,
        np.asarray(inputs["c_wk"], dtype=np.float32).T,
        np.asarray(inputs["c_wv"], dtype=np.float32).T,
        np.asarray(inputs["t2_w"], dtype=np.float32).T,
    ]).astype(NPBF)
    bq = np.asarray(inputs["c_bq"], dtype=np.float32)
    bk = np.asarray(inputs["c_bk"], dtype=np.float32)
    bv = np.asarray(inputs["c_bv"], dtype=np.float32)
    t2b = np.asarray(inputs["t2_b"], dtype=np.float32)
    peT = _positional_embedding(D, S).T  # [D, S]

    candTb = np.ascontiguousarray(emb[2:].T.astype(NPBF))  # [D, VOCAB]

    # bf16 const blob
    c16 = np.zeros((128, C16), dtype=NPBF)
    c16[:, C_PE:C_SEL] = peT.reshape(KD, 128, S).transpose(1, 0, 2).reshape(128, KD * S)
    c16[:, C_ONE] = 1.0
    c16[:, C_BVB:C_BVB + D] = np.broadcast_to(bv, (128, D))
    c16[:, C_ID:C_ID + 128] = np.eye(128, dtype=NPBF)
    # fp32 const blob (core-independent part)
    c32 = np.zeros((128, F32), dtype=np.float32)
    c32[:, F_BQ:F_BQ + KD] = bq.reshape(KD, 128).T
    c32[:, F_BK:F_BK + KD] = bk.reshape(KD, 128).T
    c32[:, F_T2B:F_T2B + KD] = t2b.reshape(KD, 128).T

    in_maps = []
    for c in range(N_CORES):
        lo = c * VSH
        hi = min((c + 1) * VSH, VOCAB)
        shard = np.zeros((D, VSH), dtype=NPBF)
        shard[:, : hi - lo] = candTb[:, lo:hi]
        n_inv = VSH - (hi - lo)
        c32_c = c32.copy()
        c32_c[:, F_CORR] = n_inv * math.exp(-SH_SC)
        mp = mask_pos[c * B_LOC:(c + 1) * B_LOC]  # [B_LOC, NM]
        c16_c = c16.copy()
        sel_c = np.zeros((S, B_LOC, NM), dtype=NPBF)
        for b in range(B_LOC):
            sel_c[mp[b], b, np.arange(NM)] = 1.0
        c16_c[:, C_SEL:C_ONE] = sel_c.reshape(S, B_LOC * NM)
        in_maps.append(dict(
            embb=embb,
            candT=np.ascontiguousarray(shard),
            idx=np.ascontiguousarray(mask_curr[c * B_LOC:(c + 1) * B_LOC].reshape(-1)),
            w4=w4, cst16=c16_c, cst32=c32_c,
        ))
    return in_maps


def assemble_output(results):
    parts = []
    for c in range(N_CORES):
        lo = c * VSH
        hi = min((c + 1) * VSH, VOCAB)
        parts.append(results[c]["out"][:, : hi - lo].astype(np.float32))
    return np.ascontiguousarray(np.concatenate(parts, axis=1))


def kernel(**inputs):
    nc = _get_nc()
    in_maps = prepare_in_maps(inputs)
    res = run_bass_kernel_spmd(nc, in_maps, core_ids=list(range(N_CORES)))
    return assemble_output(res.results)


# revision 44
# speedup vs baseline: 1.0673x; 1.0007x over previous
"""Trainium2 Bass kernel for nn_AttnMoveModel (dense_transformer).

Strategy (8 NeuronCores):
  - Only the `curr` path of the reference affects the output (hist self-attn and
    cross-attn results are dead), so only that path is computed.
  - Attention is data-parallel over batch (4 of 32 batches per core).
  - The vocab projection (gathered @ emb[2:].T) is tensor-parallel, column-split
    over the vocab (5120 padded columns per core), with an AllGather of the
    gathered activations before it and per-row-group AllGathers of exp-sums for
    the log_softmax denominator (so the subtract+writeout of row group i
    pipelines behind row group i+1's matmuls).
  - All matmul inputs are bf16 (rel err ~2e-3 vs 2e-2 budget): 1 cycle/row on
    the PE array for every shape and half the HBM traffic of fp32.
  - Attention computes S^T (keys on partitions) so the exp output IS P^T in
    SBUF: no P transposes / PSUM copies; softmax row sums come from free N=1
    matmuls against a ones vector; 1/rowsum is folded in post-AV.
  - The full candidate shard (bf16) is preloaded into SBUF during attention
    (ordered behind the gathers on the DMA engines), so the score phase runs
    back-to-back matmuls with no input DMA.
  - log(sum) is computed with a fast-log bit trick + one Newton step using Exp
    (err ~5e-4), so the kernel never touches the Ln activation table: the whole
    kernel uses one table (exp+tanh), avoiding 1.3us table swaps per use.
  - The score phase persists exp(sc-30) (the softmax numerators, bf16) instead
    of raw scores: GPSIMD cannot read PSUM, and this removes all PSUM->SBUF
    copies and subtracts. The epilogue recovers log-probs in one DVE op per
    chunk: out = bitcast_i16(p)*(ln2/128) + (K2 + 30 - lnS).

Host-side prep (inside kernel()): shard indices/batches, pre-transpose weights
and the emb vocab shard into bf16, build one-hot selection matrices from
mask_pos, positional-encoding table.
"""
import contextlib
import math
import sys

sys.path.insert(0, "/opt/trn_rl_repo")

import numpy as np
import ml_dtypes

import concourse.bass as bass
import concourse.mybir as mybir
import concourse.tile as tile
from concourse.tile import add_dep_helper
from concourse import bacc
from concourse.bass_utils import run_bass_kernel_spmd

FP32 = mybir.dt.float32
BF16 = mybir.dt.bfloat16
INT32 = mybir.dt.int32
INT16 = mybir.dt.int16
ACTF = mybir.ActivationFunctionType
ALU = mybir.AluOpType
NPBF = ml_dtypes.bfloat16

N_CORES = 8
B, S, D, H, DH = 32, 128, 512, 8, 64
B_LOC = B // N_CORES              # 4 batches per core
NM = 16                           # mask positions per batch
I_LOC = B_LOC * NM                # 64 gathered rows per core
I_TOT = B * NM                    # 512 gathered rows total
GRID = 40000
VOCAB = GRID - 2                  # 39998 candidate rows
VSH = 5120                        # padded vocab shard per core (8*5120 >= VOCAB)
VCH = 512                         # vocab chunk (matmul N)
NCH = VSH // VCH                  # 10 chunks
KD = D // 128                     # 4 contraction tiles
SH_ATT = 15.0                     # exp shift for attention softmax
SH_SC = 30.0                      # exp shift for final log_softmax
# fast-log: ln(x) ~= bitcast_i32(x)*K1 + K2, |err| <= 0.030; one Newton step
# with exp brings it to ~5e-4
FL_K1 = math.log(2.0) / (1 << 23)
FL_K2 = -(127.0 - 0.0430) * math.log(2.0)
FL_K1B = math.log(2.0) / 128          # bf16 variant (bits in the high 16)

# bf16 const blob layout (columns)
C_PE = 0                          # peT [128, KD*S]    (kd, s)
C_SEL = C_PE + KD * S             # sel [128, B_LOC*NM] (b, m); partition = s
C_ONE = C_SEL + B_LOC * NM        # ones [128, 1]
C_BVB = C_ONE + 1                 # bv broadcast [128, D]
C_ID = C_BVB + D                  # identity [128, 128] for PE transposes
C16 = C_ID + 128
# fp32 const blob layout (columns)
F_BQ = 0                          # bq [128, KD]
F_BK = F_BQ + KD
F_T2B = F_BK + KD
F_CORR = F_T2B + KD               # padding correction [128, 1]
F32 = F_CORR + 1


def _positional_embedding(d_model, max_len):
    pe = np.zeros((max_len, d_model), dtype=np.float32)
    position = np.arange(max_len, dtype=np.float32)[:, None]
    div_term = np.exp(np.arange(0, d_model, 2, dtype=np.float32) * -(math.log(10000.0) / d_model))
    pe[:, 0::2] = np.sin(position * div_term)
    pe[:, 1::2] = np.cos(position * div_term)
    return pe


def build(sim_local=False):
    nc = bacc.Bacc("TRN2", target_bir_lowering=False, debug=False, num_devices=N_CORES)

    # ---- I/O ----
    embb = nc.dram_tensor("embb", [GRID, D], BF16, kind="ExternalInput")
    candT = nc.dram_tensor("candT", [D, VSH], BF16, kind="ExternalInput")
    idx = nc.dram_tensor("idx", [B_LOC * S], INT32, kind="ExternalInput")
    w4 = nc.dram_tensor("w4", [4, D, D], BF16, kind="ExternalInput")  # wqt wkt wvt t2wt
    cst16 = nc.dram_tensor("cst16", [128, C16], BF16, kind="ExternalInput")
    cst32 = nc.dram_tensor("cst32", [128, F32], FP32, kind="ExternalInput")
    out = nc.dram_tensor("out", [I_TOT, VSH], BF16, kind="ExternalOutput")

    with tile.TileContext(nc) as tc:
        with (
            tc.tile_pool(name="const", bufs=1) as constp,
            tc.tile_pool(name="persist", bufs=1) as persp,
            tc.tile_pool(name="small", bufs=4) as smallp,
            tc.tile_pool(name="dram", bufs=1, space="DRAM") as dramp,
        ):
            # ================= constant loads (order matters on the DMA dev) ====
            idx_sb = constp.tile([S, B_LOC], INT32)
            nc.sync.dma_start(out=idx_sb[:, :],
                              in_=idx.ap().rearrange("(b s) -> s b", s=S))
            c16_sb = constp.tile([128, C16], BF16)
            nc.sync.dma_start(out=c16_sb[:, :], in_=cst16.ap())
            w4_sb = constp.tile([128, 4, KD, D], BF16)  # [d%128, which, kd, j]
            w4v = w4.ap().rearrange("w (kd p) j -> p w kd j", p=128)
            for w in range(2):  # wq, wk first (attention critical path)
                nc.sync.dma_start(out=w4_sb[:, w, :, :], in_=w4v[:, w, :, :])
            c32_sb = constp.tile([128, F32], FP32)
            nc.sync.dma_start(out=c32_sb[:, :], in_=cst32.ap())
            peT_sb = c16_sb[:, C_PE:C_SEL].rearrange("p (kd s) -> p kd s", kd=KD)
            sel_sb = c16_sb[:, C_SEL:C_ONE].rearrange("p (b m) -> p b m", b=B_LOC)
            ones_sb = c16_sb[:, C_ONE:C_ONE + 1]
            bvb_sb = c16_sb[:, C_BVB:C_BVB + D]

            shatt_sb = constp.tile([128, 1], FP32)
            nc.vector.memset(shatt_sb[:, :], -SH_ATT)
            shsc_sb = constp.tile([128, 1], FP32)
            nc.vector.memset(shsc_sb[:, :], -SH_SC)
            cm1_sb = constp.tile([128, 1], FP32)
            nc.vector.memset(cm1_sb[:, :], -1.0)

            # persistent across phases
            GT_sb = persp.tile([128, KD, I_TOT], BF16)    # [d%128, kd, i]
            candT_sb = persp.tile([128, KD, VSH], BF16)   # full candidate shard
            sums_sb = persp.tile([128, KD, NCH], FP32)    # per-chunk exp sums
            pexp_sb = persp.tile([128, KD, VSH], BF16)    # exp(sc-30) numerators
            lnS_sb = persp.tile([128, KD], FP32)

            ag_g_in = dramp.tile([D, I_LOC], BF16)
            ag_g_out = dramp.tile([N_CORES * D, I_LOC], BF16, addr_space="Shared")
            ag_s_in = [dramp.tile([128, 1], FP32, name=f"ag_s_in{m}")
                       for m in range(KD + 1)]
            ag_s_out = [dramp.tile([N_CORES * 128, 1], FP32, addr_space="Shared",
                                   name=f"ag_s_out{m}")
                        for m in range(KD + 1)]

            # ================= Phase A: gather + self-attention =================
            with (
                tc.tile_pool(name="acts", bufs=1) as actsp,
                tc.tile_pool(name="gath", bufs=1) as gathp,
                tc.tile_pool(name="ph", bufs=8) as php,
                tc.tile_pool(name="ps_proj", bufs=2, space="PSUM") as ps_proj,
                tc.tile_pool(name="ps_st", bufs=3, space="PSUM") as ps_st,
                tc.tile_pool(name="ps_rs", bufs=1, space="PSUM") as ps_rs,
                tc.tile_pool(name="ps_av", bufs=2, space="PSUM") as ps_av,
            ):
                # per-batch indirect gathers (multi-column offset APs gather
                # with a different layout than assumed — verified broken on HW)
                with tc.high_priority():
                    g_all = gathp.tile([S, B_LOC, D], BF16, tag="gather")
                    for b in range(B_LOC):
                        gi = nc.gpsimd.indirect_dma_start(
                            out=g_all[:, b, :], out_offset=None,
                            in_=embb.ap(),
                            in_offset=bass.IndirectOffsetOnAxis(ap=idx_sb[:, b:b + 1], axis=0),
                        )

                # wv/t2w and the candidate shard stream behind the gather on
                # the serialized DMA device (they are needed later)
                for w in range(2, 4):
                    wd = nc.sync.dma_start(out=w4_sb[:, w, :, :], in_=w4v[:, w, :, :])
                    add_dep_helper(wd.ins, gi.ins,
                                   reason="wv/t2w stream behind the emb gather")
                cv = candT.ap().rearrange("(kd p) n -> p kd n", p=128)
                HV = VSH // 2
                for hh in range(2):
                    cd = nc.sync.dma_start(
                        out=candT_sb[:, :, hh * HV:(hh + 1) * HV],
                        in_=cv[:, :, hh * HV:(hh + 1) * HV])
                    add_dep_helper(cd.ins, gi.ins,
                                   reason="candT streams behind the emb gather")

                # PE pstate warm-up during the gather wait: the transposes and
                # projections then start at speed (scratch bank, values unused)
                warmA_ps = ps_proj.tile([128, 4 * S], FP32, tag="big")
                for wi in range(3):
                    nc.tensor.matmul(warmA_ps[:, :],
                                     c16_sb[:, C_ID:C_ID + 128],
                                     c16_sb[:, C_PE:C_PE + 4 * S],
                                     start=True, stop=True)

                # currT[d%128, kd, (b s)] = transpose(gather) + peT, in bf16
                currT_sb = actsp.tile([128, KD, B_LOC * S], BF16)
                for b in range(B_LOC):
                    tp_ps = ps_st.tile([128, KD, 128], BF16, tag="st")
                    for kd in range(KD):
                        nc.tensor.transpose(tp_ps[:, kd, :],
                                            g_all[:, b, kd * 128:(kd + 1) * 128],
                                            c16_sb[:, C_ID:C_ID + 128])
                    nc.vector.tensor_add(
                        out=currT_sb[:, :, b * S:(b + 1) * S],
                        in0=tp_ps[:, :, :],
                        in1=peT_sb[:, :, :],
                    )

                # projections: QT/KT [j%128, kj, (b,s)] bf16 with bias, streamed
                # per batch-pair so the first pair starts before gathers b2/b3
                QT_sb = actsp.tile([128, KD, B_LOC * S], BF16)
                KT_sb = actsp.tile([128, KD, B_LOC * S], BF16)
                th_sb = actsp.tile([128, B_LOC, D], BF16)  # tanh(attn) [s, b, j]
                V_sb = actsp.tile([128, B_LOC, D], BF16)
                p_tiles = {}
                HBS = 2 * S
                for bh in range(2):
                    bsl = slice(bh * HBS, (bh + 1) * HBS)
                    for kj in range(KD):
                        q_ps = ps_proj.tile([128, HBS], FP32, tag="big")
                        for kd in range(KD):
                            nc.tensor.matmul(q_ps[:, :],
                                             w4_sb[:, 0, kd, kj * 128:(kj + 1) * 128],
                                             currT_sb[:, kd, bsl],
                                             start=(kd == 0), stop=(kd == KD - 1))
                        nc.vector.tensor_scalar_add(QT_sb[:, kj, bsl], q_ps[:, :],
                                                    c32_sb[:, F_BQ + kj:F_BQ + kj + 1])
                        k_ps = ps_proj.tile([128, HBS], FP32, tag="big")
                        for kd in range(KD):
                            nc.tensor.matmul(k_ps[:, :],
                                             w4_sb[:, 1, kd, kj * 128:(kj + 1) * 128],
                                             currT_sb[:, kd, bsl],
                                             start=(kd == 0), stop=(kd == KD - 1))
                        nc.vector.tensor_scalar_add(KT_sb[:, kj, bsl], k_ps[:, :],
                                                    c32_sb[:, F_BK + kj:F_BK + kj + 1])
                    # S^T + exp for this batch pair immediately: these 8 exps on
                    # Act overlap the next pair's QK matmuls on PE
                    for b in (2 * bh, 2 * bh + 1):
                        for half in range(2):
                            st_ps = ps_st.tile([128, 4 * S], FP32, tag="st")
                            for hh in range(4):  # head = hh*2 + half
                                qs = QT_sb[half * 64:(half + 1) * 64, hh, b * S:(b + 1) * S]
                                ks = KT_sb[half * 64:(half + 1) * 64, hh, b * S:(b + 1) * S]
                                nc.tensor.matmul(st_ps[:, hh * S:(hh + 1) * S], ks, qs,
                                                 start=True, stop=True)
                            p_sb = php.tile([128, 4 * S], BF16, tag="p")
                            nc.scalar.activation(p_sb[:, :], st_ps[:, :], ACTF.Exp,
                                                 bias=shatt_sb[:, :1])
                            p_tiles[(b, half)] = p_sb
                # per batch: V projection (overlaps the exps on Act), rowsums via
                # N=1 matmuls, AV, per-head 1/rowsum rescale, tanh
                for b in range(B_LOC):
                    v_ps = ps_proj.tile([128, D], FP32, tag="big")
                    for kd in range(KD):
                        nc.tensor.matmul(v_ps[:, :],
                                         currT_sb[:, kd, b * S:(b + 1) * S],
                                         w4_sb[:, 2, kd, :],
                                         start=(kd == 0), stop=(kd == KD - 1))
                    nc.vector.tensor_add(out=V_sb[:, b, :], in0=v_ps[:, :], in1=bvb_sb[:, :])
                    rs_ps = ps_rs.tile([128, H], FP32, tag="rs")
                    av_ps = ps_av.tile([128, D], FP32, tag="av")
                    last_av = None
                    for half in range(2):
                        for hh in range(4):
                            h = hh * 2 + half
                            nc.tensor.matmul(rs_ps[:, h:h + 1],
                                             p_tiles[(b, half)][:, hh * S:(hh + 1) * S],
                                             ones_sb[:, :],
                                             start=True, stop=True)
                            last_av = nc.tensor.matmul(
                                av_ps[:, h * DH:(h + 1) * DH],
                                p_tiles[(b, half)][:, hh * S:(hh + 1) * S],
                                V_sb[:, b, h * DH:(h + 1) * DH],
                                start=True, stop=True)
                    rec_sb = smallp.tile([128, H], FP32, tag="rec")
                    nc.vector.reciprocal(rec_sb[:, :], rs_ps[:, :])
                    # 1/rowsum rescale as one broadcast mult (rec stride-0 over
                    # dh); the bank has 8 matmul writers and this is a full-bank
                    # read, so the dep helper pins the final drain
                    att_sb = php.tile([128, D], BF16, tag="att")
                    op = nc.vector.tensor_mul(
                        out=att_sb[:, :].rearrange("p (h x) -> p h x", h=H),
                        in0=av_ps[:, :].rearrange("p (h x) -> p h x", h=H),
                        in1=rec_sb[:, :].rearrange("p (h one) -> p h one", one=1)
                            .to_broadcast([128, H, DH]))
                    add_dep_helper(op.ins, last_av.ins,
                                   reason="att bank read after all AV writes")
                    nc.scalar.activation(th_sb[:, b, :], att_sb[:, :], ACTF.Tanh)

                # select mask positions (transposed): thselT [d%128, kd, i_loc] bf16
                thsel_sb = actsp.tile([128, KD, I_LOC], BF16)
                for kd in range(KD):
                    ts_ps = ps_st.tile([128, I_LOC], FP32, tag="st")
                    last_ts = None
                    for b in range(B_LOC):
                        last_ts = nc.tensor.matmul(ts_ps[:, b * NM:(b + 1) * NM],
                                                   th_sb[:, b, kd * 128:(kd + 1) * 128],
                                                   sel_sb[:, b, :],
                                                   start=True, stop=True)
                    op = nc.vector.tensor_copy(out=thsel_sb[:, kd, :], in_=ts_ps[:, :])
                    add_dep_helper(op.ins, last_ts.ins,
                                   reason="ts bank read after all sel writes")
                # t2 projection -> G_localT [d, i_loc] bf16 -> DRAM for AllGather
                gt_sb = actsp.tile([128, KD, I_LOC], BF16)
                for mj in range(KD):
                    g_ps = ps_proj.tile([128, I_LOC], FP32, tag="big")
                    for kd in range(KD):
                        nc.tensor.matmul(g_ps[:, :],
                                         w4_sb[:, 3, kd, mj * 128:(mj + 1) * 128],
                                         thsel_sb[:, kd, :],
                                         start=(kd == 0), stop=(kd == KD - 1))
                    nc.vector.tensor_scalar_add(gt_sb[:, mj, :], g_ps[:, :],
                                                c32_sb[:, F_T2B + mj:F_T2B + mj + 1])
                nc.sync.dma_start(out=ag_g_in[:, :].rearrange("(mj p) i -> p mj i", p=128),
                                  in_=gt_sb[:, :, :])

                # ---- AllGather G ----
                if sim_local:
                    agg_i = nc.sync.dma_start(
                        out=ag_g_out[:, :].rearrange("(c d) i -> c d i", c=N_CORES),
                        in_=ag_g_in[:, :].rearrange("(one d) i -> one d i", one=1)
                            .to_broadcast([N_CORES, D, I_LOC]))
                else:
                    agg_i = nc.gpsimd.collective_compute(
                        "AllGather", mybir.AluOpType.bypass,
                        replica_groups=[list(range(N_CORES))],
                        ins=[ag_g_in[:, :].opt()], outs=[ag_g_out[:, :].opt()],
                    )
                ag_g_view = ag_g_out[:, :].rearrange("(c kd p) i -> p kd c i", p=128, kd=KD)
                rb_is = []
                for kd in range(KD):
                    rb = nc.sync.dma_start(
                        out=GT_sb[:, kd, :].rearrange("p (c i) -> p c i", c=N_CORES),
                        in_=ag_g_view[:, kd, :, :],
                    )
                    rb_is.append(rb)

            # ================= Phase B: scores, exp, sums, sub, writeout =========
            # row-group-major: group mi's AllGather + subtract + output DMA overlap
            # groups mi+1..3's matmuls
            with (
                tc.tile_pool(name="ps_sc", bufs=8, space="PSUM") as ps_sc,
            ):
                warm_ps = ps_sc.tile([128, VCH], FP32, tag="sc")
                for wi in range(10):
                    wm = nc.tensor.matmul(warm_ps[:, :],
                                          c16_sb[:, C_ID:C_ID + 128],
                                          candT_sb[:, 0, 0:VCH],
                                          start=True, stop=True)
                    if wi >= 4:
                        add_dep_helper(wm.ins, agg_i.ins,
                                       reason="pe ramp warm-up spans AllGather")

                def epilogue(mi):
                    # stot readback -> lnS (fast-log + 1 Newton step via Exp)
                    # -> subtract -> quarter writeout DMAs.
                    # Emitted AFTER group mi+1's exps/copies so the AllGather
                    # wait never head-of-line-blocks the in-order engine queues.
                    last = mi == KD - 1
                    nread = 2 * N_CORES if last else N_CORES
                    stot_sb = smallp.tile([128, 2 * N_CORES], FP32, tag="stot")
                    nc.sync.dma_start(
                        out=stot_sb[:, 0:N_CORES],
                        in_=ag_s_out[mi][:, 0].rearrange("(c p) -> p c", p=128))
                    if last:
                        nc.sync.dma_start(
                            out=stot_sb[:, N_CORES:],
                            in_=ag_s_out[KD][:, 0].rearrange("(c p) -> p c", p=128))
                    stl_sb = smallp.tile([128, 3], FP32, tag="stl")
                    nc.vector.reduce_sum(stl_sb[:, 0:1], stot_sb[:, 0:nread],
                                         axis=mybir.AxisListType.X)
                    # y0 = fast-log(S); lnS30 = y0 + S*exp(-y0) - 1 + SH_SC
                    nc.vector.tensor_scalar(
                        out=stl_sb[:, 1:2], in0=stl_sb[:, 0:1].bitcast(INT32),
                        scalar1=FL_K1, scalar2=FL_K2, op0=ALU.mult, op1=ALU.add)
                    ey_sb = smallp.tile([128, 1], FP32, tag="ey")
                    nc.scalar.activation(ey_sb[:, :], stl_sb[:, 1:2], ACTF.Exp,
                                         scale=cm1_sb[:, :1])
                    nc.vector.tensor_mul(out=stl_sb[:, 2:3], in0=stl_sb[:, 0:1],
                                          in1=ey_sb[:, :])
                    nc.vector.tensor_add(out=stl_sb[:, 2:3], in0=stl_sb[:, 2:3],
                                         in1=stl_sb[:, 1:2])
                    # cc = FL_K2 + SH_SC - lnS30  (lnS30 = y1 - 1 + SH_SC)
                    cc_sb = smallp.tile([128, 1], FP32, tag="cc")
                    nc.vector.tensor_scalar(
                        out=cc_sb[:, :], in0=stl_sb[:, 2:3],
                        scalar1=-1.0, scalar2=FL_K2 + 1.0, op0=ALU.mult, op1=ALU.add)
                    QV = VSH // 4
                    for v in range(NCH):
                        sl = pexp_sb[:, mi, v * VCH:(v + 1) * VCH]
                        nc.vector.tensor_scalar(
                            out=sl, in0=sl.bitcast(INT16),
                            scalar1=FL_K1B, scalar2=cc_sb[:, :1],
                            op0=ALU.mult, op1=ALU.add)
                    for qq in range(4):
                        nc.sync.dma_start(
                            out=out.ap()[mi * 128:(mi + 1) * 128,
                                         qq * QV:(qq + 1) * QV],
                            in_=pexp_sb[:, mi, qq * QV:(qq + 1) * QV],
                        )

                def launch_ag(slot, src_ap):
                    nc.sync.dma_start(out=ag_s_in[slot][:, :], in_=src_ap)
                    if sim_local:
                        nc.sync.dma_start(
                            out=ag_s_out[slot][:, :].rearrange("(c i) one -> c i one", c=N_CORES),
                            in_=ag_s_in[slot][:, :].rearrange("(one i) x -> one i x", one=1)
                                .to_broadcast([N_CORES, 128, 1]))
                    else:
                        nc.gpsimd.collective_compute(
                            "AllGather", mybir.AluOpType.bypass,
                            replica_groups=[list(range(N_CORES))],
                            ins=[ag_s_in[slot][:, :].opt()], outs=[ag_s_out[slot][:, :].opt()],
                        )

                for mi in range(KD):
                    last = mi == KD - 1
                    for v in range(NCH):
                        sc_ps = ps_sc.tile([128, VCH], FP32, tag="sc")
                        for kd in range(KD):
                            nc.tensor.matmul(sc_ps[:, :],
                                             GT_sb[:, kd, mi * 128:(mi + 1) * 128],
                                             candT_sb[:, kd, v * VCH:(v + 1) * VCH],
                                             start=(kd == 0), stop=(kd == KD - 1))
                        if v % 2 == 0 or (last and v == NCH - 1):
                            nc.scalar.activation(pexp_sb[:, mi, v * VCH:(v + 1) * VCH],
                                                 sc_ps[:, :],
                                                 ACTF.Exp, bias=shsc_sb[:, :1],
                                                 accum_out=sums_sb[:, mi, v:v + 1])
                        else:
                            nc.scalar.activation(pexp_sb[:, mi, v * VCH:(v + 1) * VCH],
                                                 sc_ps[:, :],
                                                 ACTF.Exp, bias=shsc_sb[:, :1])
                            nc.vector.reduce_sum(sums_sb[:, mi, v:v + 1],
                                                 pexp_sb[:, mi, v * VCH:(v + 1) * VCH],
                                                 axis=mybir.AxisListType.X)
                        if last and v == NCH - 2:
                            # last group: AllGather chunks 0..8 early (hides
                            # under chunk 9); chunk 9's sum goes in a second,
                            # concurrent AllGather right after its accum lands
                            sl_sb = smallp.tile([128, 1], FP32, tag="sl")
                            nc.vector.reduce_sum(sl_sb[:, :], sums_sb[:, mi, 0:NCH - 1],
                                                 axis=mybir.AxisListType.X)
                            nc.vector.tensor_sub(out=sl_sb[:, :], in0=sl_sb[:, :],
                                                 in1=c32_sb[:, F_CORR:F_CORR + 1])
                            launch_ag(mi, sl_sb[:, :])
                    if not last:
                        sl_sb = smallp.tile([128, 1], FP32, tag="sl")
                        nc.vector.reduce_sum(sl_sb[:, :], sums_sb[:, mi, :],
                                             axis=mybir.AxisListType.X)
                        nc.vector.tensor_sub(out=sl_sb[:, :], in0=sl_sb[:, :],
                                             in1=c32_sb[:, F_CORR:F_CORR + 1])
                        launch_ag(mi, sl_sb[:, :])
                        if mi >= 1:
                            epilogue(mi - 1)
                    else:
                        launch_ag(KD, sums_sb[:, mi, NCH - 1:NCH])
                        epilogue(mi - 1)
                epilogue(KD - 1)
    nc.compile()
    return nc


_NC_CACHE = None


def _get_nc():
    global _NC_CACHE
    if _NC_CACHE is None:
        _NC_CACHE = build()
    return _NC_CACHE


def prepare_in_maps(inputs):
    emb = np.asarray(inputs["emb"], dtype=np.float32)
    embb = np.ascontiguousarray(emb.astype(NPBF))
    mask_curr = np.asarray(inputs["mask_curr_traj_grid"]).astype(np.int32)
    mask_pos = np.asarray(inputs["mask_pos"]).astype(np.int32)
    w4 = np.stack([
        np.asarray(inputs["c_wq"], dtype=np.float32).T,
        np.asarray(inputs["c_wk"], dtype=np.float32).T,
        np.asarray(inputs["c_wv"], dtype=np.float32).T,
        np.asarray(inputs["t2_w"], dtype=np.float32).T,
    ]).astype(NPBF)
    bq = np.asarray(inputs["c_bq"], dtype=np.float32)
    bk = np.asarray(inputs["c_bk"], dtype=np.float32)
    bv = np.asarray(inputs["c_bv"], dtype=np.float32)
    t2b = np.asarray(inputs["t2_b"], dtype=np.float32)
    peT = _positional_embedding(D, S).T  # [D, S]

    candTb = np.ascontiguousarray(emb[2:].T.astype(NPBF))  # [D, VOCAB]

    # bf16 const blob
    c16 = np.zeros((128, C16), dtype=NPBF)
    c16[:, C_PE:C_SEL] = peT.reshape(KD, 128, S).transpose(1, 0, 2).reshape(128, KD * S)
    c16[:, C_ONE] = 1.0
    c16[:, C_BVB:C_BVB + D] = np.broadcast_to(bv, (128, D))
    c16[:, C_ID:C_ID + 128] = np.eye(128, dtype=NPBF)
    # fp32 const blob (core-independent part)
    c32 = np.zeros((128, F32), dtype=np.float32)
    c32[:, F_BQ:F_BQ + KD] = bq.reshape(KD, 128).T
    c32[:, F_BK:F_BK + KD] = bk.reshape(KD, 128).T
    c32[:, F_T2B:F_T2B + KD] = t2b.reshape(KD, 128).T

    in_maps = []
    for c in range(N_CORES):
        lo = c * VSH
        hi = min((c + 1) * VSH, VOCAB)
        shard = np.zeros((D, VSH), dtype=NPBF)
        shard[:, : hi - lo] = candTb[:, lo:hi]
        n_inv = VSH - (hi - lo)
        c32_c = c32.copy()
        c32_c[:, F_CORR] = n_inv * math.exp(-SH_SC)
        mp = mask_pos[c * B_LOC:(c + 1) * B_LOC]  # [B_LOC, NM]
        c16_c = c16.copy()
        sel_c = np.zeros((S, B_LOC, NM), dtype=NPBF)
        for b in range(B_LOC):
            sel_c[mp[b], b, np.arange(NM)] = 1.0
        c16_c[:, C_SEL:C_ONE] = sel_c.reshape(S, B_LOC * NM)
        in_maps.append(dict(
            embb=embb,
            candT=np.ascontiguousarray(shard),
            idx=np.ascontiguousarray(mask_curr[c * B_LOC:(c + 1) * B_LOC].reshape(-1)),
            w4=w4, cst16=c16_c, cst32=c32_c,
        ))
    return in_maps


def assemble_output(results):
    parts = []
    for c in range(N_CORES):
        lo = c * VSH
        hi = min((c + 1) * VSH, VOCAB)
        parts.append(results[c]["out"][:, : hi - lo].astype(np.float32))
    return np.ascontiguousarray(np.concatenate(parts, axis=1))


def kernel(**inputs):
    nc = _get_nc()
    in_maps = prepare_in_maps(inputs)
    res = run_bass_kernel_spmd(nc, in_maps, core_ids=list(range(N_CORES)))
    return assemble_output(res.results)


# revision 48
# speedup vs baseline: 1.0705x; 1.0030x over previous
"""Trainium2 Bass kernel for nn_AttnMoveModel (dense_transformer).

Strategy (8 NeuronCores):
  - Only the `curr` path of the reference affects the output (hist self-attn and
    cross-attn results are dead), so only that path is computed.
  - Attention is data-parallel over batch (4 of 32 batches per core).
  - The vocab projection (gathered @ emb[2:].T) is tensor-parallel, column-split
    over the vocab (5120 padded columns per core), with an AllGather of the
    gathered activations before it and per-row-group AllGathers of exp-sums for
    the log_softmax denominator (so the subtract+writeout of row group i
    pipelines behind row group i+1's matmuls).
  - All matmul inputs are bf16 (rel err ~2e-3 vs 2e-2 budget): 1 cycle/row on
    the PE array for every shape and half the HBM traffic of fp32.
  - Attention computes S^T (keys on partitions) so the exp output IS P^T in
    SBUF: no P transposes / PSUM copies; softmax row sums come from free N=1
    matmuls against a ones vector; 1/rowsum is folded in post-AV.
  - The full candidate shard (bf16) is preloaded into SBUF during attention
    (ordered behind the gathers on the DMA engines), so the score phase runs
    back-to-back matmuls with no input DMA.
  - log(sum) is computed with a fast-log bit trick + one Newton step using Exp
    (err ~5e-4), so the kernel never touches the Ln activation table: the whole
    kernel uses one table (exp+tanh), avoiding 1.3us table swaps per use.
  - The score phase persists exp(sc-30) (the softmax numerators, bf16) instead
    of raw scores: GPSIMD cannot read PSUM, and this removes all PSUM->SBUF
    copies and subtracts. The epilogue recovers log-probs in one DVE op per
    chunk: out = bitcast_i16(p)*(ln2/128) + (K2 + 30 - lnS).

Host-side prep (inside kernel()): shard indices/batches, pre-transpose weights
and the emb vocab shard into bf16, build one-hot selection matrices from
mask_pos, positional-encoding table.
"""
import contextlib
import math
import sys

sys.path.insert(0, "/opt/trn_rl_repo")

import numpy as np
import ml_dtypes

import concourse.bass as bass
import concourse.mybir as mybir
import concourse.tile as tile
from concourse.tile import add_dep_helper
from concourse import bacc
from concourse.bass_utils import run_bass_kernel_spmd

FP32 = mybir.dt.float32
BF16 = mybir.dt.bfloat16
INT32 = mybir.dt.int32
INT16 = mybir.dt.int16
ACTF = mybir.ActivationFunctionType
ALU = mybir.AluOpType
NPBF = ml_dtypes.bfloat16

N_CORES = 8
B, S, D, H, DH = 32, 128, 512, 8, 64
B_LOC = B // N_CORES              # 4 batches per core
NM = 16                           # mask positions per batch
I_LOC = B_LOC * NM                # 64 gathered rows per core
I_TOT = B * NM                    # 512 gathered rows total
GRID = 40000
VOCAB = GRID - 2                  # 39998 candidate rows
VSH = 5120                        # padded vocab shard per core (8*5120 >= VOCAB)
VCH = 512                         # vocab chunk (matmul N)
NCH = VSH // VCH                  # 10 chunks
KD = D // 128                     # 4 contraction tiles
SH_ATT = 15.0                     # exp shift for attention softmax
SH_SC = 30.0                      # exp shift for final log_softmax
# fast-log: ln(x) ~= bitcast_i32(x)*K1 + K2, |err| <= 0.030; one Newton step
# with exp brings it to ~5e-4
FL_K1 = math.log(2.0) / (1 << 23)
FL_K2 = -(127.0 - 0.0430) * math.log(2.0)
FL_K1B = math.log(2.0) / 128          # bf16 variant (bits in the high 16)

# bf16 const blob layout (columns)
C_PE = 0                          # peT [128, KD*S]    (kd, s)
C_SEL = C_PE + KD * S             # sel [128, B_LOC*NM] (b, m); partition = s
C_ONE = C_SEL + B_LOC * NM        # ones [128, 1]
C_BVB = C_ONE + 1                 # bv broadcast [128, D]
C_ID = C_BVB + D                  # identity [128, 128] for PE transposes
C16 = C_ID + 128
# fp32 const blob layout (columns)
F_BQ = 0                          # bq [128, KD]
F_BK = F_BQ + KD
F_T2B = F_BK + KD
F_CORR = F_T2B + KD               # padding correction [128, 1]
F32 = F_CORR + 1


def _positional_embedding(d_model, max_len):
    pe = np.zeros((max_len, d_model), dtype=np.float32)
    position = np.arange(max_len, dtype=np.float32)[:, None]
    div_term = np.exp(np.arange(0, d_model, 2, dtype=np.float32) * -(math.log(10000.0) / d_model))
    pe[:, 0::2] = np.sin(position * div_term)
    pe[:, 1::2] = np.cos(position * div_term)
    return pe


def build(sim_local=False):
    nc = bacc.Bacc("TRN2", target_bir_lowering=False, debug=False, num_devices=N_CORES)

    # ---- I/O ----
    embb = nc.dram_tensor("embb", [GRID, D], BF16, kind="ExternalInput")
    candT = nc.dram_tensor("candT", [D, VSH], BF16, kind="ExternalInput")
    idx = nc.dram_tensor("idx", [B_LOC * S], INT32, kind="ExternalInput")
    w4 = nc.dram_tensor("w4", [4, D, D], BF16, kind="ExternalInput")  # wqt wkt wvt t2wt
    cst16 = nc.dram_tensor("cst16", [128, C16], BF16, kind="ExternalInput")
    cst32 = nc.dram_tensor("cst32", [128, F32], FP32, kind="ExternalInput")
    out = nc.dram_tensor("out", [I_TOT, VSH], BF16, kind="ExternalOutput")

    with tile.TileContext(nc) as tc:
        with (
            tc.tile_pool(name="const", bufs=1) as constp,
            tc.tile_pool(name="persist", bufs=1) as persp,
            tc.tile_pool(name="small", bufs=4) as smallp,
            tc.tile_pool(name="dram", bufs=1, space="DRAM") as dramp,
        ):
            # ================= constant loads (order matters on the DMA dev) ====
            idx_sb = constp.tile([S, B_LOC], INT32)
            nc.sync.dma_start(out=idx_sb[:, :],
                              in_=idx.ap().rearrange("(b s) -> s b", s=S))
            c16_sb = constp.tile([128, C16], BF16)
            nc.sync.dma_start(out=c16_sb[:, :], in_=cst16.ap())
            w4_sb = constp.tile([128, 4, KD, D], BF16)  # [d%128, which, kd, j]
            w4v = w4.ap().rearrange("w (kd p) j -> p w kd j", p=128)
            for w in range(2):  # wq, wk first (attention critical path)
                nc.sync.dma_start(out=w4_sb[:, w, :, :], in_=w4v[:, w, :, :])
            c32_sb = constp.tile([128, F32], FP32)
            nc.sync.dma_start(out=c32_sb[:, :], in_=cst32.ap())
            peT_sb = c16_sb[:, C_PE:C_SEL].rearrange("p (kd s) -> p kd s", kd=KD)
            sel_sb = c16_sb[:, C_SEL:C_ONE].rearrange("p (b m) -> p b m", b=B_LOC)
            ones_sb = c16_sb[:, C_ONE:C_ONE + 1]
            bvb_sb = c16_sb[:, C_BVB:C_BVB + D]

            shatt_sb = constp.tile([128, 1], FP32)
            nc.vector.memset(shatt_sb[:, :], -SH_ATT)
            shsc_sb = constp.tile([128, 1], FP32)
            nc.vector.memset(shsc_sb[:, :], -SH_SC)
            cm1_sb = constp.tile([128, 1], FP32)
            nc.vector.memset(cm1_sb[:, :], -1.0)

            # persistent across phases
            GT_sb = persp.tile([128, KD, I_TOT], BF16)    # [d%128, kd, i]
            candT_sb = persp.tile([128, KD, VSH], BF16)   # full candidate shard
            sums_sb = persp.tile([128, KD, NCH], FP32)    # per-chunk exp sums
            pexp_sb = persp.tile([128, KD, VSH], BF16)    # exp(sc-30) numerators
            lnS_sb = persp.tile([128, KD], FP32)

            ag_g_in = dramp.tile([D, I_LOC], BF16)
            ag_g_out = dramp.tile([N_CORES * D, I_LOC], BF16, addr_space="Shared")
            ag_s_in = [dramp.tile([128, 1], FP32, name=f"ag_s_in{m}")
                       for m in range(KD + 1)]
            ag_s_out = [dramp.tile([N_CORES * 128, 1], FP32, addr_space="Shared",
                                   name=f"ag_s_out{m}")
                        for m in range(KD + 1)]

            # ================= Phase A: gather + self-attention =================
            with (
                tc.tile_pool(name="acts", bufs=1) as actsp,
                tc.tile_pool(name="gath", bufs=1) as gathp,
                tc.tile_pool(name="ph", bufs=8) as php,
                tc.tile_pool(name="ps_proj", bufs=2, space="PSUM") as ps_proj,
                tc.tile_pool(name="ps_st", bufs=3, space="PSUM") as ps_st,
                tc.tile_pool(name="ps_rs", bufs=1, space="PSUM") as ps_rs,
                tc.tile_pool(name="ps_av", bufs=2, space="PSUM") as ps_av,
            ):
                # per-batch indirect gathers (multi-column offset APs gather
                # with a different layout than assumed — verified broken on HW)
                with tc.high_priority():
                    g_all = gathp.tile([S, B_LOC, D], BF16, tag="gather")
                    for b in range(B_LOC):
                        gi = nc.gpsimd.indirect_dma_start(
                            out=g_all[:, b, :], out_offset=None,
                            in_=embb.ap(),
                            in_offset=bass.IndirectOffsetOnAxis(ap=idx_sb[:, b:b + 1], axis=0),
                        )

                # wv/t2w and the candidate shard stream behind the gather on
                # the serialized DMA device (they are needed later)
                for w in range(2, 4):
                    wd = nc.sync.dma_start(out=w4_sb[:, w, :, :], in_=w4v[:, w, :, :])
                    add_dep_helper(wd.ins, gi.ins,
                                   reason="wv/t2w stream behind the emb gather")
                cv = candT.ap().rearrange("(kd p) n -> p kd n", p=128)
                HV = VSH // 2
                for hh in range(2):
                    cd = nc.sync.dma_start(
                        out=candT_sb[:, :, hh * HV:(hh + 1) * HV],
                        in_=cv[:, :, hh * HV:(hh + 1) * HV])
                    add_dep_helper(cd.ins, gi.ins,
                                   reason="candT streams behind the emb gather")

                # PE pstate warm-up during the gather wait: the transposes and
                # projections then start at speed (scratch bank, values unused)
                warmA_ps = ps_proj.tile([128, 4 * S], FP32, tag="big")
                for wi in range(3):
                    nc.tensor.matmul(warmA_ps[:, :],
                                     c16_sb[:, C_ID:C_ID + 128],
                                     c16_sb[:, C_PE:C_PE + 4 * S],
                                     start=True, stop=True)

                # currT[d%128, kd, (b s)] = transpose(gather) + peT, in bf16
                currT_sb = actsp.tile([128, KD, B_LOC * S], BF16)
                for b in range(B_LOC):
                    tp_ps = ps_st.tile([128, KD, 128], BF16, tag="st")
                    for kd in range(KD):
                        nc.tensor.transpose(tp_ps[:, kd, :],
                                            g_all[:, b, kd * 128:(kd + 1) * 128],
                                            c16_sb[:, C_ID:C_ID + 128])
                    nc.vector.tensor_add(
                        out=currT_sb[:, :, b * S:(b + 1) * S],
                        in0=tp_ps[:, :, :],
                        in1=peT_sb[:, :, :],
                    )

                # projections: QT/KT [j%128, kj, (b,s)] bf16 with bias, streamed
                # per batch-pair so the first pair starts before gathers b2/b3
                QT_sb = actsp.tile([128, KD, B_LOC * S], BF16)
                KT_sb = actsp.tile([128, KD, B_LOC * S], BF16)
                th_sb = actsp.tile([128, B_LOC, D], BF16)  # tanh(attn) [s, b, j]
                V_sb = actsp.tile([128, B_LOC, D], BF16)
                p_tiles = {}
                HBS = 2 * S
                for bh in range(2):
                    bsl = slice(bh * HBS, (bh + 1) * HBS)
                    for kj in range(KD):
                        q_ps = ps_proj.tile([128, HBS], FP32, tag="big")
                        for kd in range(KD):
                            nc.tensor.matmul(q_ps[:, :],
                                             w4_sb[:, 0, kd, kj * 128:(kj + 1) * 128],
                                             currT_sb[:, kd, bsl],
                                             start=(kd == 0), stop=(kd == KD - 1))
                        nc.vector.tensor_scalar_add(QT_sb[:, kj, bsl], q_ps[:, :],
                                                    c32_sb[:, F_BQ + kj:F_BQ + kj + 1])
                        k_ps = ps_proj.tile([128, HBS], FP32, tag="big")
                        for kd in range(KD):
                            nc.tensor.matmul(k_ps[:, :],
                                             w4_sb[:, 1, kd, kj * 128:(kj + 1) * 128],
                                             currT_sb[:, kd, bsl],
                                             start=(kd == 0), stop=(kd == KD - 1))
                        nc.vector.tensor_scalar_add(KT_sb[:, kj, bsl], k_ps[:, :],
                                                    c32_sb[:, F_BK + kj:F_BK + kj + 1])
                    # S^T + exp for this batch pair immediately: these 8 exps on
                    # Act overlap the next pair's QK matmuls on PE
                    for b in (2 * bh, 2 * bh + 1):
                        for half in range(2):
                            st_ps = ps_st.tile([128, 4 * S], FP32, tag="st")
                            for hh in range(4):  # head = hh*2 + half
                                qs = QT_sb[half * 64:(half + 1) * 64, hh, b * S:(b + 1) * S]
                                ks = KT_sb[half * 64:(half + 1) * 64, hh, b * S:(b + 1) * S]
                                nc.tensor.matmul(st_ps[:, hh * S:(hh + 1) * S], ks, qs,
                                                 start=True, stop=True)
                            p_sb = php.tile([128, 4 * S], BF16, tag="p")
                            nc.scalar.activation(p_sb[:, :], st_ps[:, :], ACTF.Exp,
                                                 bias=shatt_sb[:, :1])
                            p_tiles[(b, half)] = p_sb
                # per batch: V projection (overlaps the exps on Act), rowsums via
                # N=1 matmuls, AV, per-head 1/rowsum rescale, tanh
                for b in range(B_LOC):
                    v_ps = ps_proj.tile([128, D], FP32, tag="big")
                    for kd in range(KD):
                        nc.tensor.matmul(v_ps[:, :],
                                         currT_sb[:, kd, b * S:(b + 1) * S],
                                         w4_sb[:, 2, kd, :],
                                         start=(kd == 0), stop=(kd == KD - 1))
                    nc.vector.tensor_add(out=V_sb[:, b, :], in0=v_ps[:, :], in1=bvb_sb[:, :])
                    rs_ps = ps_rs.tile([128, H], FP32, tag="rs")
                    av_ps = ps_av.tile([128, D], FP32, tag="av")
                    last_av = None
                    for half in range(2):
                        for hh in range(4):
                            h = hh * 2 + half
                            nc.tensor.matmul(rs_ps[:, h:h + 1],
                                             p_tiles[(b, half)][:, hh * S:(hh + 1) * S],
                                             ones_sb[:, :],
                                             start=True, stop=True)
                            last_av = nc.tensor.matmul(
                                av_ps[:, h * DH:(h + 1) * DH],
                                p_tiles[(b, half)][:, hh * S:(hh + 1) * S],
                                V_sb[:, b, h * DH:(h + 1) * DH],
                                start=True, stop=True)
                    rec_sb = smallp.tile([128, H], FP32, tag="rec")
                    nc.vector.reciprocal(rec_sb[:, :], rs_ps[:, :])
                    # 1/rowsum rescale as one broadcast mult (rec stride-0 over
                    # dh); the bank has 8 matmul writers and this is a full-bank
                    # read, so the dep helper pins the final drain
                    att_sb = php.tile([128, D], BF16, tag="att")
                    op = nc.vector.tensor_mul(
                        out=att_sb[:, :].rearrange("p (h x) -> p h x", h=H),
                        in0=av_ps[:, :].rearrange("p (h x) -> p h x", h=H),
                        in1=rec_sb[:, :].rearrange("p (h one) -> p h one", one=1)
                            .to_broadcast([128, H, DH]))
                    add_dep_helper(op.ins, last_av.ins,
                                   reason="att bank read after all AV writes")
                    nc.scalar.activation(th_sb[:, b, :], att_sb[:, :], ACTF.Tanh)

                # select mask positions (transposed): thselT [d%128, kd, i_loc] bf16
                thsel_sb = actsp.tile([128, KD, I_LOC], BF16)
                for kd in range(KD):
                    ts_ps = ps_st.tile([128, I_LOC], FP32, tag="st")
                    last_ts = None
                    for b in range(B_LOC):
                        last_ts = nc.tensor.matmul(ts_ps[:, b * NM:(b + 1) * NM],
                                                   th_sb[:, b, kd * 128:(kd + 1) * 128],
                                                   sel_sb[:, b, :],
                                                   start=True, stop=True)
                    op = nc.vector.tensor_copy(out=thsel_sb[:, kd, :], in_=ts_ps[:, :])
                    add_dep_helper(op.ins, last_ts.ins,
                                   reason="ts bank read after all sel writes")
                # t2 projection -> G_localT [d, i_loc] bf16 -> DRAM for AllGather
                gt_sb = actsp.tile([128, KD, I_LOC], BF16)
                for mj in range(KD):
                    g_ps = ps_proj.tile([128, I_LOC], FP32, tag="big")
                    for kd in range(KD):
                        nc.tensor.matmul(g_ps[:, :],
                                         w4_sb[:, 3, kd, mj * 128:(mj + 1) * 128],
                                         thsel_sb[:, kd, :],
                                         start=(kd == 0), stop=(kd == KD - 1))
                    nc.vector.tensor_scalar_add(gt_sb[:, mj, :], g_ps[:, :],
                                                c32_sb[:, F_T2B + mj:F_T2B + mj + 1])
                nc.sync.dma_start(out=ag_g_in[:, :].rearrange("(mj p) i -> p mj i", p=128),
                                  in_=gt_sb[:, :, :])

                # ---- AllGather G ----
                if sim_local:
                    agg_i = nc.sync.dma_start(
                        out=ag_g_out[:, :].rearrange("(c d) i -> c d i", c=N_CORES),
                        in_=ag_g_in[:, :].rearrange("(one d) i -> one d i", one=1)
                            .to_broadcast([N_CORES, D, I_LOC]))
                else:
                    agg_i = nc.gpsimd.collective_compute(
                        "AllGather", mybir.AluOpType.bypass,
                        replica_groups=[list(range(N_CORES))],
                        ins=[ag_g_in[:, :].opt()], outs=[ag_g_out[:, :].opt()],
                    )
                ag_g_view = ag_g_out[:, :].rearrange("(c kd p) i -> p kd c i", p=128, kd=KD)
                rb_is = []
                for kd in range(KD):
                    rb = nc.sync.dma_start(
                        out=GT_sb[:, kd, :].rearrange("p (c i) -> p c i", c=N_CORES),
                        in_=ag_g_view[:, kd, :, :],
                    )
                    rb_is.append(rb)

            # ================= Phase B: scores, exp, sums, sub, writeout =========
            # row-group-major: group mi's AllGather + subtract + output DMA overlap
            # groups mi+1..3's matmuls
            with (
                tc.tile_pool(name="ps_sc", bufs=8, space="PSUM") as ps_sc,
            ):
                warm_ps = ps_sc.tile([128, VCH], FP32, tag="sc")
                for wi in range(10):
                    wm = nc.tensor.matmul(warm_ps[:, :],
                                          c16_sb[:, C_ID:C_ID + 128],
                                          candT_sb[:, 0, 0:VCH],
                                          start=True, stop=True)
                    if wi >= 4:
                        add_dep_helper(wm.ins, agg_i.ins,
                                       reason="pe ramp warm-up spans AllGather")

                def epilogue(mi):
                    # stot readback -> lnS (fast-log + 1 Newton step via Exp)
                    # -> subtract -> quarter writeout DMAs.
                    # Emitted AFTER group mi+1's exps/copies so the AllGather
                    # wait never head-of-line-blocks the in-order engine queues.
                    last = mi == KD - 1
                    nread = 2 * N_CORES if last else N_CORES
                    stot_sb = smallp.tile([128, 2 * N_CORES], FP32, tag="stot")
                    nc.sync.dma_start(
                        out=stot_sb[:, 0:N_CORES],
                        in_=ag_s_out[mi][:, 0].rearrange("(c p) -> p c", p=128))
                    if last:
                        nc.sync.dma_start(
                            out=stot_sb[:, N_CORES:],
                            in_=ag_s_out[KD][:, 0].rearrange("(c p) -> p c", p=128))
                    stl_sb = smallp.tile([128, 3], FP32, tag="stl")
                    nc.vector.reduce_sum(stl_sb[:, 0:1], stot_sb[:, 0:nread],
                                         axis=mybir.AxisListType.X)
                    # lnS via linear fastlog only (|err|<=0.03, ~1e-3 norm-rel):
                    # skipping the Newton step removes an Act exp + two DVE ops
                    # and their cross-engine semaphore hops from the tail chain.
                    # cc = FL_K2 + SH_SC - lnS30 = FL_K2 - (I*K1 + FL_K2)
                    cc_sb = smallp.tile([128, 1], FP32, tag="cc")
                    nc.vector.tensor_scalar(
                        out=cc_sb[:, :], in0=stl_sb[:, 0:1].bitcast(INT32),
                        scalar1=-FL_K1, scalar2=0.0, op0=ALU.mult, op1=ALU.add)
                    QV = VSH // 4
                    for v in range(NCH):
                        sl = pexp_sb[:, mi, v * VCH:(v + 1) * VCH]
                        nc.vector.tensor_scalar(
                            out=sl, in0=sl.bitcast(INT16),
                            scalar1=FL_K1B, scalar2=cc_sb[:, :1],
                            op0=ALU.mult, op1=ALU.add)
                    for qq in range(4):
                        nc.sync.dma_start(
                            out=out.ap()[mi * 128:(mi + 1) * 128,
                                         qq * QV:(qq + 1) * QV],
                            in_=pexp_sb[:, mi, qq * QV:(qq + 1) * QV],
                        )

                def launch_ag(slot, src_ap):
                    nc.sync.dma_start(out=ag_s_in[slot][:, :], in_=src_ap)
                    if sim_local:
                        nc.sync.dma_start(
                            out=ag_s_out[slot][:, :].rearrange("(c i) one -> c i one", c=N_CORES),
                            in_=ag_s_in[slot][:, :].rearrange("(one i) x -> one i x", one=1)
                                .to_broadcast([N_CORES, 128, 1]))
                    else:
                        nc.gpsimd.collective_compute(
                            "AllGather", mybir.AluOpType.bypass,
                            replica_groups=[list(range(N_CORES))],
                            ins=[ag_s_in[slot][:, :].opt()], outs=[ag_s_out[slot][:, :].opt()],
                        )

                for mi in range(KD):
                    last = mi == KD - 1
                    for v in range(NCH):
                        sc_ps = ps_sc.tile([128, VCH], FP32, tag="sc")
                        for kd in range(KD):
                            nc.tensor.matmul(sc_ps[:, :],
                                             GT_sb[:, kd, mi * 128:(mi + 1) * 128],
                                             candT_sb[:, kd, v * VCH:(v + 1) * VCH],
                                             start=(kd == 0), stop=(kd == KD - 1))
                        if v % 2 == 0 or (last and v == NCH - 1):
                            nc.scalar.activation(pexp_sb[:, mi, v * VCH:(v + 1) * VCH],
                                                 sc_ps[:, :],
                                                 ACTF.Exp, bias=shsc_sb[:, :1],
                                                 accum_out=sums_sb[:, mi, v:v + 1])
                        else:
                            nc.scalar.activation(pexp_sb[:, mi, v * VCH:(v + 1) * VCH],
                                                 sc_ps[:, :],
                                                 ACTF.Exp, bias=shsc_sb[:, :1])
                            nc.vector.reduce_sum(sums_sb[:, mi, v:v + 1],
                                                 pexp_sb[:, mi, v * VCH:(v + 1) * VCH],
                                                 axis=mybir.AxisListType.X)
                        if last and v == NCH - 2:
                            # last group: AllGather chunks 0..8 early (hides
                            # under chunk 9); chunk 9's sum goes in a second,
                            # concurrent AllGather right after its accum lands
                            sl_sb = smallp.tile([128, 1], FP32, tag="sl")
                            nc.vector.reduce_sum(sl_sb[:, :], sums_sb[:, mi, 0:NCH - 1],
                                                 axis=mybir.AxisListType.X)
                            nc.vector.tensor_sub(out=sl_sb[:, :], in0=sl_sb[:, :],
                                                 in1=c32_sb[:, F_CORR:F_CORR + 1])
                            launch_ag(mi, sl_sb[:, :])
                    if not last:
                        sl_sb = smallp.tile([128, 1], FP32, tag="sl")
                        nc.vector.reduce_sum(sl_sb[:, :], sums_sb[:, mi, :],
                                             axis=mybir.AxisListType.X)
                        nc.vector.tensor_sub(out=sl_sb[:, :], in0=sl_sb[:, :],
                                             in1=c32_sb[:, F_CORR:F_CORR + 1])
                        launch_ag(mi, sl_sb[:, :])
                        if mi >= 1:
                            epilogue(mi - 1)
                    else:
                        launch_ag(KD, sums_sb[:, mi, NCH - 1:NCH])
                        epilogue(mi - 1)
                epilogue(KD - 1)
    nc.compile()
    return nc


_NC_CACHE = None


def _get_nc():
    global _NC_CACHE
    if _NC_CACHE is None:
        _NC_CACHE = build()
    return _NC_CACHE


def prepare_in_maps(inputs):
    emb = np.asarray(inputs["emb"], dtype=np.float32)
    embb = np.ascontiguousarray(emb.astype(NPBF))
    mask_curr = np.asarray(inputs["mask_curr_traj_grid"]).astype(np.int32)
    mask_pos = np.asarray(inputs["mask_pos"]).astype(np.int32)
    w4 = np.stack([
        np.asarray(inputs["c_wq"], dtype=np.float32).T,
        np.asarray(inputs["c_wk"], dtype=np.float32).T,
        np.asarray(inputs["c_wv"], dtype=np.float32).T,
        np.asarray(inputs["t2_w"], dtype=np.float32).T,
    ]).astype(NPBF)
    bq = np.asarray(inputs["c_bq"], dtype=np.float32)
    bk = np.asarray(inputs["c_bk"], dtype=np.float32)
    bv = np.asarray(inputs["c_bv"], dtype=np.float32)
    t2b = np.asarray(inputs["t2_b"], dtype=np.float32)
    peT = _positional_embedding(D, S).T  # [D, S]

    candTb = np.ascontiguousarray(emb[2:].T.astype(NPBF))  # [D, VOCAB]

    # bf16 const blob
    c16 = np.zeros((128, C16), dtype=NPBF)
    c16[:, C_PE:C_SEL] = peT.reshape(KD, 128, S).transpose(1, 0, 2).reshape(128, KD * S)
    c16[:, C_ONE] = 1.0
    c16[:, C_BVB:C_BVB + D] = np.broadcast_to(bv, (128, D))
    c16[:, C_ID:C_ID + 128] = np.eye(128, dtype=NPBF)
    # fp32 const blob (core-independent part)
    c32 = np.zeros((128, F32), dtype=np.float32)
    c32[:, F_BQ:F_BQ + KD] = bq.reshape(KD, 128).T
    c32[:, F_BK:F_BK + KD] = bk.reshape(KD, 128).T
    c32[:, F_T2B:F_T2B + KD] = t2b.reshape(KD, 128).T

    in_maps = []
    for c in range(N_CORES):
        lo = c * VSH
        hi = min((c + 1) * VSH, VOCAB)
        shard = np.zeros((D, VSH), dtype=NPBF)
        shard[:, : hi - lo] = candTb[:, lo:hi]
        n_inv = VSH - (hi - lo)
        c32_c = c32.copy()
        c32_c[:, F_CORR] = n_inv * math.exp(-SH_SC)
        mp = mask_pos[c * B_LOC:(c + 1) * B_LOC]  # [B_LOC, NM]
        c16_c = c16.copy()
        sel_c = np.zeros((S, B_LOC, NM), dtype=NPBF)
        for b in range(B_LOC):
            sel_c[mp[b], b, np.arange(NM)] = 1.0
        c16_c[:, C_SEL:C_ONE] = sel_c.reshape(S, B_LOC * NM)
        in_maps.append(dict(
            embb=embb,
            candT=np.ascontiguousarray(shard),
            idx=np.ascontiguousarray(mask_curr[c * B_LOC:(c + 1) * B_LOC].reshape(-1)),
            w4=w4, cst16=c16_c, cst32=c32_c,
        ))
    return in_maps


def assemble_output(results):
    parts = []
    for c in range(N_CORES):
        lo = c * VSH
        hi = min((c + 1) * VSH, VOCAB)
        parts.append(results[c]["out"][:, : hi - lo].astype(np.float32))
    return np.ascontiguousarray(np.concatenate(parts, axis=1))


def kernel(**inputs):
    nc = _get_nc()
    in_maps = prepare_in_maps(inputs)
    res = run_bass_kernel_spmd(nc, in_maps, core_ids=list(range(N_CORES)))
    return assemble_output(res.results)
